# revision 36
# baseline (speedup 1.0000x reference)
"""Navier-Stokes momentum-residual loss on 8 Trainium2 NeuronCores.

Reference computes, per momentum component m in {z,y,x}:
    R_m = rho*(uz_c*d_dz(u_m) + uy_c*d_dy(u_m) + ux_c*d_dx(u_m))
          + d_dm(p) - MU*lap(u_m)
    loss = sum_m mean(R_m^2)   over the interior [2,158,158,158]

Sharding: 8 cores = (batch b in {0,1}) x (z-chunk zc in {0..3}).  Each core
gets a z-slab of 44 planes [4, 44, 162, 160] (z planes 40*zc .. 40*zc+43,
y padded 160->162, zero-padded out of range).

On-core layout: partition p = y_block*16 + z_loc (8 y-blocks of 20 interior
rows, 16 z-planes per supertile).  3 z-supertiles x 2 x-halves per core.
z-direction stencil terms are computed on the TensorEngine with banded
128x128 matrices (PSUM accumulation); y/x stencils on the VectorEngine via
free-dim AP offsets; squared residuals are summed by the ScalarEngine's
activation(Square, accum_out=...) with a per-partition z-validity mask.
Host sums the per-core [128, NSLOT] partials and divides by N.
"""

import numpy as np

import concourse.bass as bass
import concourse.tile as tile
from concourse import bacc, mybir
from concourse.bass_utils import run_bass_kernel_spmd

try:  # persistent XLA/NEFF compile cache across processes (best effort)
    import jax as _jax
    _jax.config.update("jax_compilation_cache_dir", "/tmp/jax_ns_cache")
    _jax.config.update("jax_persistent_cache_min_entry_size_bytes", -1)
    _jax.config.update("jax_persistent_cache_min_compile_time_secs", 0.0)
except Exception:
    pass

MU = 0.01
RHO = 1.0

# geometry
NZ_SLAB = 44          # z planes per core slab
NY_PAD = 162          # y rows (160 + 2 zero pad)
NX = 160
NSUP = 3              # z supertiles per core
ZSUP = 16             # z planes per supertile (14 interior)
ZINT = 14
NYB = 8               # y blocks
YROWS = 22            # input y rows per block (20 interior + 2 halo)
XTW = 82              # x columns per x-half tile
NSLOT = 6 * 3 * 6     # units * momenta * accum slots


def _band_matrices():
    """lhsT matrices for the z-direction banded matmuls.

    out[p, f] = sum_k lhsT[k, p] * rhs[k, f];  p = yblk*16 + z_loc.
    D:  0.5*(u[z+1] - u[z-1]);  VU: -MU*(u[z+1] + u[z-1]) + 6*MU*u
    (only emitted for interior z_loc 1..14; edge columns all-zero).
    """
    D = np.zeros((128, 128), dtype=np.float32)
    VU = np.zeros((128, 128), dtype=np.float32)
    for p in range(128):
        z = p % ZSUP
        if 1 <= z <= ZINT:
            D[p + 1, p] = 0.5
            D[p - 1, p] = -0.5
            VU[p, p] = 6.0 * MU
            VU[p + 1, p] = -MU
            VU[p - 1, p] = -MU
    return np.concatenate([D, VU], axis=1)  # [128, 256]


def _zmask(zc):
    """[3, 128] validity mask per supertile/partition for core z-chunk zc."""
    smax = min(40, 158 - 40 * zc)
    m = np.zeros((3, 128), dtype=np.float32)
    for k in range(3):
        for p in range(128):
            z = p % ZSUP
            s = 14 * k + z
            if 1 <= z <= ZINT and 1 <= s <= smax:
                m[k, p] = 1.0
    return m


def build_program():
    f32 = mybir.dt.float32
    nc = bacc.Bacc("TRN2", target_bir_lowering=False, debug=False,
                   num_devices=8)
    # pre-packed: [channel, supertile, partition(=yblk*16+z), y_row, x]
    slab = nc.declare_dram_parameter("slab", [4, NSUP, 128, YROWS, NX], f32,
                                     isOutput=False)
    dmats = nc.declare_dram_parameter("dmats", [128, 256], f32, isOutput=False)
    zmask = nc.declare_dram_parameter("zmask", [3, 128], f32, isOutput=False)
    out = nc.declare_dram_parameter("out", [128, NSLOT], f32, isOutput=True)

    AL = mybir.AluOpType
    SQ = mybir.ActivationFunctionType.Square

    with tile.TileContext(nc) as tc:
        with (
            tc.tile_pool(name="const", bufs=1) as cpool,
            tc.tile_pool(name="inp", bufs=2) as inpool,
            tc.tile_pool(name="tmp", bufs=1) as tpool,
            tc.tile_pool(name="ctmp", bufs=2) as ctpool,
            tc.tile_pool(name="psA", bufs=3, space=bass.MemorySpace.PSUM) as psa,
            tc.tile_pool(name="psV", bufs=3, space=bass.MemorySpace.PSUM) as psv,
        ):
            dm = cpool.tile([128, 256], f32, tag="dm")
            nc.sync.dma_start(dm[:], dmats[:])
            zm = cpool.tile([128, 3], f32, tag="zm")
            for k in range(3):
                nc.sync.dma_start(zm[:, k : k + 1], zmask[k, :][:, None])
            acc = cpool.tile([128, NSLOT], f32, tag="acc")
            nc.vector.memset(acc[:], 0.0)

            lhs_D = dm[:, 0:128]
            lhs_VU = dm[:, 128:256]

            unit = 0
            for k in range(3):
                for xh in range(2):
                    x0 = 0 if xh == 0 else 78
                    xo = 1 if xh == 0 else 3   # first out col within tile
                    xn = 80 if xh == 0 else 78  # out col count
                    U = []
                    for c in range(4):
                        t = inpool.tile([128, YROWS, XTW], f32, tag=f"U{c}")
                        nc.sync.dma_start(t[:], slab[c, k, :, :, x0 : x0 + XTW])
                        U.append(t)

                    def cen(c, r0=1, nr=20):
                        return U[c][:, r0 : r0 + nr, xo : xo + xn]

                    def yp(c):
                        return U[c][:, 2:22, xo : xo + xn]

                    def ym(c):
                        return U[c][:, 0:20, xo : xo + xn]

                    def xp(c):
                        return U[c][:, 1:21, xo + 1 : xo + 1 + xn]

                    def xm(c):
                        return U[c][:, 1:21, xo - 1 : xo - 1 + xn]

                    for m in range(3):
                        Dy = tpool.tile([128, 20, 80], f32, tag="dy")
                        nc.vector.tensor_tensor(Dy[:, :, :xn], yp(m), ym(m),
                                                op=AL.subtract)
                        Dx = tpool.tile([128, 20, 80], f32, tag="dx")
                        nc.vector.tensor_tensor(Dx[:, :, :xn], xp(m), xm(m),
                                                op=AL.subtract)
                        NYt = tpool.tile([128, 20, 80], f32, tag="ny")
                        nc.vector.tensor_tensor(NYt[:, :, :xn], yp(m), ym(m),
                                                op=AL.add)
                        NXt = tpool.tile([128, 20, 80], f32, tag="nx")
                        nc.vector.tensor_tensor(NXt[:, :, :xn], xp(m), xm(m),
                                                op=AL.add)
                        T1 = tpool.tile([128, 20, 80], f32, tag="t1")
                        nc.vector.scalar_tensor_tensor(
                            T1[:, :, :xn], Dy[:, :, :xn], 0.5 * RHO, cen(1),
                            op0=AL.mult, op1=AL.mult)
                        T2 = tpool.tile([128, 20, 80], f32, tag="t2")
                        nc.vector.scalar_tensor_tensor(
                            T2[:, :, :xn], Dx[:, :, :xn], 0.5 * RHO, cen(2),
                            op0=AL.mult, op1=AL.mult)
                        S1 = tpool.tile([128, 20, 80], f32, tag="s1")
                        nc.vector.tensor_tensor(S1[:, :, :xn], T1[:, :, :xn],
                                                T2[:, :, :xn], op=AL.add)
                        NS = tpool.tile([128, 20, 80], f32, tag="ns")
                        nc.vector.tensor_tensor(NS[:, :, :xn], NYt[:, :, :xn],
                                                NXt[:, :, :xn], op=AL.add)
                        S2 = tpool.tile([128, 20, 80], f32, tag="s2")
                        nc.vector.scalar_tensor_tensor(
                            S2[:, :, :xn], NS[:, :, :xn], -MU, S1[:, :, :xn],
                            op0=AL.mult, op1=AL.add)
                        Dp = None
                        if m == 1:
                            Dp = tpool.tile([128, 20, 80], f32, tag="dp")
                            nc.vector.tensor_tensor(Dp[:, :, :xn], yp(3), ym(3),
                                                    op=AL.subtract)
                        elif m == 2:
                            Dp = tpool.tile([128, 20, 80], f32, tag="dp")
                            nc.vector.tensor_tensor(Dp[:, :, :xn], xp(3), xm(3),
                                                    op=AL.subtract)

                        for ch in range(4):
                            r0 = 1 + 5 * ch          # input-row of chunk start
                            L = 5 * xn
                            pA = psa.tile([128, 512], f32, tag="psA")
                            nc.tensor.matmul(pA[:, :L], lhs_D, cen(m, r0, 5),
                                             start=True, stop=True)
                            pV = psv.tile([128, 512], f32, tag="psV")
                            if m == 0:
                                nc.tensor.matmul(pV[:, :L], lhs_VU,
                                                 cen(0, r0, 5),
                                                 start=True, stop=False)
                                nc.tensor.matmul(pV[:, :L], lhs_D,
                                                 cen(3, r0, 5),
                                                 start=False, stop=True)
                            else:
                                nc.tensor.matmul(pV[:, :L], lhs_VU,
                                                 cen(m, r0, 5),
                                                 start=True, stop=True)

                            T3 = ctpool.tile([128, 5, 80], f32, tag="t3")
                            nc.vector.tensor_tensor(
                                T3[:, :, :xn], pA[:, :L], cen(0, r0, 5),
                                op=AL.mult)
                            S3 = ctpool.tile([128, 5, 80], f32, tag="s3")
                            nc.vector.tensor_tensor(
                                S3[:, :, :xn],
                                S2[:, 5 * ch : 5 * ch + 5, :xn],
                                T3[:, :, :xn], op=AL.add)
                            R = ctpool.tile([128, 5, 80], f32, tag="s4")
                            if m == 0:
                                nc.vector.tensor_tensor(
                                    R[:, :, :xn], S3[:, :, :xn], pV[:, :L],
                                    op=AL.add)
                            else:
                                S4 = ctpool.tile([128, 5, 80], f32, tag="s4b")
                                nc.vector.tensor_tensor(
                                    S4[:, :, :xn], S3[:, :, :xn], pV[:, :L],
                                    op=AL.add)
                                nc.vector.scalar_tensor_tensor(
                                    R[:, :, :xn],
                                    Dp[:, 5 * ch : 5 * ch + 5, :xn], 0.5,
                                    S4[:, :, :xn], op0=AL.mult, op1=AL.add)

                            sq = ctpool.tile([128, 5, 80], f32, tag="sq")
                            base = (unit * 3 + m) * 6
                            if ch < 3:
                                nc.scalar.activation(
                                    sq[:, :, :xn], R[:, :, :xn], SQ,
                                    scale=zm[:, k : k + 1],
                                    accum_out=acc[:, base + ch : base + ch + 1])
                            else:
                                # rows 16..20: y rows 159,160 are garbage on
                                # y-block 7 (partitions 112..127)
                                nc.scalar.activation(
                                    sq[0:96, :, :xn], R[0:96, :, :xn], SQ,
                                    scale=zm[0:96, k : k + 1],
                                    accum_out=acc[0:96, base + 3 : base + 4])
                                nc.scalar.activation(
                                    sq[96:128, 0:3, :xn], R[96:128, 0:3, :xn],
                                    SQ, scale=zm[96:128, k : k + 1],
                                    accum_out=acc[96:128, base + 4 : base + 5])
                                nc.scalar.activation(
                                    sq[96:112, 3:5, :xn], R[96:112, 3:5, :xn],
                                    SQ, scale=zm[96:112, k : k + 1],
                                    accum_out=acc[96:112, base + 5 : base + 6])
                    unit += 1

            nc.sync.dma_start(out[:], acc[:])
    nc.compile()
    return nc


def _band_matrices_v2():
    """bf16 lhsT matrices, packed [128, 5*128]: D, VU, IP(0.5I), IM(-0.5I),
    IMU(-MU*I)."""
    import ml_dtypes
    D = np.zeros((128, 128), dtype=np.float32)
    VU = np.zeros((128, 128), dtype=np.float32)
    for p in range(128):
        z = p % ZSUP
        if 1 <= z <= ZINT:
            D[p + 1, p] = 0.5
            D[p - 1, p] = -0.5
            VU[p, p] = 6.0 * MU
            VU[p + 1, p] = -MU
            VU[p - 1, p] = -MU
    eye = np.eye(128, dtype=np.float32)
    packed = np.concatenate([D, VU, 0.5 * eye, -0.5 * eye, -MU * eye], axis=1)
    return packed.astype(ml_dtypes.bfloat16)


def _band_matrices_v2():
    """bf16 lhsT matrices packed [128, 5*128]: D, VU, IP(0.5I), IM(-0.5I),
    IMU(-MU*I)."""
    import ml_dtypes
    D = np.zeros((128, 128), dtype=np.float32)
    VU = np.zeros((128, 128), dtype=np.float32)
    for p in range(128):
        z = p % ZSUP
        if 1 <= z <= ZINT:
            D[p + 1, p] = 0.5
            D[p - 1, p] = -0.5
            VU[p, p] = 6.0 * MU
            VU[p + 1, p] = -MU
            VU[p - 1, p] = -MU
    eye = np.eye(128, dtype=np.float32)
    packed = np.concatenate([D, VU, 0.5 * eye, -0.5 * eye, -MU * eye], axis=1)
    return packed.astype(ml_dtypes.bfloat16)


NSLOT2 = 3 * 3 * 8
NRC = 7  # row chunks: six of 3 rows + one of 2


def build_program_v2():
    """bf16 non-conservative variant, engine-balanced.

    Per momentum m the TensorEngine accumulates into PSUM:
      A_m = 0.5*dz(u_m)                                  [banded D]
      V_m = -MU*lap(u_m) + 0.5*d_m(p)   (z-lap banded VU + 6MU center;
            y/x neighbors via -MU*I shifted; dp via D band or +-0.5I shifts)
    The ScalarEngine copies A_m/V_m to bf16 SBUF and does the masked R^2
    accumulation; the VectorEngine (all-bf16 2x ops) does
      Dy, Dx subs; T1=A*uzc; T2=0.5*Dy*uyc; T3=0.5*Dx*uxc;
      S=T1+T2; S2=S+T3; R=S2+V.
    """
    f32 = mybir.dt.float32
    bf16 = mybir.dt.bfloat16
    nc = bacc.Bacc("TRN2", target_bir_lowering=False, debug=False,
                   num_devices=8)
    slab = nc.declare_dram_parameter("slab", [4, NSUP, 128, YROWS, NX], bf16,
                                     isOutput=False)
    dmats = nc.declare_dram_parameter("dmats", [128, 5 * 128], bf16,
                                      isOutput=False)
    zmask = nc.declare_dram_parameter("zmask", [3, 128], f32, isOutput=False)
    out = nc.declare_dram_parameter("out", [128, NSLOT2], f32, isOutput=True)

    AL = mybir.AluOpType
    SQ = mybir.ActivationFunctionType.Square

    with tile.TileContext(nc) as tc:
        with (
            tc.tile_pool(name="const", bufs=1) as cpool,
            tc.tile_pool(name="inp", bufs=2) as inpool,
            tc.tile_pool(name="ctmp", bufs=3) as ctpool,
            tc.tile_pool(name="psA", bufs=1, space=bass.MemorySpace.PSUM) as psa,
            tc.tile_pool(name="psV", bufs=1, space=bass.MemorySpace.PSUM) as psv,
        ):
            dm = cpool.tile([128, 5 * 128], bf16, tag="dm")
            nc.sync.dma_start(dm[:], dmats[:])
            zm = cpool.tile([128, 3], f32, tag="zm")
            for k in range(3):
                nc.sync.dma_start(zm[:, k : k + 1], zmask[k, :][:, None])
            acc = cpool.tile([128, NSLOT2], f32, tag="acc")
            nc.vector.memset(acc[:], 0.0)

            M_D = dm[:, 0:128]
            M_VU = dm[:, 128:256]
            M_IP = dm[:, 256:384]
            M_IM = dm[:, 384:512]
            M_IMU = dm[:, 512:640]

            for k in range(3):
                U = []
                for c in range(4):
                    t = inpool.tile([128, YROWS, NX], bf16, tag=f"U{c}")
                    nc.sync.dma_start(t[:], slab[c, k])
                    U.append(t)

                for rc in range(NRC):
                    r0 = 1 + 3 * rc
                    nr = 3 if rc < 6 else 2
                    NCH = nr * 158

                    def ap(c, dy=0, dx=0):
                        return U[c][:, r0 + dy : r0 + dy + nr,
                                    1 + dx : 159 + dx]

                    # ---- PE ----
                    A = [psa.tile([128, 512], f32, tag=f"psA{m}",
                                  name=f"A{m}_{k}_{rc}", bufs=1)
                         for m in range(3)]
                    V = [psv.tile([128, 512], f32, tag=f"psV{m}",
                                  name=f"V{m}_{k}_{rc}", bufs=1)
                         for m in range(3)]
                    # D group: A_m and dp_z
                    for m in range(3):
                        nc.tensor.matmul(A[m][:, :NCH], M_D, ap(m),
                                         start=True, stop=True)
                    nc.tensor.matmul(V[0][:, :NCH], M_D, ap(3),
                                     start=True, stop=False)
                    # VU group: z-lap + 6MU center
                    for m in range(3):
                        nc.tensor.matmul(V[m][:, :NCH], M_VU, ap(m),
                                         start=(m != 0), stop=False)
                    # IMU group: -MU * (y and x neighbors)
                    for m in range(3):
                        nc.tensor.matmul(V[m][:, :NCH], M_IMU, ap(m, dy=1),
                                         start=False, stop=False)
                        nc.tensor.matmul(V[m][:, :NCH], M_IMU, ap(m, dy=-1),
                                         start=False, stop=False)
                        nc.tensor.matmul(V[m][:, :NCH], M_IMU, ap(m, dx=1),
                                         start=False, stop=False)
                        nc.tensor.matmul(V[m][:, :NCH], M_IMU, ap(m, dx=-1),
                                         start=False, stop=(m == 0))
                    # IP/IM: dp_y, dp_x
                    nc.tensor.matmul(V[1][:, :NCH], M_IP, ap(3, dy=1),
                                     start=False, stop=False)
                    nc.tensor.matmul(V[2][:, :NCH], M_IP, ap(3, dx=1),
                                     start=False, stop=False)
                    nc.tensor.matmul(V[1][:, :NCH], M_IM, ap(3, dy=-1),
                                     start=False, stop=True)
                    nc.tensor.matmul(V[2][:, :NCH], M_IM, ap(3, dx=-1),
                                     start=False, stop=True)

                    # ---- ACT: copy PSUM -> bf16 SBUF ----
                    Ab, Vb = [], []
                    for m in range(3):
                        ab = ctpool.tile([128, 512], bf16, tag=f"ab{m}",
                                         name=f"Ab{m}_{k}_{rc}")
                        nc.scalar.copy(ab[:, :NCH], A[m][:, :NCH])
                        Ab.append(ab)
                        vb = ctpool.tile([128, 512], bf16, tag=f"vb{m}",
                                         name=f"Vb{m}_{k}_{rc}")
                        nc.scalar.copy(vb[:, :NCH], V[m][:, :NCH])
                        Vb.append(vb)

                    # ---- DVE (bf16) ----
                    for m in range(3):
                        Dy = ctpool.tile([128, 3, 158], bf16, tag="dy",
                                         name=f"Dy{m}_{k}_{rc}")
                        nc.vector.tensor_tensor(Dy[:, :nr, :], ap(m, dy=1),
                                                ap(m, dy=-1), op=AL.subtract)
                        Dx = ctpool.tile([128, 3, 158], bf16, tag="dx",
                                         name=f"Dx{m}_{k}_{rc}")
                        nc.vector.tensor_tensor(Dx[:, :nr, :], ap(m, dx=1),
                                                ap(m, dx=-1), op=AL.subtract)
                        T1 = ctpool.tile([128, 512], bf16, tag="t1",
                                         name=f"T1{m}_{k}_{rc}")
                        nc.vector.tensor_tensor(T1[:, :NCH], Ab[m][:, :NCH],
                                                ap(0), op=AL.mult)
                        T2 = ctpool.tile([128, 3, 158], bf16, tag="t2",
                                         name=f"T2{m}_{k}_{rc}")
                        nc.vector.scalar_tensor_tensor(
                            T2[:, :nr, :], Dy[:, :nr, :], 0.5 * RHO, ap(1),
                            op0=AL.mult, op1=AL.mult)
                        T3 = ctpool.tile([128, 3, 158], bf16, tag="t3",
                                         name=f"T3{m}_{k}_{rc}")
                        nc.vector.scalar_tensor_tensor(
                            T3[:, :nr, :], Dx[:, :nr, :], 0.5 * RHO, ap(2),
                            op0=AL.mult, op1=AL.mult)
                        S = ctpool.tile([128, 512], bf16, tag="s",
                                        name=f"S{m}_{k}_{rc}")
                        nc.vector.tensor_tensor(S[:, :NCH], T1[:, :NCH],
                                                T2[:, :nr, :], op=AL.add)
                        S2 = ctpool.tile([128, 512], bf16, tag="s2",
                                         name=f"S2{m}_{k}_{rc}")
                        nc.vector.tensor_tensor(S2[:, :NCH], S[:, :NCH],
                                                T3[:, :nr, :], op=AL.add)
                        R = ctpool.tile([128, 512], bf16, tag="r",
                                        name=f"R{m}_{k}_{rc}")
                        nc.vector.tensor_tensor(R[:, :NCH], S2[:, :NCH],
                                                Vb[m][:, :NCH], op=AL.add)

                        # ---- ACT: masked square-accumulate ----
                        sq = ctpool.tile([128, 512], bf16, tag="sq",
                                         name=f"sq{m}_{k}_{rc}")
                        base = (k * 3 + m) * 8
                        if rc < 6:
                            nc.scalar.activation(
                                sq[:, :NCH], R[:, :NCH], SQ,
                                scale=zm[:, k : k + 1],
                                accum_out=acc[:, base + rc : base + rc + 1])
                        else:
                            # rows 19,20: garbage on y-block 7 (parts 112-127)
                            nc.scalar.activation(
                                sq[0:96, :NCH], R[0:96, :NCH], SQ,
                                scale=zm[0:96, k : k + 1],
                                accum_out=acc[0:96, base + 6 : base + 7])
                            nc.scalar.activation(
                                sq[96:112, :NCH], R[96:112, :NCH], SQ,
                                scale=zm[96:112, k : k + 1],
                                accum_out=acc[96:112, base + 7 : base + 8])

            nc.sync.dma_start(out[:], acc[:])
    nc.compile()
    return nc


NSLOT3 = 3 * 3 * 2


def _amask(zc):
    """[128, NSLOT3] end-mask: slot = (k*3+m)*2 + j; j=0 rows 1-18, j=1 rows
    19-20 (garbage on y-block 7 = partitions 112..127)."""
    zm = _zmask(zc)  # [3, 128]
    m = np.zeros((128, NSLOT3), dtype=np.float32)
    for k in range(3):
        for mm in range(3):
            for j in range(2):
                s = (k * 3 + mm) * 2 + j
                col = zm[k].copy()
                if j == 1:
                    col[112:] = 0.0
                m[:, s] = col
    return m


def build_program_v3():
    """Like v2 but with full-supertile DVE ops (amortizes the per-op pipeline
    bubble), in-place tile reuse, tensor_scalar pre-scales instead of
    scalar_tensor_tensor, ACT squares without per-op masks, and one end-mask
    multiply on the [128, NSLOT3] partial sums."""
    f32 = mybir.dt.float32
    bf16 = mybir.dt.bfloat16
    nc = bacc.Bacc("TRN2", target_bir_lowering=False, debug=False,
                   num_devices=8)
    slab = nc.declare_dram_parameter("slab", [4, NSUP, 128, YROWS, NX], bf16,
                                     isOutput=False)
    dmats = nc.declare_dram_parameter("dmats", [128, 5 * 128], bf16,
                                      isOutput=False)
    amask = nc.declare_dram_parameter("amask", [128, NSLOT3], f32,
                                      isOutput=False)
    out = nc.declare_dram_parameter("out", [128, NSLOT3], f32, isOutput=True)

    AL = mybir.AluOpType
    SQ = mybir.ActivationFunctionType.Square

    with tile.TileContext(nc) as tc:
        with (
            tc.tile_pool(name="const", bufs=1) as cpool,
            tc.tile_pool(name="inp", bufs=2) as inpool,
            tc.tile_pool(name="fld", bufs=2) as fpool,
            tc.tile_pool(name="psA", bufs=1, space=bass.MemorySpace.PSUM) as psa,
            tc.tile_pool(name="psV", bufs=1, space=bass.MemorySpace.PSUM) as psv,
        ):
            dm = cpool.tile([128, 5 * 128], bf16, tag="dm")
            nc.sync.dma_start(dm[:], dmats[:])
            am = cpool.tile([128, NSLOT3], f32, tag="am")
            nc.sync.dma_start(am[:], amask[:])
            acc = cpool.tile([128, NSLOT3], f32, tag="acc")

            M_D = dm[:, 0:128]
            M_VU = dm[:, 128:256]
            M_IP = dm[:, 256:384]
            M_IM = dm[:, 384:512]
            M_IMU = dm[:, 512:640]

            for k in range(3):
                U = []
                for c in range(4):
                    t = inpool.tile([128, YROWS, NX], bf16, tag=f"U{c}")
                    nc.sync.dma_start(t[:], slab[c, k])
                    U.append(t)

                # pre-scaled center factors 0.5*uy, 0.5*ux (full interior)
                HUY = fpool.tile([128, 20, 158], bf16, tag="huy")
                nc.vector.tensor_scalar_mul(HUY[:], U[1][:, 1:21, 1:159],
                                            0.5 * RHO)
                HUX = fpool.tile([128, 20, 158], bf16, tag="hux")
                nc.vector.tensor_scalar_mul(HUX[:], U[2][:, 1:21, 1:159],
                                            0.5 * RHO)

                Ab, Vb = [], []
                for m in range(3):
                    ab = fpool.tile([128, 20, 158], bf16, tag=f"ab{m}",
                                    name=f"Ab{m}_{k}")
                    Ab.append(ab)
                    vb = fpool.tile([128, 20, 158], bf16, tag=f"vb{m}",
                                    name=f"Vb{m}_{k}")
                    Vb.append(vb)

                for rc in range(NRC):
                    r0 = 1 + 3 * rc
                    nr = 3 if rc < 6 else 2
                    NCH = nr * 158

                    def ap(c, dy=0, dx=0):
                        return U[c][:, r0 + dy : r0 + dy + nr,
                                    1 + dx : 159 + dx]

                    A = [psa.tile([128, 512], f32, tag=f"psA{m}",
                                  name=f"A{m}_{k}_{rc}")
                         for m in range(3)]
                    V = [psv.tile([128, 512], f32, tag=f"psV{m}",
                                  name=f"V{m}_{k}_{rc}")
                         for m in range(3)]
                    for m in range(3):
                        nc.tensor.matmul(A[m][:, :NCH], M_D, ap(m),
                                         start=True, stop=True)
                    nc.tensor.matmul(V[0][:, :NCH], M_D, ap(3),
                                     start=True, stop=False)
                    for m in range(3):
                        nc.tensor.matmul(V[m][:, :NCH], M_VU, ap(m),
                                         start=(m != 0), stop=False)
                    for m in range(3):
                        nc.tensor.matmul(V[m][:, :NCH], M_IMU, ap(m, dy=1),
                                         start=False, stop=False)
                        nc.tensor.matmul(V[m][:, :NCH], M_IMU, ap(m, dy=-1),
                                         start=False, stop=False)
                        nc.tensor.matmul(V[m][:, :NCH], M_IMU, ap(m, dx=1),
                                         start=False, stop=False)
                        nc.tensor.matmul(V[m][:, :NCH], M_IMU, ap(m, dx=-1),
                                         start=False, stop=(m == 0))
                    nc.tensor.matmul(V[1][:, :NCH], M_IP, ap(3, dy=1),
                                     start=False, stop=False)
                    nc.tensor.matmul(V[2][:, :NCH], M_IP, ap(3, dx=1),
                                     start=False, stop=False)
                    nc.tensor.matmul(V[1][:, :NCH], M_IM, ap(3, dy=-1),
                                     start=False, stop=True)
                    nc.tensor.matmul(V[2][:, :NCH], M_IM, ap(3, dx=-1),
                                     start=False, stop=True)

                    # ACT: drain PSUM chunks into the full-supertile tiles
                    rows = slice(r0 - 1, r0 - 1 + nr)
                    for m in range(3):
                        nc.scalar.copy(Ab[m][:, rows, :], A[m][:, :NCH])
                        nc.scalar.copy(Vb[m][:, rows, :], V[m][:, :NCH])

                # DVE: full-supertile assembly (in-place chains)
                for m in range(3):
                    Dy = fpool.tile([128, 20, 158], bf16, tag="dy",
                                    name=f"Dy{m}_{k}")
                    nc.vector.tensor_tensor(Dy[:], U[m][:, 2:22, 1:159],
                                            U[m][:, 0:20, 1:159],
                                            op=AL.subtract)
                    Dx = fpool.tile([128, 20, 158], bf16, tag="dx",
                                    name=f"Dx{m}_{k}")
                    nc.vector.tensor_tensor(Dx[:], U[m][:, 1:21, 2:160],
                                            U[m][:, 1:21, 0:158],
                                            op=AL.subtract)
                    # T1 = Ab*uzc (in place over Ab)
                    nc.vector.tensor_tensor(Ab[m][:], Ab[m][:],
                                            U[0][:, 1:21, 1:159], op=AL.mult)
                    # T2 = Dy*0.5uy (in place over Dy); T3 likewise
                    nc.vector.tensor_tensor(Dy[:], Dy[:], HUY[:], op=AL.mult)
                    nc.vector.tensor_tensor(Dx[:], Dx[:], HUX[:], op=AL.mult)
                    # S = T1+T2 -> Ab; S2 = S+T3 -> Ab; R = S2+Vb -> Vb
                    nc.vector.tensor_tensor(Ab[m][:], Ab[m][:], Dy[:],
                                            op=AL.add)
                    nc.vector.tensor_tensor(Ab[m][:], Ab[m][:], Dx[:],
                                            op=AL.add)
                    nc.vector.tensor_tensor(Vb[m][:], Ab[m][:], Vb[m][:],
                                            op=AL.add)

                    # ACT: plain square-accumulate, split rows 1-18 / 19-20
                    s = (k * 3 + m) * 2
                    sq = fpool.tile([128, 20, 158], bf16, tag="sq",
                                    name=f"sq{m}_{k}")
                    nc.scalar.activation(sq[:, 0:18, :], Vb[m][:, 0:18, :],
                                         SQ, accum_out=acc[:, s : s + 1])
                    nc.scalar.activation(sq[:, 18:20, :], Vb[m][:, 18:20, :],
                                         SQ, accum_out=acc[:, s + 1 : s + 2])

            # end-mask and ship
            nc.vector.tensor_tensor(acc[:], acc[:], am[:], op=AL.mult)
            nc.sync.dma_start(out[:], acc[:])
    nc.compile()
    return nc


def build_program_v5():
    """Like v2 but with full-supertile DVE ops (amortizes the per-op pipeline
    bubble), in-place tile reuse, tensor_scalar pre-scales instead of
    scalar_tensor_tensor, ACT squares without per-op masks, and one end-mask
    multiply on the [128, NSLOT3] partial sums."""
    f32 = mybir.dt.float32
    bf16 = mybir.dt.bfloat16
    nc = bacc.Bacc("TRN2", target_bir_lowering=False, debug=False,
                   num_devices=8)
    slab = nc.declare_dram_parameter("slab", [4, NSUP, 128, YROWS, NX], bf16,
                                     isOutput=False)
    dmats = nc.declare_dram_parameter("dmats", [128, 5 * 128], bf16,
                                      isOutput=False)
    amask = nc.declare_dram_parameter("amask", [128, NSLOT3], f32,
                                      isOutput=False)
    out = nc.declare_dram_parameter("out", [128, NSLOT3], f32, isOutput=True)

    AL = mybir.AluOpType
    SQ = mybir.ActivationFunctionType.Square

    with tile.TileContext(nc) as tc:
        with (
            tc.tile_pool(name="const", bufs=1) as cpool,
            tc.tile_pool(name="inp", bufs=2) as inpool,
            tc.tile_pool(name="fld", bufs=2) as fpool,
            tc.tile_pool(name="psA", bufs=1, space=bass.MemorySpace.PSUM) as psa,
            tc.tile_pool(name="psV", bufs=1, space=bass.MemorySpace.PSUM) as psv,
        ):
            dm = cpool.tile([128, 5 * 128], bf16, tag="dm")
            nc.sync.dma_start(dm[:], dmats[:])
            am = cpool.tile([128, NSLOT3], f32, tag="am")
            nc.sync.dma_start(am[:], amask[:])
            acc = cpool.tile([128, NSLOT3], f32, tag="acc")

            M_D = dm[:, 0:128]
            M_VU = dm[:, 128:256]
            M_IP = dm[:, 256:384]
            M_IM = dm[:, 384:512]
            M_IMU = dm[:, 512:640]

            for k in range(3):
                U = []
                for c in range(4):
                    t = inpool.tile([128, YROWS, NX], bf16, tag=f"U{c}")
                    nc.sync.dma_start(t[:], slab[c, k])
                    U.append(t)

                # pre-scaled center factors 0.5*uy, 0.5*ux (full interior)
                HUY = fpool.tile([128, 20, 158], bf16, tag="huy")
                nc.vector.tensor_scalar_mul(HUY[:], U[1][:, 1:21, 1:159],
                                            0.5 * RHO)
                HUX = fpool.tile([128, 20, 158], bf16, tag="hux")
                nc.vector.tensor_scalar_mul(HUX[:], U[2][:, 1:21, 1:159],
                                            0.5 * RHO)

                Ab, Vb = [], []
                for m in range(3):
                    ab = fpool.tile([128, 20, 158], bf16, tag=f"ab{m}",
                                    name=f"Ab{m}_{k}")
                    Ab.append(ab)
                    vb = fpool.tile([128, 20, 158], bf16, tag=f"vb{m}",
                                    name=f"Vb{m}_{k}")
                    Vb.append(vb)

                for rc in range(NRC):
                    r0 = 1 + 3 * rc
                    nr = 3 if rc < 6 else 2
                    NCH = nr * 158

                    def ap(c, dy=0, dx=0):
                        return U[c][:, r0 + dy : r0 + dy + nr,
                                    1 + dx : 159 + dx]

                    A = [psa.tile([128, 512], f32, tag=f"psA{m}",
                                  name=f"A{m}_{k}_{rc}", bufs=1)
                         for m in range(3)]
                    V = [psv.tile([128, 512], f32, tag=f"psV{m}",
                                  name=f"V{m}_{k}_{rc}",
                                  bufs=(2 if m < 2 else 1))
                         for m in range(3)]
                    for m in range(3):
                        nc.tensor.matmul(A[m][:, :NCH], M_D, ap(m),
                                         start=True, stop=True)
                    nc.tensor.matmul(V[0][:, :NCH], M_D, ap(3),
                                     start=True, stop=False)
                    for m in range(3):
                        nc.tensor.matmul(V[m][:, :NCH], M_VU, ap(m),
                                         start=(m != 0), stop=False)
                    for m in range(3):
                        nc.tensor.matmul(V[m][:, :NCH], M_IMU, ap(m, dy=1),
                                         start=False, stop=False)
                        nc.tensor.matmul(V[m][:, :NCH], M_IMU, ap(m, dy=-1),
                                         start=False, stop=False)
                        nc.tensor.matmul(V[m][:, :NCH], M_IMU, ap(m, dx=1),
                                         start=False, stop=False)
                        nc.tensor.matmul(V[m][:, :NCH], M_IMU, ap(m, dx=-1),
                                         start=False, stop=(m == 0))
                    nc.tensor.matmul(V[1][:, :NCH], M_IP, ap(3, dy=1),
                                     start=False, stop=False)
                    nc.tensor.matmul(V[2][:, :NCH], M_IP, ap(3, dx=1),
                                     start=False, stop=False)
                    nc.tensor.matmul(V[1][:, :NCH], M_IM, ap(3, dy=-1),
                                     start=False, stop=True)
                    nc.tensor.matmul(V[2][:, :NCH], M_IM, ap(3, dx=-1),
                                     start=False, stop=True)

                    # ACT: drain PSUM chunks into the full-supertile tiles
                    rows = slice(r0 - 1, r0 - 1 + nr)
                    for m in range(3):
                        nc.scalar.copy(Ab[m][:, rows, :], A[m][:, :NCH])
                        nc.scalar.copy(Vb[m][:, rows, :], V[m][:, :NCH])

                # DVE: full-supertile assembly (in-place chains)
                for m in range(3):
                    Dy = fpool.tile([128, 20, 158], bf16, tag="dy",
                                    name=f"Dy{m}_{k}")
                    nc.vector.tensor_tensor(Dy[:], U[m][:, 2:22, 1:159],
                                            U[m][:, 0:20, 1:159],
                                            op=AL.subtract)
                    Dx = fpool.tile([128, 20, 158], bf16, tag="dx",
                                    name=f"Dx{m}_{k}")
                    nc.vector.tensor_tensor(Dx[:], U[m][:, 1:21, 2:160],
                                            U[m][:, 1:21, 0:158],
                                            op=AL.subtract)
                    # T1 = Ab*uzc (in place over Ab)
                    nc.vector.tensor_tensor(Ab[m][:], Ab[m][:],
                                            U[0][:, 1:21, 1:159], op=AL.mult)
                    # T2 = Dy*0.5uy (in place over Dy); T3 likewise
                    nc.vector.tensor_tensor(Dy[:], Dy[:], HUY[:], op=AL.mult)
                    nc.vector.tensor_tensor(Dx[:], Dx[:], HUX[:], op=AL.mult)
                    # S = T1+T2 -> Ab; S2 = S+T3 -> Ab; R = S2+Vb -> Vb
                    nc.vector.tensor_tensor(Ab[m][:], Ab[m][:], Dy[:],
                                            op=AL.add)
                    nc.vector.tensor_tensor(Ab[m][:], Ab[m][:], Dx[:],
                                            op=AL.add)
                    nc.vector.tensor_tensor(Vb[m][:], Ab[m][:], Vb[m][:],
                                            op=AL.add)

                    # ACT: plain square-accumulate, split rows 1-18 / 19-20
                    s = (k * 3 + m) * 2
                    sq = fpool.tile([128, 20, 158], bf16, tag="sq",
                                    name=f"sq{m}_{k}")
                    nc.scalar.activation(sq[:, 0:18, :], Vb[m][:, 0:18, :],
                                         SQ, accum_out=acc[:, s : s + 1])
                    nc.scalar.activation(sq[:, 18:20, :], Vb[m][:, 18:20, :],
                                         SQ, accum_out=acc[:, s + 1 : s + 2])

            # end-mask and ship
            nc.vector.tensor_tensor(acc[:], acc[:], am[:], op=AL.mult)
            nc.sync.dma_start(out[:], acc[:])
    nc.compile()
    return nc




def build_program_v4():
    """Like v2 but with full-supertile DVE ops (amortizes the per-op pipeline
    bubble), in-place tile reuse, tensor_scalar pre-scales instead of
    scalar_tensor_tensor, ACT squares without per-op masks, and one end-mask
    multiply on the [128, NSLOT3] partial sums."""
    f32 = mybir.dt.float32
    bf16 = mybir.dt.bfloat16
    nc = bacc.Bacc("TRN2", target_bir_lowering=False, debug=False,
                   num_devices=8)
    slab = nc.declare_dram_parameter("slab", [4, NSUP, 128, YROWS, NX], bf16,
                                     isOutput=False)
    dmats = nc.declare_dram_parameter("dmats", [128, 5 * 128], bf16,
                                      isOutput=False)
    amask = nc.declare_dram_parameter("amask", [128, NSLOT3], f32,
                                      isOutput=False)
    out = nc.declare_dram_parameter("out", [128, NSLOT3], f32, isOutput=True)

    AL = mybir.AluOpType
    SQ = mybir.ActivationFunctionType.Square

    with tile.TileContext(nc) as tc:
        with (
            tc.tile_pool(name="const", bufs=1) as cpool,
            tc.tile_pool(name="inp", bufs=2) as inpool,
            tc.tile_pool(name="fld", bufs=2) as fpool,
            tc.tile_pool(name="psAV", bufs=1, space=bass.MemorySpace.PSUM) as psav,
        ):
            dm = cpool.tile([128, 5 * 128], bf16, tag="dm")
            nc.sync.dma_start(dm[:], dmats[:])
            am = cpool.tile([128, NSLOT3], f32, tag="am")
            nc.sync.dma_start(am[:], amask[:])
            acc = cpool.tile([128, NSLOT3], f32, tag="acc")

            M_D = dm[:, 0:128]
            M_VU = dm[:, 128:256]
            M_IP = dm[:, 256:384]
            M_IM = dm[:, 384:512]
            M_IMU = dm[:, 512:640]

            for k in range(3):
                U = []
                for c in range(4):
                    t = inpool.tile([128, YROWS, NX], bf16, tag=f"U{c}")
                    nc.sync.dma_start(t[:], slab[c, k])
                    U.append(t)

                # pre-scaled center factors 0.5*uy, 0.5*ux (full interior)
                HUY = fpool.tile([128, 20, 158], bf16, tag="huy")
                nc.vector.tensor_scalar_mul(HUY[:], U[1][:, 1:21, 1:159],
                                            0.5 * RHO)
                HUX = fpool.tile([128, 20, 158], bf16, tag="hux")
                nc.vector.tensor_scalar_mul(HUX[:], U[2][:, 1:21, 1:159],
                                            0.5 * RHO)

                AVb = [fpool.tile([128, 2, 20, 158], bf16, tag=f"avb{m}",
                                  name=f"AVb{m}_{k}") for m in range(3)]
                Ab = [t[:, 0] for t in AVb]
                Vb = [t[:, 1] for t in AVb]

                for rc in range(NRC):
                    r0 = 1 + 3 * rc
                    nr = 3 if rc < 6 else 2
                    NCH = nr * 158

                    def ap(c, dy=0, dx=0):
                        return U[c][:, r0 + dy : r0 + dy + nr,
                                    1 + dx : 159 + dx]

                    AV = [psav.tile([128, 1024], f32, tag=f"psAV{m}",
                                    name=f"AV{m}_{k}_{rc}")
                          for m in range(3)]
                    A = [t[:, 0:512] for t in AV]
                    V = [t[:, 512:1024] for t in AV]
                    for m in range(3):
                        nc.tensor.matmul(A[m][:, :NCH], M_D, ap(m),
                                         start=True, stop=True)
                    nc.tensor.matmul(V[0][:, :NCH], M_D, ap(3),
                                     start=True, stop=False)
                    for m in range(3):
                        nc.tensor.matmul(V[m][:, :NCH], M_VU, ap(m),
                                         start=(m != 0), stop=False)
                    for m in range(3):
                        nc.tensor.matmul(V[m][:, :NCH], M_IMU, ap(m, dy=1),
                                         start=False, stop=False)
                        nc.tensor.matmul(V[m][:, :NCH], M_IMU, ap(m, dy=-1),
                                         start=False, stop=False)
                        nc.tensor.matmul(V[m][:, :NCH], M_IMU, ap(m, dx=1),
                                         start=False, stop=False)
                        nc.tensor.matmul(V[m][:, :NCH], M_IMU, ap(m, dx=-1),
                                         start=False, stop=(m == 0))
                    nc.tensor.matmul(V[1][:, :NCH], M_IP, ap(3, dy=1),
                                     start=False, stop=False)
                    nc.tensor.matmul(V[2][:, :NCH], M_IP, ap(3, dx=1),
                                     start=False, stop=False)
                    nc.tensor.matmul(V[1][:, :NCH], M_IM, ap(3, dy=-1),
                                     start=False, stop=True)
                    nc.tensor.matmul(V[2][:, :NCH], M_IM, ap(3, dx=-1),
                                     start=False, stop=True)

                    # ACT: drain PSUM chunks into the full-supertile tiles
                    rows = slice(r0 - 1, r0 - 1 + nr)
                    for m in range(3):
                        src2 = AV[m].rearrange("p (b n) -> p b n", b=2)
                        nc.scalar.copy(AVb[m][:, :, rows, :],
                                       src2[:, :, :NCH])

                # DVE: full-supertile assembly (in-place chains)
                for m in range(3):
                    Dy = fpool.tile([128, 20, 158], bf16, tag="dy",
                                    name=f"Dy{m}_{k}")
                    nc.vector.tensor_tensor(Dy[:], U[m][:, 2:22, 1:159],
                                            U[m][:, 0:20, 1:159],
                                            op=AL.subtract)
                    Dx = fpool.tile([128, 20, 158], bf16, tag="dx",
                                    name=f"Dx{m}_{k}")
                    nc.vector.tensor_tensor(Dx[:], U[m][:, 1:21, 2:160],
                                            U[m][:, 1:21, 0:158],
                                            op=AL.subtract)
                    # T1 = Ab*uzc (in place over Ab)
                    nc.vector.tensor_tensor(Ab[m][:], Ab[m][:],
                                            U[0][:, 1:21, 1:159], op=AL.mult)
                    # T2 = Dy*0.5uy (in place over Dy); T3 likewise
                    nc.vector.tensor_tensor(Dy[:], Dy[:], HUY[:], op=AL.mult)
                    nc.vector.tensor_tensor(Dx[:], Dx[:], HUX[:], op=AL.mult)
                    # S = T1+T2 -> Ab; S2 = S+T3 -> Ab; R = S2+Vb -> Vb
                    nc.vector.tensor_tensor(Ab[m][:], Ab[m][:], Dy[:],
                                            op=AL.add)
                    nc.vector.tensor_tensor(Ab[m][:], Ab[m][:], Dx[:],
                                            op=AL.add)
                    nc.vector.tensor_tensor(Vb[m][:], Ab[m][:], Vb[m][:],
                                            op=AL.add)

                    # ACT: plain square-accumulate, split rows 1-18 / 19-20
                    s = (k * 3 + m) * 2
                    sq = fpool.tile([128, 20, 158], bf16, tag="sq",
                                    name=f"sq{m}_{k}")
                    nc.scalar.activation(sq[:, 0:18, :], Vb[m][:, 0:18, :],
                                         SQ, accum_out=acc[:, s : s + 1])
                    nc.scalar.activation(sq[:, 18:20, :], Vb[m][:, 18:20, :],
                                         SQ, accum_out=acc[:, s + 1 : s + 2])

            # end-mask and ship
            nc.vector.tensor_tensor(acc[:], acc[:], am[:], op=AL.mult)
            nc.sync.dma_start(out[:], acc[:])
    nc.compile()
    return nc




def _band_matrices_x2():
    """bf16 matrices packed [128, 5*128], all scaled x2 vs _band_matrices_v2:
    D2 (dz band, +-1), VU2 (12MU diag / -2MU off), IP2 (+I), IM2 (-I),
    IMU2 (-2MU*I).  Kernel computes R' = 2R; host divides the loss by 4."""
    import ml_dtypes
    D = np.zeros((128, 128), dtype=np.float32)
    VU = np.zeros((128, 128), dtype=np.float32)
    for p in range(128):
        z = p % ZSUP
        if 1 <= z <= ZINT:
            D[p + 1, p] = 1.0
            D[p - 1, p] = -1.0
            VU[p, p] = 12.0 * MU
            VU[p + 1, p] = -2.0 * MU
            VU[p - 1, p] = -2.0 * MU
    eye = np.eye(128, dtype=np.float32)
    packed = np.concatenate([D, VU, eye, -eye, -2.0 * MU * eye], axis=1)
    return packed.astype(ml_dtypes.bfloat16)


NSLOT6 = 3 * 3 * 2  # (supertile x momentum) x 2 row-groups


def build_program_v6():
    """All-STT DVE chain (4x mode) + fused masked square-accum on DVE;
    ACT only drains PSUM->SBUF with multi-bank strided copies; PE is the
    v5 banded bf16 scheme with x2 weights (loss /4 on host)."""
    f32 = mybir.dt.float32
    bf16 = mybir.dt.bfloat16
    nc = bacc.Bacc("TRN2", target_bir_lowering=False, debug=False,
                   num_devices=8)
    # host repacks channel-inside-partition: [NSUP, 128, 4, YROWS, NX]
    slab = nc.declare_dram_parameter("slab", [NSUP, 128, 4, YROWS, NX], bf16,
                                     isOutput=False)
    dmats = nc.declare_dram_parameter("dmats", [128, 5 * 128], bf16,
                                      isOutput=False)
    zmask = nc.declare_dram_parameter("zmask", [3, 128], f32, isOutput=False)
    out = nc.declare_dram_parameter("out", [128, NSLOT6], f32, isOutput=True)

    AL = mybir.AluOpType

    with tile.TileContext(nc) as tc:
        with (
            tc.tile_pool(name="const", bufs=1) as cpool,
            tc.tile_pool(name="inp", bufs=2) as inpool,
            tc.tile_pool(name="drn", bufs=2) as dpool,
            tc.tile_pool(name="tmp", bufs=2) as tpool,
            tc.tile_pool(name="plo", bufs=1, space=bass.MemorySpace.PSUM) as plo,
            tc.tile_pool(name="phi", bufs=1, space=bass.MemorySpace.PSUM) as phi,
        ):
            dm = cpool.tile([128, 5 * 128], bf16, tag="dm")
            nc.sync.dma_start(dm[:], dmats[:])
            zm = cpool.tile([128, 3], f32, tag="zm")
            for k in range(3):
                nc.sync.dma_start(zm[:, k : k + 1], zmask[k, :][:, None])
            acc = cpool.tile([128, NSLOT6], f32, tag="acc")
            nc.vector.memset(acc[:], 0.0)

            M_D = dm[:, 0:128]
            M_VU = dm[:, 128:256]
            M_IP = dm[:, 256:384]
            M_IM = dm[:, 384:512]
            M_IMU = dm[:, 512:640]

            for k in range(3):
                U = inpool.tile([128, 4, YROWS, NX], bf16, tag="U",
                                name=f"U_{k}")
                nc.sync.dma_start(U[:], slab[k])

                def ap(c, rc, dy=0, dx=0):
                    r0 = 1 + 3 * rc
                    nr = 3 if rc < 6 else 2
                    return U[:, c, r0 + dy : r0 + dy + nr, 1 + dx : 159 + dx]

                def cen(c):
                    return U[:, c, 1:21, 1:159]

                for m in range(3):
                    A2b = dpool.tile([128, 20, 158], bf16, tag="a2b",
                                     name=f"A2b{m}_{k}")
                    Vb = dpool.tile([128, 20, 158], bf16, tag="vb",
                                    name=f"Vb{m}_{k}")
                    # ---- PE: A then V, chunked into lo(0-3)/hi(4-6) banks
                    for half, rng, ptag in ((0, range(0, 4), "alo"),
                                            (1, range(4, 7), "ahi")):
                        pool_ = plo if half == 0 else phi
                        nb = 4 if half == 0 else 3
                        At = pool_.tile([128, nb, 512], f32, tag=f"p{half}",
                                        name=f"A{m}_{k}_{half}")
                        for c in rng:
                            nr = 3 if c < 6 else 2
                            NCH = nr * 158
                            nc.tensor.matmul(At[:, c - (0 if half == 0 else 4),
                                                :NCH],
                                             M_D, ap(m, c),
                                             start=True, stop=True)
                        # drain this half
                        if half == 0:
                            nc.scalar.copy(A2b[:, 0:12, :], At[:, :, 0:474])
                        else:
                            nc.scalar.copy(A2b[:, 12:18, :],
                                           At[:, 0:2, 0:474])
                            nc.scalar.copy(A2b[:, 18:20, :],
                                           At[:, 2:3, 0:316])
                    for half, rng in ((0, range(0, 4)), (1, range(4, 7))):
                        pool_ = plo if half == 0 else phi
                        nb = 4 if half == 0 else 3
                        Vt = pool_.tile([128, nb, 512], f32, tag=f"p{half}",
                                        name=f"V{m}_{k}_{half}")
                        for c in rng:
                            nr = 3 if c < 6 else 2
                            NCH = nr * 158
                            vt = Vt[:, c - (0 if half == 0 else 4), :NCH]
                            nc.tensor.matmul(vt, M_VU, ap(m, c),
                                             start=True, stop=False)
                            nc.tensor.matmul(vt, M_IMU, ap(m, c, dy=1),
                                             start=False, stop=False)
                            nc.tensor.matmul(vt, M_IMU, ap(m, c, dy=-1),
                                             start=False, stop=False)
                            nc.tensor.matmul(vt, M_IMU, ap(m, c, dx=1),
                                             start=False, stop=False)
                            nc.tensor.matmul(vt, M_IMU, ap(m, c, dx=-1),
                                             start=False, stop=False)
                            if m == 0:
                                nc.tensor.matmul(vt, M_D, ap(3, c),
                                                 start=False, stop=True)
                            elif m == 1:
                                nc.tensor.matmul(vt, M_IP, ap(3, c, dy=1),
                                                 start=False, stop=False)
                                nc.tensor.matmul(vt, M_IM, ap(3, c, dy=-1),
                                                 start=False, stop=True)
                            else:
                                nc.tensor.matmul(vt, M_IP, ap(3, c, dx=1),
                                                 start=False, stop=False)
                                nc.tensor.matmul(vt, M_IM, ap(3, c, dx=-1),
                                                 start=False, stop=True)
                        if half == 0:
                            nc.scalar.copy(Vb[:, 0:12, :], Vt[:, :, 0:474])
                        else:
                            nc.scalar.copy(Vb[:, 12:18, :], Vt[:, 0:2, 0:474])
                            nc.scalar.copy(Vb[:, 18:20, :],
                                           Vt[:, 2:3, 0:316])

                    # ---- DVE: STT chain, all 4x ----
                    def t20(tag):
                        return tpool.tile([128, 20, 158], bf16, tag=tag,
                                          name=f"{tag}{m}_{k}")

                    yp = U[:, m, 2:22, 1:159]
                    ym = U[:, m, 0:20, 1:159]
                    xp = U[:, m, 1:21, 2:160]
                    xm = U[:, m, 1:21, 0:158]
                    Dy = t20("dy")
                    nc.vector.scalar_tensor_tensor(Dy[:], yp, 1.0, ym,
                                                   op0=AL.mult,
                                                   op1=AL.subtract)
                    Dx = t20("dx")
                    nc.vector.scalar_tensor_tensor(Dx[:], xp, 1.0, xm,
                                                   op0=AL.mult,
                                                   op1=AL.subtract)
                    M1 = t20("m1")
                    nc.vector.scalar_tensor_tensor(M1[:], Dy[:], 1.0, cen(1),
                                                   op0=AL.mult, op1=AL.mult)
                    M2 = t20("m2")
                    nc.vector.scalar_tensor_tensor(M2[:], Dx[:], 1.0, cen(2),
                                                   op0=AL.mult, op1=AL.mult)
                    T1 = t20("t1")
                    nc.vector.scalar_tensor_tensor(T1[:], A2b[:], 1.0, cen(0),
                                                   op0=AL.mult, op1=AL.mult)
                    S1 = t20("s1")
                    nc.vector.scalar_tensor_tensor(S1[:], M1[:], 1.0, M2[:],
                                                   op0=AL.mult, op1=AL.add)
                    S2 = t20("s2")
                    nc.vector.scalar_tensor_tensor(S2[:], S1[:], 1.0, T1[:],
                                                   op0=AL.mult, op1=AL.add)
                    R = t20("r")
                    nc.vector.scalar_tensor_tensor(R[:], S2[:], 1.0, Vb[:],
                                                   op0=AL.mult, op1=AL.add)
                    # fused masked square + accumulate (zm scales once)
                    sq = t20("sq")
                    s = (k * 3 + m) * 2
                    nc.vector.scalar_tensor_tensor(
                        sq[:, 0:18, :], R[:, 0:18, :], zm[:, k : k + 1],
                        R[:, 0:18, :], op0=AL.mult, op1=AL.mult,
                        accum_out=acc[:, s : s + 1])
                    nc.vector.scalar_tensor_tensor(
                        sq[0:112, 18:20, :], R[0:112, 18:20, :],
                        zm[0:112, k : k + 1], R[0:112, 18:20, :],
                        op0=AL.mult, op1=AL.mult,
                        accum_out=acc[0:112, s + 1 : s + 2])

            nc.sync.dma_start(out[:], acc[:])
    nc.compile()
    return nc


# ---------------------------------------------------------------------------
# v7: fp8 DoubleRow PE stencils + bf16 identity-matmul adds into V-PSUM,
# DVE tensor_tensor products at 2x, gpsimd M2 product, ACT drains + squares.
# x-half units, 4-bank PSUM regions rotated 2-deep.

NSLOT7 = 3 * 3 * 2 * 2  # k x m x xh x rowgroup


def _w_bands_v7():
    """fp8 weight pair tensor [128, 14, 2, 128] + bf16 identity [128,128].

    All stencil weights are x2 (kernel computes R' = 2R; host divides by 4).
    Pair table (slot: (channel, dy, dx, matrix)):
      pair 3*m+?? -> see _PAIRS7 below.
    """
    import ml_dtypes
    D2 = np.zeros((128, 128), dtype=np.float32)
    VU2 = np.zeros((128, 128), dtype=np.float32)
    for p in range(128):
        z = p % ZSUP
        if 1 <= z <= ZINT:
            D2[p + 1, p] = 1.0
            D2[p - 1, p] = -1.0
            VU2[p, p] = 12.0 * MU
            VU2[p + 1, p] = -2.0 * MU
            VU2[p - 1, p] = -2.0 * MU
    eye = np.eye(128, dtype=np.float32)
    mats = {"D2": D2, "VU2": VU2, "IP2": eye, "IM2": -eye,
            "IMU2": -2.0 * MU * eye, "Z": np.zeros((128, 128), np.float32)}
    W = np.zeros((14, 128, 2, 128), dtype=np.float32)
    for pi, pair in enumerate(_PAIRS7):
        for sl in range(2):
            W[pi, :, sl, :] = mats[pair[sl][3]]
    # -> [128, 14, 2, 128]
    W = np.transpose(W, (1, 0, 2, 3)).copy()
    return (W.astype(ml_dtypes.float8_e4m3),
            eye.astype(ml_dtypes.bfloat16))


# pair index layout: 0-2 = A pairs for m=0,1,2; then V pairs:
# m=0: 3,4,5   m=1: 6,7,8,9   m=2: 10,11,12,13
_PAIRS7 = [
    [(0, 0, 0, "D2"), (0, 0, 1, "Z")],
    [(1, 0, 0, "D2"), (1, 0, 1, "Z")],
    [(2, 0, 0, "D2"), (2, 0, 1, "Z")],
    # V m=0
    [(0, 0, 0, "VU2"), (3, 0, 0, "D2")],
    [(0, 1, 0, "IMU2"), (0, -1, 0, "IMU2")],
    [(0, 0, 1, "IMU2"), (0, 0, -1, "IMU2")],
    # V m=1
    [(1, 0, 0, "VU2"), (3, 1, 0, "IP2")],
    [(1, 1, 0, "IMU2"), (1, -1, 0, "IMU2")],
    [(1, 0, 1, "IMU2"), (1, 0, -1, "IMU2")],
    [(3, -1, 0, "IM2"), (3, -1, 1, "Z")],
    # V m=2
    [(2, 0, 0, "VU2"), (3, 0, 1, "IP2")],
    [(2, 1, 0, "IMU2"), (2, -1, 0, "IMU2")],
    [(2, 0, 1, "IMU2"), (2, 0, -1, "IMU2")],
    [(3, 0, -1, "IM2"), (3, 0, 0, "Z")],
]

_VPAIRS7 = {0: [3, 4, 5], 1: [6, 7, 8, 9], 2: [10, 11, 12, 13]}

# row chunks per x-half unit: (out_row0, nrows); out rows 0..19
_CH7 = [(0, 6), (6, 6), (12, 6), (18, 2)]


def build_program_v7():
    from concourse.ap import AP as _AP
    f32 = mybir.dt.float32
    bf16 = mybir.dt.bfloat16
    f8 = mybir.dt.float8e4
    DRm = mybir.MatmulPerfMode.DoubleRow
    AL = mybir.AluOpType
    SQf = mybir.ActivationFunctionType.Square

    nc = bacc.Bacc("TRN2", target_bir_lowering=False, debug=False,
                   num_devices=8)
    slabb = nc.declare_dram_parameter("slabb", [NSUP, 128, 3, YROWS, NX],
                                      bf16, isOutput=False)
    slab8 = nc.declare_dram_parameter("slab8", [NSUP, 128, 4, YROWS, NX],
                                      f8, isOutput=False)
    wp = nc.declare_dram_parameter("wp", [128, 14, 2, 128], f8,
                                   isOutput=False)
    wi = nc.declare_dram_parameter("wi", [128, 128], bf16, isOutput=False)
    out = nc.declare_dram_parameter("out", [128, NSLOT7], f32, isOutput=True)

    with tile.TileContext(nc) as tc:
        with (
            tc.tile_pool(name="const", bufs=1) as cpool,
            tc.tile_pool(name="inp", bufs=2) as inpool,
            tc.tile_pool(name="drn", bufs=3) as dpool,
            tc.tile_pool(name="tmp", bufs=3) as tpool,
            tc.tile_pool(name="pv", bufs=2, space=bass.MemorySpace.PSUM) as pv,
        ):
            W = cpool.tile([128, 14, 2, 128], f8, tag="W")
            nc.sync.dma_start(W[:], wp[:])
            WI = cpool.tile([128, 128], bf16, tag="WI")
            nc.sync.dma_start(WI[:], wi[:])
            acc = cpool.tile([128, NSLOT7], f32, tag="acc")
            nc.vector.memset(acc[:], 0.0)

            pending_sq = []

            def flush_sq():
                while pending_sq:
                    pending_sq.pop(0)()

            for k in range(3):
                B = inpool.tile([128, 3, YROWS, NX], bf16, tag="B",
                                name=f"B_{k}")
                nc.sync.dma_start(B[:], slabb[k])
                F = inpool.tile([128, 4, YROWS, NX], f8, tag="F",
                                name=f"F_{k}")
                nc.sync.dma_start(F[:], slab8[k])

                fp_stride = F[:].ap[0][0]

                def pairAP(pi, r0, nr, x0):
                    (c0, dy0, dx0, _), (c1, dy1, dx1, _) = _PAIRS7[pi]
                    s0 = F[:, c0, 1 + r0 + dy0 : 1 + r0 + dy0 + nr,
                           x0 + dx0 : x0 + dx0 + 79]
                    s1 = F[:, c1, 1 + r0 + dy1 : 1 + r0 + dy1 + nr,
                           x0 + dx1 : x0 + dx1 + 79]
                    return _AP(s0.tensor, s0.offset,
                               [[fp_stride, 128], [s1.offset - s0.offset, 2],
                                [NX, nr], [1, 79]])

                for m in range(3):
                    for xh in range(2):
                        x0 = 1 + 79 * xh
                        kk, mm = k, m  # capture
                        P = pv.tile([128, 4, 512], f32, tag="pv",
                                    name=f"P_{k}_{m}_{xh}")
                        # ---- A fill ----
                        for c, (r0, nr) in enumerate(_CH7):
                            nc.tensor.matmul(P[:, c, : nr * 79],
                                             W[:, m], pairAP(m, r0, nr, x0),
                                             start=True, stop=True,
                                             perf_mode=DRm)
                        # ---- A drain (one strided op; rows 20-23 junk) ----
                        A2b = dpool.tile([128, 24, 79], bf16, tag="a2b",
                                         name=f"A2b_{k}_{m}_{xh}")
                        nc.scalar.copy(A2b[:], P[:, :, 0:474])
                        # ---- V fill (group stays open; ids close it) ----
                        for c, (r0, nr) in enumerate(_CH7):
                            vps = _VPAIRS7[m]
                            for j, pi in enumerate(vps):
                                nc.tensor.matmul(P[:, c, : nr * 79],
                                                 W[:, pi],
                                                 pairAP(pi, r0, nr, x0),
                                                 start=(j == 0), stop=False,
                                                 perf_mode=DRm)
                        # ---- DVE products ----
                        def t20(tag):
                            return tpool.tile([128, 20, 79], bf16, tag=tag,
                                              name=f"{tag}_{k}_{m}_{xh}")

                        Dy = t20("dy")
                        nc.vector.tensor_tensor(Dy[:], B[:, m, 2:22, x0:x0 + 79],
                                                B[:, m, 0:20, x0:x0 + 79],
                                                op=AL.subtract)
                        Dx = t20("dx")
                        nc.vector.tensor_tensor(Dx[:],
                                                B[:, m, 1:21, x0 + 1:x0 + 80],
                                                B[:, m, 1:21, x0 - 1:x0 + 78],
                                                op=AL.subtract)
                        M1 = t20("m1")
                        nc.vector.tensor_tensor(M1[:], Dy[:],
                                                B[:, 1, 1:21, x0:x0 + 79],
                                                op=AL.mult)
                        M2 = t20("m2")
                        nc.gpsimd.tensor_tensor(M2[:], Dx[:],
                                                B[:, 2, 1:21, x0:x0 + 79],
                                                op=AL.mult)
                        T1 = t20("t1")
                        nc.vector.tensor_tensor(T1[:], A2b[:, 0:20, :],
                                                B[:, 0, 1:21, x0:x0 + 79],
                                                op=AL.mult)
                        # ---- ids: accumulate products into V ----
                        for fi, Ft in enumerate((M1, M2, T1)):
                            for c, (r0, nr) in enumerate(_CH7):
                                nc.tensor.matmul(
                                    P[:, c, : nr * 79], WI[:],
                                    Ft[:, r0 : r0 + nr, :],
                                    start=False,
                                    stop=(fi == 2))
                        # ---- SQ (deferred one xunit for pipelining) ----
                        s = ((k * 3 + m) * 2 + xh) * 2
                        Pq = _AP(P[:].tensor, P[:].offset,
                                 [[P[:].ap[0][0], 128], [512, 3], [1, 474]])

                        def do_sq(P=P, Pq=Pq, s=s):
                            sq1 = tpool.tile([128, 3, 474], bf16, tag="sq1",
                                             name=f"sq1_{s}")
                            nc.scalar.activation(sq1[:], Pq, SQf,
                                                 accum_out=acc[:, s : s + 1])
                            sq2 = tpool.tile([128, 158], bf16, tag="sq2",
                                             name=f"sq2_{s}")
                            nc.scalar.activation(
                                sq2[0:112], P[0:112, 3, 0:158], SQf,
                                accum_out=acc[0:112, s + 1 : s + 2])

                        pending_sq.append(do_sq)
                        if len(pending_sq) > 1:
                            pending_sq.pop(0)()
            flush_sq()
            nc.sync.dma_start(out[:], acc[:])
    nc.compile()
    return nc


def build_program_v8():
    """v7 with 2-stage software-pipelined emission: per xunit i the engine
    queues carry [PE: A-fill(i), ids(i-1), V-fill(i)], [ACT: drain(i),
    SQ(i-1)], [DVE: T1(i-1), Dy/Dx/M1(i)], [Pool: M2(i)] so no engine
    head-of-line blocks on another engine's latency."""
    from concourse.ap import AP as _AP
    f32 = mybir.dt.float32
    bf16 = mybir.dt.bfloat16
    f8 = mybir.dt.float8e4
    DRm = mybir.MatmulPerfMode.DoubleRow
    AL = mybir.AluOpType
    SQf = mybir.ActivationFunctionType.Square

    nc = bacc.Bacc("TRN2", target_bir_lowering=False, debug=False,
                   num_devices=8)
    slabb = nc.declare_dram_parameter("slabb", [NSUP, 128, 3, YROWS, NX],
                                      bf16, isOutput=False)
    slab8 = nc.declare_dram_parameter("slab8", [NSUP, 128, 4, YROWS, NX],
                                      f8, isOutput=False)
    wp = nc.declare_dram_parameter("wp", [128, 14, 2, 128], f8,
                                   isOutput=False)
    wi = nc.declare_dram_parameter("wi", [128, 128], bf16, isOutput=False)
    out = nc.declare_dram_parameter("out", [128, NSLOT7], f32, isOutput=True)

    units = [(k, m, xh) for k in range(3) for m in range(3)
             for xh in range(2)]

    with tile.TileContext(nc) as tc:
        with (
            tc.tile_pool(name="const", bufs=1) as cpool,
            tc.tile_pool(name="inp", bufs=2) as inpool,
            tc.tile_pool(name="drn", bufs=3) as dpool,
            tc.tile_pool(name="tmp", bufs=3) as tpool,
            tc.tile_pool(name="pv", bufs=2, space=bass.MemorySpace.PSUM) as pv,
        ):
            W = cpool.tile([128, 14, 2, 128], f8, tag="W")
            nc.sync.dma_start(W[:], wp[:])
            WI = cpool.tile([128, 128], bf16, tag="WI")
            nc.sync.dma_start(WI[:], wi[:])
            acc = cpool.tile([128, NSLOT7], f32, tag="acc")
            nc.vector.memset(acc[:], 0.0)

            BF = {}  # k -> (B tile, F tile)

            def load_k(k):
                if k in BF or k >= 3:
                    return
                B = inpool.tile([128, 3, YROWS, NX], bf16, tag="B",
                                name=f"B_{k}")
                F = inpool.tile([128, 4, YROWS, NX], f8, tag="F",
                                name=f"F_{k}")
                nc.sync.dma_start(B[:], slabb[k])
                nc.sync.dma_start(F[:], slab8[k])
                BF[k] = (B, F)

            load_k(0)

            def pairAP(F, pi, r0, nr, x0):
                fp_stride = F[:].ap[0][0]
                (c0, dy0, dx0, _), (c1, dy1, dx1, _) = _PAIRS7[pi]
                s0 = F[:, c0, 1 + r0 + dy0 : 1 + r0 + dy0 + nr,
                       x0 + dx0 : x0 + dx0 + 79]
                s1 = F[:, c1, 1 + r0 + dy1 : 1 + r0 + dy1 + nr,
                       x0 + dx1 : x0 + dx1 + 79]
                return _AP(s0.tensor, s0.offset,
                           [[fp_stride, 128], [s1.offset - s0.offset, 2],
                            [NX, nr], [1, 79]])

            st = {}  # unit index -> state dict

            def stage_T1(j):
                u = st[j]
                T1 = tpool.tile([128, 20, 79], bf16, tag="t1",
                                name=f"t1_{j}")
                nc.vector.tensor_tensor(
                    T1[:], u["A2b"][:, 0:20, :],
                    u["B"][:, 0, 1:21, u["x0"]:u["x0"] + 79], op=AL.mult)
                u["T1"] = T1

            def stage_ids(j):
                u = st[j]
                for fi, Ft in enumerate((u["M1"], u["M2"], u["T1"])):
                    for c, (r0, nr) in enumerate(_CH7):
                        nc.tensor.matmul(u["P"][:, c, : nr * 79], WI[:],
                                         Ft[:, r0 : r0 + nr, :],
                                         start=False, stop=(fi == 2))

            def stage_sq(j):
                u = st[j]
                P, s = u["P"], u["s"]
                Pq = _AP(P[:].tensor, P[:].offset,
                         [[P[:].ap[0][0], 128], [512, 3], [1, 474]])
                sq2 = tpool.tile([128, 158], bf16, tag="sq2",
                                 name=f"sq2_{j}")
                nc.scalar.activation(sq2[0:112], P[0:112, 3, 0:158], SQf,
                                     accum_out=acc[0:112, s + 1 : s + 2])
                sq1 = tpool.tile([128, 3, 474], bf16, tag="sq1",
                                 name=f"sq1_{j}")
                nc.scalar.activation(sq1[:], Pq, SQf,
                                     accum_out=acc[:, s : s + 1])
                del st[j]

            for i, (k, m, xh) in enumerate(units):
                B, F = BF[k]
                x0 = 1 + 79 * xh
                u = {"B": B, "x0": x0,
                     "s": ((k * 3 + m) * 2 + xh) * 2}
                st[i] = u

                # DVE: finish previous unit's T1 first (A2b ready long ago)
                if i - 1 in st:
                    stage_T1(i - 1)

                # PE: A fill
                P = pv.tile([128, 4, 512], f32, tag="pv", name=f"P_{i}")
                u["P"] = P
                for c in (3, 0, 1, 2):
                    r0, nr = _CH7[c]
                    nc.tensor.matmul(P[:, c, : nr * 79], W[:, m],
                                     pairAP(F, m, r0, nr, x0),
                                     start=True, stop=True, perf_mode=DRm)
                # ACT: A drain
                A2b = dpool.tile([128, 24, 79], bf16, tag="a2b",
                                 name=f"a2b_{i}")
                nc.scalar.copy(A2b[:], P[:, :, 0:474])
                u["A2b"] = A2b

                # PE: previous unit's ids; ACT: previous unit's SQ
                if i - 1 in st:
                    stage_ids(i - 1)
                    stage_sq(i - 1)

                # PE: V fill
                for c, (r0, nr) in enumerate(_CH7):
                    for j, pi in enumerate(_VPAIRS7[m]):
                        nc.tensor.matmul(P[:, c, : nr * 79], W[:, pi],
                                         pairAP(F, pi, r0, nr, x0),
                                         start=(j == 0), stop=False,
                                         perf_mode=DRm)

                # DVE: this unit's Dy/Dx/M1 ; Pool: M2
                Dy = tpool.tile([128, 20, 79], bf16, tag="dy", name=f"dy_{i}")
                nc.vector.tensor_tensor(Dy[:], B[:, m, 2:22, x0:x0 + 79],
                                        B[:, m, 0:20, x0:x0 + 79],
                                        op=AL.subtract)
                Dx = tpool.tile([128, 20, 79], bf16, tag="dx", name=f"dx_{i}")
                nc.vector.tensor_tensor(Dx[:], B[:, m, 1:21, x0 + 1:x0 + 80],
                                        B[:, m, 1:21, x0 - 1:x0 + 78],
                                        op=AL.subtract)
                M1 = tpool.tile([128, 20, 79], bf16, tag="m1", name=f"m1_{i}")
                nc.vector.tensor_tensor(M1[:], Dy[:],
                                        B[:, 1, 1:21, x0:x0 + 79],
                                        op=AL.mult)
                u["M1"] = M1
                M2 = tpool.tile([128, 20, 79], bf16, tag="m2", name=f"m2_{i}")
                nc.gpsimd.tensor_tensor(M2[:], Dx[:],
                                        B[:, 2, 1:21, x0:x0 + 79],
                                        op=AL.mult)
                u["M2"] = M2

                # prefetch next supertile mid-way through this one
                if m == 0 and xh == 1:
                    load_k(k + 1)

            # drain the pipeline
            last = len(units) - 1
            stage_T1(last)
            stage_ids(last)
            stage_sq(last)

            nc.sync.dma_start(out[:], acc[:])
    nc.compile()
    return nc


def build_program_v10(order="v8"):
    """v7 with 2-stage software-pipelined emission: per xunit i the engine
    queues carry [PE: A-fill(i), ids(i-1), V-fill(i)], [ACT: drain(i),
    SQ(i-1)], [DVE: T1(i-1), Dy/Dx/M1(i)], [Pool: M2(i)] so no engine
    head-of-line blocks on another engine's latency."""
    from concourse.ap import AP as _AP
    f32 = mybir.dt.float32
    bf16 = mybir.dt.bfloat16
    f8 = mybir.dt.float8e4
    DRm = mybir.MatmulPerfMode.DoubleRow
    AL = mybir.AluOpType
    SQf = mybir.ActivationFunctionType.Square

    nc = bacc.Bacc("TRN2", target_bir_lowering=False, debug=False,
                   num_devices=8)
    slabb = nc.declare_dram_parameter("slabb", [NSUP, 128, 3, YROWS, NX],
                                      bf16, isOutput=False)
    slab8 = nc.declare_dram_parameter("slab8", [NSUP, 128, 4, YROWS, NX],
                                      f8, isOutput=False)
    wp = nc.declare_dram_parameter("wp", [128, 14, 2, 128], f8,
                                   isOutput=False)
    wi = nc.declare_dram_parameter("wi", [128, 128], bf16, isOutput=False)
    out = nc.declare_dram_parameter("out", [128, NSLOT7], f32, isOutput=True)

    units = [(k, m, xh) for k in range(3) for m in range(3)
             for xh in range(2)]

    with tile.TileContext(nc) as tc:
        with (
            tc.tile_pool(name="const", bufs=1) as cpool,
            tc.tile_pool(name="inp", bufs=2) as inpool,
            tc.tile_pool(name="drn", bufs=3) as dpool,
            tc.tile_pool(name="tmp", bufs=3) as tpool,
            tc.tile_pool(name="pv", bufs=2, space=bass.MemorySpace.PSUM) as pv,
        ):
            W = cpool.tile([128, 14, 2, 128], f8, tag="W")
            nc.sync.dma_start(W[:], wp[:])
            WI = cpool.tile([128, 128], bf16, tag="WI")
            nc.sync.dma_start(WI[:], wi[:])
            acc = cpool.tile([128, NSLOT7], f32, tag="acc")
            nc.vector.memset(acc[:], 0.0)

            BF = {}  # k -> (B tile, F tile)

            def load_k(k):
                if k in BF or k >= 3:
                    return
                B = inpool.tile([128, 3, YROWS, NX], bf16, tag="B",
                                name=f"B_{k}")
                nc.sync.dma_start(B[:], slabb[k])
                F = inpool.tile([128, 4, YROWS, NX], f8, tag="F",
                                name=f"F_{k}")
                nc.sync.dma_start(F[:], slab8[k])
                BF[k] = (B, F)

            load_k(0)

            def pairAP(F, pi, r0, nr, x0):
                fp_stride = F[:].ap[0][0]
                (c0, dy0, dx0, _), (c1, dy1, dx1, _) = _PAIRS7[pi]
                s0 = F[:, c0, 1 + r0 + dy0 : 1 + r0 + dy0 + nr,
                       x0 + dx0 : x0 + dx0 + 79]
                s1 = F[:, c1, 1 + r0 + dy1 : 1 + r0 + dy1 + nr,
                       x0 + dx1 : x0 + dx1 + 79]
                return _AP(s0.tensor, s0.offset,
                           [[fp_stride, 128], [s1.offset - s0.offset, 2],
                            [NX, nr], [1, 79]])

            st = {}  # unit index -> state dict

            def stage_T1(j):
                u = st[j]
                T1 = tpool.tile([128, 20, 79], bf16, tag="t1",
                                name=f"t1_{j}")
                nc.vector.tensor_tensor(
                    T1[:], u["A2b"][:, 0:20, :],
                    u["B"][:, 0, 1:21, u["x0"]:u["x0"] + 79], op=AL.mult)
                u["T1"] = T1

            def stage_ids(j):
                u = st[j]
                for fi, Ft in enumerate((u["M1"], u["M2"], u["T1"])):
                    for c, (r0, nr) in enumerate(_CH7):
                        nc.tensor.matmul(u["P"][:, c, : nr * 79], WI[:],
                                         Ft[:, r0 : r0 + nr, :],
                                         start=False, stop=(fi == 2))

            def stage_sq(j):
                u = st[j]
                P, s = u["P"], u["s"]
                Pq = _AP(P[:].tensor, P[:].offset,
                         [[P[:].ap[0][0], 128], [512, 3], [1, 474]])
                sq1 = tpool.tile([128, 3, 474], bf16, tag="sq1",
                                 name=f"sq1_{j}")
                nc.scalar.activation(sq1[:], Pq, SQf,
                                     accum_out=acc[:, s : s + 1])
                sq2 = tpool.tile([128, 158], bf16, tag="sq2",
                                 name=f"sq2_{j}")
                nc.scalar.activation(sq2[0:112], P[0:112, 3, 0:158], SQf,
                                     accum_out=acc[0:112, s + 1 : s + 2])
                del st[j]

            for i, (k, m, xh) in enumerate(units):
                B, F = BF[k]
                x0 = 1 + 79 * xh
                u = {"B": B, "x0": x0,
                     "s": ((k * 3 + m) * 2 + xh) * 2}
                st[i] = u

                # DVE: finish previous unit's T1 first (A2b ready long ago)
                if i - 1 in st:
                    stage_T1(i - 1)

                # PE: A fill
                P = pv.tile([128, 4, 512], f32, tag="pv", name=f"P_{i}")
                u["P"] = P
                for c, (r0, nr) in enumerate(_CH7):
                    nc.tensor.matmul(P[:, c, : nr * 79], W[:, m],
                                     pairAP(F, m, r0, nr, x0),
                                     start=True, stop=True, perf_mode=DRm)
                A2b = dpool.tile([128, 24, 79], bf16, tag="a2b",
                                 name=f"a2b_{i}")
                u["A2b"] = A2b

                def drain(P=P, A2b=A2b):
                    nc.scalar.copy(A2b[:], P[:, :, 0:474])

                prev = i - 1 in st
                if order == "v8":
                    drain()
                    if prev:
                        stage_ids(i - 1)
                        stage_sq(i - 1)
                elif order == "sqfirst":
                    if prev:
                        stage_ids(i - 1)
                        stage_sq(i - 1)
                    drain()
                elif order == "idsfirst":
                    if prev:
                        stage_ids(i - 1)
                    drain()
                    if prev:
                        stage_sq(i - 1)

                # PE: V fill
                for c, (r0, nr) in enumerate(_CH7):
                    for j, pi in enumerate(_VPAIRS7[m]):
                        nc.tensor.matmul(P[:, c, : nr * 79], W[:, pi],
                                         pairAP(F, pi, r0, nr, x0),
                                         start=(j == 0), stop=False,
                                         perf_mode=DRm)

                # DVE: this unit's Dy/Dx/M1 ; Pool: M2
                Dy = tpool.tile([128, 20, 79], bf16, tag="dy", name=f"dy_{i}")
                nc.vector.tensor_tensor(Dy[:], B[:, m, 2:22, x0:x0 + 79],
                                        B[:, m, 0:20, x0:x0 + 79],
                                        op=AL.subtract)
                Dx = tpool.tile([128, 20, 79], bf16, tag="dx", name=f"dx_{i}")
                nc.vector.tensor_tensor(Dx[:], B[:, m, 1:21, x0 + 1:x0 + 80],
                                        B[:, m, 1:21, x0 - 1:x0 + 78],
                                        op=AL.subtract)
                M1 = tpool.tile([128, 20, 79], bf16, tag="m1", name=f"m1_{i}")
                nc.vector.tensor_tensor(M1[:], Dy[:],
                                        B[:, 1, 1:21, x0:x0 + 79],
                                        op=AL.mult)
                u["M1"] = M1
                M2 = tpool.tile([128, 20, 79], bf16, tag="m2", name=f"m2_{i}")
                nc.gpsimd.tensor_tensor(M2[:], Dx[:],
                                        B[:, 2, 1:21, x0:x0 + 79],
                                        op=AL.mult)
                u["M2"] = M2

                # prefetch next supertile mid-way through this one
                if m == 0 and xh == 1:
                    load_k(k + 1)

            # drain the pipeline
            last = len(units) - 1
            stage_T1(last)
            stage_ids(last)
            stage_sq(last)

            nc.sync.dma_start(out[:], acc[:])
    nc.compile()
    return nc




def build_program_v9():
    """v8 + split A-drain (bank pairs) so V-fill chunks 0-1 start early,
    ACT queue [drain-a, drain-b, SQ1, SQ2], Dx emitted first so Pool's M2
    starts sooner."""
    from concourse.ap import AP as _AP
    f32 = mybir.dt.float32
    bf16 = mybir.dt.bfloat16
    f8 = mybir.dt.float8e4
    DRm = mybir.MatmulPerfMode.DoubleRow
    AL = mybir.AluOpType
    SQf = mybir.ActivationFunctionType.Square

    nc = bacc.Bacc("TRN2", target_bir_lowering=False, debug=False,
                   num_devices=8)
    slabb = nc.declare_dram_parameter("slabb", [NSUP, 128, 3, YROWS, NX],
                                      bf16, isOutput=False)
    slab8 = nc.declare_dram_parameter("slab8", [NSUP, 128, 4, YROWS, NX],
                                      f8, isOutput=False)
    wp = nc.declare_dram_parameter("wp", [128, 14, 2, 128], f8,
                                   isOutput=False)
    wi = nc.declare_dram_parameter("wi", [128, 128], bf16, isOutput=False)
    out = nc.declare_dram_parameter("out", [128, NSLOT7], f32, isOutput=True)

    units = [(k, m, xh) for k in range(3) for m in range(3)
             for xh in range(2)]

    with tile.TileContext(nc) as tc:
        with (
            tc.tile_pool(name="const", bufs=1) as cpool,
            tc.tile_pool(name="inp", bufs=2) as inpool,
            tc.tile_pool(name="drn", bufs=3) as dpool,
            tc.tile_pool(name="tmp", bufs=3) as tpool,
            tc.tile_pool(name="pv", bufs=2, space=bass.MemorySpace.PSUM) as pv,
        ):
            W = cpool.tile([128, 14, 2, 128], f8, tag="W")
            nc.sync.dma_start(W[:], wp[:])
            WI = cpool.tile([128, 128], bf16, tag="WI")
            nc.sync.dma_start(WI[:], wi[:])
            acc = cpool.tile([128, NSLOT7], f32, tag="acc")
            nc.vector.memset(acc[:], 0.0)

            BF = {}  # k -> (B tile, F tile)

            def load_k(k):
                if k in BF or k >= 3:
                    return
                B = inpool.tile([128, 3, YROWS, NX], bf16, tag="B",
                                name=f"B_{k}")
                nc.sync.dma_start(B[:], slabb[k])
                F = inpool.tile([128, 4, YROWS, NX], f8, tag="F",
                                name=f"F_{k}")
                nc.sync.dma_start(F[:], slab8[k])
                BF[k] = (B, F)

            load_k(0)

            def pairAP(F, pi, r0, nr, x0):
                fp_stride = F[:].ap[0][0]
                (c0, dy0, dx0, _), (c1, dy1, dx1, _) = _PAIRS7[pi]
                s0 = F[:, c0, 1 + r0 + dy0 : 1 + r0 + dy0 + nr,
                       x0 + dx0 : x0 + dx0 + 79]
                s1 = F[:, c1, 1 + r0 + dy1 : 1 + r0 + dy1 + nr,
                       x0 + dx1 : x0 + dx1 + 79]
                return _AP(s0.tensor, s0.offset,
                           [[fp_stride, 128], [s1.offset - s0.offset, 2],
                            [NX, nr], [1, 79]])

            st = {}  # unit index -> state dict

            def stage_T1(j):
                u = st[j]
                T1 = tpool.tile([128, 20, 79], bf16, tag="t1",
                                name=f"t1_{j}")
                nc.vector.tensor_tensor(
                    T1[:], u["A2b"][:, 0:20, :],
                    u["B"][:, 0, 1:21, u["x0"]:u["x0"] + 79], op=AL.mult)
                u["T1"] = T1

            def stage_ids(j):
                u = st[j]
                for fi, Ft in enumerate((u["M1"], u["M2"], u["T1"])):
                    for c, (r0, nr) in enumerate(_CH7):
                        nc.tensor.matmul(u["P"][:, c, : nr * 79], WI[:],
                                         Ft[:, r0 : r0 + nr, :],
                                         start=False, stop=(fi == 2))

            def stage_sq(j):
                u = st[j]
                P, s = u["P"], u["s"]
                Pq = _AP(P[:].tensor, P[:].offset,
                         [[P[:].ap[0][0], 128], [512, 3], [1, 474]])
                sq1 = tpool.tile([128, 3, 474], bf16, tag="sq1",
                                 name=f"sq1_{j}")
                nc.scalar.activation(sq1[:], Pq, SQf,
                                     accum_out=acc[:, s : s + 1])
                sq2 = tpool.tile([128, 158], bf16, tag="sq2",
                                 name=f"sq2_{j}")
                nc.scalar.activation(sq2[0:112], P[0:112, 3, 0:158], SQf,
                                     accum_out=acc[0:112, s + 1 : s + 2])
                del st[j]

            for i, (k, m, xh) in enumerate(units):
                B, F = BF[k]
                x0 = 1 + 79 * xh
                u = {"B": B, "x0": x0,
                     "s": ((k * 3 + m) * 2 + xh) * 2}
                st[i] = u

                # DVE: finish previous unit's T1 first (A2b ready long ago)
                if i - 1 in st:
                    stage_T1(i - 1)

                # PE: A fill
                P = pv.tile([128, 4, 512], f32, tag="pv", name=f"P_{i}")
                u["P"] = P
                for c, (r0, nr) in enumerate(_CH7):
                    nc.tensor.matmul(P[:, c, : nr * 79], W[:, m],
                                     pairAP(F, m, r0, nr, x0),
                                     start=True, stop=True, perf_mode=DRm)
                # ACT: A drain, split into bank pairs so V chunks 0-1 can
                # start as soon as the first half lands
                A2b = dpool.tile([128, 24, 79], bf16, tag="a2b",
                                 name=f"a2b_{i}")
                nc.scalar.copy(A2b[:, 0:12, :], P[:, 0:2, 0:474])
                nc.scalar.copy(A2b[:, 12:24, :], P[:, 2:4, 0:474])
                u["A2b"] = A2b

                # PE: previous unit's ids; ACT: previous unit's SQ
                if i - 1 in st:
                    stage_ids(i - 1)
                    stage_sq(i - 1)

                # PE: V fill
                for c, (r0, nr) in enumerate(_CH7):
                    for j, pi in enumerate(_VPAIRS7[m]):
                        nc.tensor.matmul(P[:, c, : nr * 79], W[:, pi],
                                         pairAP(F, pi, r0, nr, x0),
                                         start=(j == 0), stop=False,
                                         perf_mode=DRm)

                # DVE: Dx first so Pool's M2 can start early
                Dx = tpool.tile([128, 20, 79], bf16, tag="dx", name=f"dx_{i}")
                nc.vector.tensor_tensor(Dx[:], B[:, m, 1:21, x0 + 1:x0 + 80],
                                        B[:, m, 1:21, x0 - 1:x0 + 78],
                                        op=AL.subtract)
                M2 = tpool.tile([128, 20, 79], bf16, tag="m2", name=f"m2_{i}")
                nc.gpsimd.tensor_tensor(M2[:], Dx[:],
                                        B[:, 2, 1:21, x0:x0 + 79],
                                        op=AL.mult)
                u["M2"] = M2
                Dy = tpool.tile([128, 20, 79], bf16, tag="dy", name=f"dy_{i}")
                nc.vector.tensor_tensor(Dy[:], B[:, m, 2:22, x0:x0 + 79],
                                        B[:, m, 0:20, x0:x0 + 79],
                                        op=AL.subtract)
                M1 = tpool.tile([128, 20, 79], bf16, tag="m1", name=f"m1_{i}")
                nc.vector.tensor_tensor(M1[:], Dy[:],
                                        B[:, 1, 1:21, x0:x0 + 79],
                                        op=AL.mult)
                u["M1"] = M1

                # prefetch next supertile mid-way through this one
                if m == 0 and xh == 1:
                    load_k(k + 1)

            # drain the pipeline
            last = len(units) - 1
            stage_T1(last)
            stage_ids(last)
            stage_sq(last)

            nc.sync.dma_start(out[:], acc[:])
    nc.compile()
    return nc


def build_program_v9a():
    """v8 + split A-drain (bank pairs) so V-fill chunks 0-1 start early,
    ACT queue [drain-a, drain-b, SQ1, SQ2], Dx emitted first so Pool's M2
    starts sooner."""
    from concourse.ap import AP as _AP
    f32 = mybir.dt.float32
    bf16 = mybir.dt.bfloat16
    f8 = mybir.dt.float8e4
    DRm = mybir.MatmulPerfMode.DoubleRow
    AL = mybir.AluOpType
    SQf = mybir.ActivationFunctionType.Square

    nc = bacc.Bacc("TRN2", target_bir_lowering=False, debug=False,
                   num_devices=8)
    slabb = nc.declare_dram_parameter("slabb", [NSUP, 128, 3, YROWS, NX],
                                      bf16, isOutput=False)
    slab8 = nc.declare_dram_parameter("slab8", [NSUP, 128, 4, YROWS, NX],
                                      f8, isOutput=False)
    wp = nc.declare_dram_parameter("wp", [128, 14, 2, 128], f8,
                                   isOutput=False)
    wi = nc.declare_dram_parameter("wi", [128, 128], bf16, isOutput=False)
    out = nc.declare_dram_parameter("out", [128, NSLOT7], f32, isOutput=True)

    units = [(k, m, xh) for k in range(3) for m in range(3)
             for xh in range(2)]

    with tile.TileContext(nc) as tc:
        with (
            tc.tile_pool(name="const", bufs=1) as cpool,
            tc.tile_pool(name="inp", bufs=2) as inpool,
            tc.tile_pool(name="drn", bufs=3) as dpool,
            tc.tile_pool(name="tmp", bufs=3) as tpool,
            tc.tile_pool(name="pv", bufs=2, space=bass.MemorySpace.PSUM) as pv,
        ):
            W = cpool.tile([128, 14, 2, 128], f8, tag="W")
            nc.sync.dma_start(W[:], wp[:])
            WI = cpool.tile([128, 128], bf16, tag="WI")
            nc.sync.dma_start(WI[:], wi[:])
            acc = cpool.tile([128, NSLOT7], f32, tag="acc")
            nc.vector.memset(acc[:], 0.0)

            BF = {}  # k -> (B tile, F tile)

            def load_k(k):
                if k in BF or k >= 3:
                    return
                B = inpool.tile([128, 3, YROWS, NX], bf16, tag="B",
                                name=f"B_{k}")
                nc.sync.dma_start(B[:], slabb[k])
                F = inpool.tile([128, 4, YROWS, NX], f8, tag="F",
                                name=f"F_{k}")
                nc.sync.dma_start(F[:], slab8[k])
                BF[k] = (B, F)

            load_k(0)

            def pairAP(F, pi, r0, nr, x0):
                fp_stride = F[:].ap[0][0]
                (c0, dy0, dx0, _), (c1, dy1, dx1, _) = _PAIRS7[pi]
                s0 = F[:, c0, 1 + r0 + dy0 : 1 + r0 + dy0 + nr,
                       x0 + dx0 : x0 + dx0 + 79]
                s1 = F[:, c1, 1 + r0 + dy1 : 1 + r0 + dy1 + nr,
                       x0 + dx1 : x0 + dx1 + 79]
                return _AP(s0.tensor, s0.offset,
                           [[fp_stride, 128], [s1.offset - s0.offset, 2],
                            [NX, nr], [1, 79]])

            st = {}  # unit index -> state dict

            def stage_T1(j):
                u = st[j]
                T1 = tpool.tile([128, 20, 79], bf16, tag="t1",
                                name=f"t1_{j}")
                nc.vector.tensor_tensor(
                    T1[:], u["A2b"][:, 0:20, :],
                    u["B"][:, 0, 1:21, u["x0"]:u["x0"] + 79], op=AL.mult)
                u["T1"] = T1

            def stage_ids(j):
                u = st[j]
                for fi, Ft in enumerate((u["M1"], u["M2"], u["T1"])):
                    for c, (r0, nr) in enumerate(_CH7):
                        nc.tensor.matmul(u["P"][:, c, : nr * 79], WI[:],
                                         Ft[:, r0 : r0 + nr, :],
                                         start=False, stop=(fi == 2))

            def stage_sq(j):
                u = st[j]
                P, s = u["P"], u["s"]
                Pq = _AP(P[:].tensor, P[:].offset,
                         [[P[:].ap[0][0], 128], [512, 3], [1, 474]])
                sq1 = tpool.tile([128, 3, 474], bf16, tag="sq1",
                                 name=f"sq1_{j}")
                nc.scalar.activation(sq1[:], Pq, SQf,
                                     accum_out=acc[:, s : s + 1])
                sq2 = tpool.tile([128, 158], bf16, tag="sq2",
                                 name=f"sq2_{j}")
                nc.scalar.activation(sq2[0:112], P[0:112, 3, 0:158], SQf,
                                     accum_out=acc[0:112, s + 1 : s + 2])
                del st[j]

            for i, (k, m, xh) in enumerate(units):
                B, F = BF[k]
                x0 = 1 + 79 * xh
                u = {"B": B, "x0": x0,
                     "s": ((k * 3 + m) * 2 + xh) * 2}
                st[i] = u

                # DVE: finish previous unit's T1 first (A2b ready long ago)
                if i - 1 in st:
                    stage_T1(i - 1)

                # PE: A fill
                P = pv.tile([128, 4, 512], f32, tag="pv", name=f"P_{i}")
                u["P"] = P
                for c, (r0, nr) in enumerate(_CH7):
                    nc.tensor.matmul(P[:, c, : nr * 79], W[:, m],
                                     pairAP(F, m, r0, nr, x0),
                                     start=True, stop=True, perf_mode=DRm)
                # ACT: A drain, split into bank pairs so V chunks 0-1 can
                # start as soon as the first half lands
                A2b = dpool.tile([128, 24, 79], bf16, tag="a2b",
                                 name=f"a2b_{i}")
                nc.scalar.copy(A2b[:, 0:12, :], P[:, 0:2, 0:474])
                nc.scalar.copy(A2b[:, 12:24, :], P[:, 2:4, 0:474])
                u["A2b"] = A2b

                # PE: previous unit's ids; ACT: previous unit's SQ
                if i - 1 in st:
                    stage_ids(i - 1)
                    stage_sq(i - 1)

                # PE: V fill
                for c, (r0, nr) in enumerate(_CH7):
                    for j, pi in enumerate(_VPAIRS7[m]):
                        nc.tensor.matmul(P[:, c, : nr * 79], W[:, pi],
                                         pairAP(F, pi, r0, nr, x0),
                                         start=(j == 0), stop=False,
                                         perf_mode=DRm)

                Dy = tpool.tile([128, 20, 79], bf16, tag="dy", name=f"dy_{i}")
                nc.vector.tensor_tensor(Dy[:], B[:, m, 2:22, x0:x0 + 79],
                                        B[:, m, 0:20, x0:x0 + 79],
                                        op=AL.subtract)
                Dx = tpool.tile([128, 20, 79], bf16, tag="dx", name=f"dx_{i}")
                nc.vector.tensor_tensor(Dx[:], B[:, m, 1:21, x0 + 1:x0 + 80],
                                        B[:, m, 1:21, x0 - 1:x0 + 78],
                                        op=AL.subtract)
                M1 = tpool.tile([128, 20, 79], bf16, tag="m1", name=f"m1_{i}")
                nc.vector.tensor_tensor(M1[:], Dy[:],
                                        B[:, 1, 1:21, x0:x0 + 79],
                                        op=AL.mult)
                u["M1"] = M1
                M2 = tpool.tile([128, 20, 79], bf16, tag="m2", name=f"m2_{i}")
                nc.gpsimd.tensor_tensor(M2[:], Dx[:],
                                        B[:, 2, 1:21, x0:x0 + 79],
                                        op=AL.mult)
                u["M2"] = M2

                # prefetch next supertile mid-way through this one
                if m == 0 and xh == 1:
                    load_k(k + 1)

            # drain the pipeline
            last = len(units) - 1
            stage_T1(last)
            stage_ids(last)
            stage_sq(last)

            nc.sync.dma_start(out[:], acc[:])
    nc.compile()
    return nc




def build_program_v9b():
    """v8 + split A-drain (bank pairs) so V-fill chunks 0-1 start early,
    ACT queue [drain-a, drain-b, SQ1, SQ2], Dx emitted first so Pool's M2
    starts sooner."""
    from concourse.ap import AP as _AP
    f32 = mybir.dt.float32
    bf16 = mybir.dt.bfloat16
    f8 = mybir.dt.float8e4
    DRm = mybir.MatmulPerfMode.DoubleRow
    AL = mybir.AluOpType
    SQf = mybir.ActivationFunctionType.Square

    nc = bacc.Bacc("TRN2", target_bir_lowering=False, debug=False,
                   num_devices=8)
    slabb = nc.declare_dram_parameter("slabb", [NSUP, 128, 3, YROWS, NX],
                                      bf16, isOutput=False)
    slab8 = nc.declare_dram_parameter("slab8", [NSUP, 128, 4, YROWS, NX],
                                      f8, isOutput=False)
    wp = nc.declare_dram_parameter("wp", [128, 14, 2, 128], f8,
                                   isOutput=False)
    wi = nc.declare_dram_parameter("wi", [128, 128], bf16, isOutput=False)
    out = nc.declare_dram_parameter("out", [128, NSLOT7], f32, isOutput=True)

    units = [(k, m, xh) for k in range(3) for m in range(3)
             for xh in range(2)]

    with tile.TileContext(nc) as tc:
        with (
            tc.tile_pool(name="const", bufs=1) as cpool,
            tc.tile_pool(name="inp", bufs=2) as inpool,
            tc.tile_pool(name="drn", bufs=3) as dpool,
            tc.tile_pool(name="tmp", bufs=3) as tpool,
            tc.tile_pool(name="pv", bufs=2, space=bass.MemorySpace.PSUM) as pv,
        ):
            W = cpool.tile([128, 14, 2, 128], f8, tag="W")
            nc.sync.dma_start(W[:], wp[:])
            WI = cpool.tile([128, 128], bf16, tag="WI")
            nc.sync.dma_start(WI[:], wi[:])
            acc = cpool.tile([128, NSLOT7], f32, tag="acc")
            nc.vector.memset(acc[:], 0.0)

            BF = {}  # k -> (B tile, F tile)

            def load_k(k):
                if k in BF or k >= 3:
                    return
                B = inpool.tile([128, 3, YROWS, NX], bf16, tag="B",
                                name=f"B_{k}")
                nc.sync.dma_start(B[:], slabb[k])
                F = inpool.tile([128, 4, YROWS, NX], f8, tag="F",
                                name=f"F_{k}")
                nc.sync.dma_start(F[:], slab8[k])
                BF[k] = (B, F)

            load_k(0)

            def pairAP(F, pi, r0, nr, x0):
                fp_stride = F[:].ap[0][0]
                (c0, dy0, dx0, _), (c1, dy1, dx1, _) = _PAIRS7[pi]
                s0 = F[:, c0, 1 + r0 + dy0 : 1 + r0 + dy0 + nr,
                       x0 + dx0 : x0 + dx0 + 79]
                s1 = F[:, c1, 1 + r0 + dy1 : 1 + r0 + dy1 + nr,
                       x0 + dx1 : x0 + dx1 + 79]
                return _AP(s0.tensor, s0.offset,
                           [[fp_stride, 128], [s1.offset - s0.offset, 2],
                            [NX, nr], [1, 79]])

            st = {}  # unit index -> state dict

            def stage_T1(j):
                u = st[j]
                T1 = tpool.tile([128, 20, 79], bf16, tag="t1",
                                name=f"t1_{j}")
                nc.vector.tensor_tensor(
                    T1[:], u["A2b"][:, 0:20, :],
                    u["B"][:, 0, 1:21, u["x0"]:u["x0"] + 79], op=AL.mult)
                u["T1"] = T1

            def stage_ids(j):
                u = st[j]
                for fi, Ft in enumerate((u["M1"], u["M2"], u["T1"])):
                    for c, (r0, nr) in enumerate(_CH7):
                        nc.tensor.matmul(u["P"][:, c, : nr * 79], WI[:],
                                         Ft[:, r0 : r0 + nr, :],
                                         start=False, stop=(fi == 2))

            def stage_sq(j):
                u = st[j]
                P, s = u["P"], u["s"]
                Pq = _AP(P[:].tensor, P[:].offset,
                         [[P[:].ap[0][0], 128], [512, 3], [1, 474]])
                sq1 = tpool.tile([128, 3, 474], bf16, tag="sq1",
                                 name=f"sq1_{j}")
                nc.scalar.activation(sq1[:], Pq, SQf,
                                     accum_out=acc[:, s : s + 1])
                sq2 = tpool.tile([128, 158], bf16, tag="sq2",
                                 name=f"sq2_{j}")
                nc.scalar.activation(sq2[0:112], P[0:112, 3, 0:158], SQf,
                                     accum_out=acc[0:112, s + 1 : s + 2])
                del st[j]

            for i, (k, m, xh) in enumerate(units):
                B, F = BF[k]
                x0 = 1 + 79 * xh
                u = {"B": B, "x0": x0,
                     "s": ((k * 3 + m) * 2 + xh) * 2}
                st[i] = u

                # DVE: finish previous unit's T1 first (A2b ready long ago)
                if i - 1 in st:
                    stage_T1(i - 1)

                # PE: A fill
                P = pv.tile([128, 4, 512], f32, tag="pv", name=f"P_{i}")
                u["P"] = P
                for c, (r0, nr) in enumerate(_CH7):
                    nc.tensor.matmul(P[:, c, : nr * 79], W[:, m],
                                     pairAP(F, m, r0, nr, x0),
                                     start=True, stop=True, perf_mode=DRm)
                A2b = dpool.tile([128, 24, 79], bf16, tag="a2b",
                                 name=f"a2b_{i}")
                nc.scalar.copy(A2b[:], P[:, :, 0:474])
                u["A2b"] = A2b

                # PE: previous unit's ids; ACT: previous unit's SQ
                if i - 1 in st:
                    stage_ids(i - 1)
                    stage_sq(i - 1)

                # PE: V fill
                for c, (r0, nr) in enumerate(_CH7):
                    for j, pi in enumerate(_VPAIRS7[m]):
                        nc.tensor.matmul(P[:, c, : nr * 79], W[:, pi],
                                         pairAP(F, pi, r0, nr, x0),
                                         start=(j == 0), stop=False,
                                         perf_mode=DRm)

                # DVE: Dx first so Pool's M2 can start early
                Dx = tpool.tile([128, 20, 79], bf16, tag="dx", name=f"dx_{i}")
                nc.vector.tensor_tensor(Dx[:], B[:, m, 1:21, x0 + 1:x0 + 80],
                                        B[:, m, 1:21, x0 - 1:x0 + 78],
                                        op=AL.subtract)
                M2 = tpool.tile([128, 20, 79], bf16, tag="m2", name=f"m2_{i}")
                nc.gpsimd.tensor_tensor(M2[:], Dx[:],
                                        B[:, 2, 1:21, x0:x0 + 79],
                                        op=AL.mult)
                u["M2"] = M2
                Dy = tpool.tile([128, 20, 79], bf16, tag="dy", name=f"dy_{i}")
                nc.vector.tensor_tensor(Dy[:], B[:, m, 2:22, x0:x0 + 79],
                                        B[:, m, 0:20, x0:x0 + 79],
                                        op=AL.subtract)
                M1 = tpool.tile([128, 20, 79], bf16, tag="m1", name=f"m1_{i}")
                nc.vector.tensor_tensor(M1[:], Dy[:],
                                        B[:, 1, 1:21, x0:x0 + 79],
                                        op=AL.mult)
                u["M1"] = M1

                # prefetch next supertile mid-way through this one
                if m == 0 and xh == 1:
                    load_k(k + 1)

            # drain the pipeline
            last = len(units) - 1
            stage_T1(last)
            stage_ids(last)
            stage_sq(last)

            nc.sync.dma_start(out[:], acc[:])
    nc.compile()
    return nc





# ---------------------------------------------------------------------------
# v11: v8 with single-channel DoubleRow pairs so every DMA is a whole-tile
# transfer (per-channel DRAM params); channel-priority DMA order shrinks the
# startup head without the partial-slice NEFF crash.

_PAIRS11 = [
    # A pairs (m = 0,1,2)
    (0, (0, 0, "D2"), (0, 1, "Z")),
    (1, (0, 0, "D2"), (0, 1, "Z")),
    (2, (0, 0, "D2"), (0, 1, "Z")),
    # V m=0
    (0, (0, 0, "VU2"), (1, 0, "IMU2")),
    (0, (-1, 0, "IMU2"), (0, 1, "IMU2")),
    (0, (0, -1, "IMU2"), (0, 0, "Z")),
    (3, (0, 0, "D2"), (0, 1, "Z")),
    # V m=1
    (1, (0, 0, "VU2"), (1, 0, "IMU2")),
    (1, (-1, 0, "IMU2"), (0, 1, "IMU2")),
    (1, (0, -1, "IMU2"), (0, 0, "Z")),
    (3, (1, 0, "IP2"), (-1, 0, "IM2")),
    # V m=2
    (2, (0, 0, "VU2"), (1, 0, "IMU2")),
    (2, (-1, 0, "IMU2"), (0, 1, "IMU2")),
    (2, (0, -1, "IMU2"), (0, 0, "Z")),
    (3, (0, 1, "IP2"), (0, -1, "IM2")),
]

_VPAIRS11 = {0: [3, 4, 5, 6], 1: [7, 8, 9, 10], 2: [11, 12, 13, 14]}


def _w_bands_v11():
    import ml_dtypes
    D2 = np.zeros((128, 128), dtype=np.float32)
    VU2 = np.zeros((128, 128), dtype=np.float32)
    for p in range(128):
        z = p % ZSUP
        if 1 <= z <= ZINT:
            D2[p + 1, p] = 1.0
            D2[p - 1, p] = -1.0
            VU2[p, p] = 12.0 * MU
            VU2[p + 1, p] = -2.0 * MU
            VU2[p - 1, p] = -2.0 * MU
    eye = np.eye(128, dtype=np.float32)
    mats = {"D2": D2, "VU2": VU2, "IP2": eye, "IM2": -eye,
            "IMU2": -2.0 * MU * eye, "Z": np.zeros((128, 128), np.float32)}
    W = np.zeros((15, 128, 2, 128), dtype=np.float32)
    for pi, (ch, s0, s1) in enumerate(_PAIRS11):
        W[pi, :, 0, :] = mats[s0[2]]
        W[pi, :, 1, :] = mats[s1[2]]
    W = np.transpose(W, (1, 0, 2, 3)).copy()
    return (W.astype(ml_dtypes.float8_e4m3), eye.astype(ml_dtypes.bfloat16))


def build_program_v11():
    from concourse.ap import AP as _AP
    f32 = mybir.dt.float32
    bf16 = mybir.dt.bfloat16
    f8 = mybir.dt.float8e4
    DRm = mybir.MatmulPerfMode.DoubleRow
    AL = mybir.AluOpType
    SQf = mybir.ActivationFunctionType.Square

    nc = bacc.Bacc("TRN2", target_bir_lowering=False, debug=False,
                   num_devices=8)
    dbs = [nc.declare_dram_parameter(f"b{c}", [NSUP, 128, YROWS, NX], bf16,
                                     isOutput=False) for c in range(3)]
    dfs = [nc.declare_dram_parameter(f"f{c}", [NSUP, 128, YROWS, NX], f8,
                                     isOutput=False) for c in range(4)]
    wp = nc.declare_dram_parameter("wp", [128, 15, 2, 128], f8,
                                   isOutput=False)
    wi = nc.declare_dram_parameter("wi", [128, 128], bf16, isOutput=False)
    out = nc.declare_dram_parameter("out", [128, NSLOT7], f32, isOutput=True)

    units = [(k, m, xh) for k in range(3) for m in range(3)
             for xh in range(2)]

    with tile.TileContext(nc) as tc:
        with (
            tc.tile_pool(name="const", bufs=1) as cpool,
            tc.tile_pool(name="inp", bufs=2) as inpool,
            tc.tile_pool(name="drn", bufs=3) as dpool,
            tc.tile_pool(name="tmp", bufs=3) as tpool,
            tc.tile_pool(name="pv", bufs=2, space=bass.MemorySpace.PSUM) as pv,
        ):
            W = cpool.tile([128, 15, 2, 128], f8, tag="W")
            nc.sync.dma_start(W[:], wp[:])
            WI = cpool.tile([128, 128], bf16, tag="WI")
            nc.sync.dma_start(WI[:], wi[:])
            acc = cpool.tile([128, NSLOT7], f32, tag="acc")
            nc.vector.memset(acc[:], 0.0)

            BF = {}  # k -> (list of 3 B tiles, list of 4 F tiles)

            def load_k(k):
                if k in BF or k >= 3:
                    return
                Bs = [inpool.tile([128, YROWS, NX], bf16, tag=f"B{c}",
                                  name=f"B{c}_{k}") for c in range(3)]
                Fs = [inpool.tile([128, YROWS, NX], f8, tag=f"F{c}",
                                  name=f"F{c}_{k}") for c in range(4)]
                # dependency-priority order: whole-tile transfers only
                nc.sync.dma_start(Bs[0][:], dbs[0][k])
                nc.sync.dma_start(Fs[0][:], dfs[0][k])
                nc.sync.dma_start(Fs[3][:], dfs[3][k])
                nc.sync.dma_start(Bs[1][:], dbs[1][k])
                nc.sync.dma_start(Bs[2][:], dbs[2][k])
                nc.sync.dma_start(Fs[1][:], dfs[1][k])
                nc.sync.dma_start(Fs[2][:], dfs[2][k])
                BF[k] = (Bs, Fs)

            load_k(0)

            def pairAP(Fs, pi, r0, nr, x0):
                ch, (dy0, dx0, _), (dy1, dx1, _) = _PAIRS11[pi]
                Ft = Fs[ch]
                fp_stride = Ft[:].ap[0][0]
                s0 = Ft[:, 1 + r0 + dy0 : 1 + r0 + dy0 + nr,
                        x0 + dx0 : x0 + dx0 + 79]
                s1 = Ft[:, 1 + r0 + dy1 : 1 + r0 + dy1 + nr,
                        x0 + dx1 : x0 + dx1 + 79]
                return _AP(s0.tensor, s0.offset,
                           [[fp_stride, 128], [s1.offset - s0.offset, 2],
                            [NX, nr], [1, 79]])

            st = {}

            def stage_T1(j):
                u = st[j]
                T1 = tpool.tile([128, 20, 79], bf16, tag="t1",
                                name=f"t1_{j}")
                nc.vector.tensor_tensor(
                    T1[:], u["A2b"][:, 0:20, :],
                    u["Bs"][0][:, 1:21, u["x0"]:u["x0"] + 79], op=AL.mult)
                u["T1"] = T1

            def stage_ids(j):
                u = st[j]
                for fi, Ft in enumerate((u["M1"], u["M2"], u["T1"])):
                    for c, (r0, nr) in enumerate(_CH7):
                        nc.tensor.matmul(u["P"][:, c, : nr * 79], WI[:],
                                         Ft[:, r0 : r0 + nr, :],
                                         start=False, stop=(fi == 2))

            def stage_sq(j):
                u = st[j]
                P, s = u["P"], u["s"]
                Pq = _AP(P[:].tensor, P[:].offset,
                         [[P[:].ap[0][0], 128], [512, 3], [1, 474]])
                sq2 = tpool.tile([128, 158], bf16, tag="sq2",
                                 name=f"sq2_{j}")
                nc.scalar.activation(sq2[0:112], P[0:112, 3, 0:158], SQf,
                                     accum_out=acc[0:112, s + 1 : s + 2])
                sq1 = tpool.tile([128, 3, 474], bf16, tag="sq1",
                                 name=f"sq1_{j}")
                nc.scalar.activation(sq1[:], Pq, SQf,
                                     accum_out=acc[:, s : s + 1])
                del st[j]

            for i, (k, m, xh) in enumerate(units):
                Bs, Fs = BF[k]
                x0 = 1 + 79 * xh
                u = {"Bs": Bs, "x0": x0,
                     "s": ((k * 3 + m) * 2 + xh) * 2}
                st[i] = u

                if i - 1 in st:
                    stage_T1(i - 1)

                P = pv.tile([128, 4, 512], f32, tag="pv", name=f"P_{i}")
                u["P"] = P
                for c in (3, 0, 1, 2):
                    r0, nr = _CH7[c]
                    nc.tensor.matmul(P[:, c, : nr * 79], W[:, m],
                                     pairAP(Fs, m, r0, nr, x0),
                                     start=True, stop=True, perf_mode=DRm)
                A2b = dpool.tile([128, 24, 79], bf16, tag="a2b",
                                 name=f"a2b_{i}")
                nc.scalar.copy(A2b[:], P[:, :, 0:474])
                u["A2b"] = A2b

                if i - 1 in st:
                    stage_ids(i - 1)
                    stage_sq(i - 1)

                for c, (r0, nr) in enumerate(_CH7):
                    for j, pi in enumerate(_VPAIRS11[m]):
                        nc.tensor.matmul(P[:, c, : nr * 79], W[:, pi],
                                         pairAP(Fs, pi, r0, nr, x0),
                                         start=(j == 0), stop=False,
                                         perf_mode=DRm)

                Dy = tpool.tile([128, 20, 79], bf16, tag="dy", name=f"dy_{i}")
                nc.vector.tensor_tensor(Dy[:], Bs[m][:, 2:22, x0:x0 + 79],
                                        Bs[m][:, 0:20, x0:x0 + 79],
                                        op=AL.subtract)
                Dx = tpool.tile([128, 20, 79], bf16, tag="dx", name=f"dx_{i}")
                nc.vector.tensor_tensor(Dx[:],
                                        Bs[m][:, 1:21, x0 + 1:x0 + 80],
                                        Bs[m][:, 1:21, x0 - 1:x0 + 78],
                                        op=AL.subtract)
                M1 = tpool.tile([128, 20, 79], bf16, tag="m1", name=f"m1_{i}")
                nc.vector.tensor_tensor(M1[:], Dy[:],
                                        Bs[1][:, 1:21, x0:x0 + 79],
                                        op=AL.mult)
                u["M1"] = M1
                M2 = tpool.tile([128, 20, 79], bf16, tag="m2", name=f"m2_{i}")
                nc.gpsimd.tensor_tensor(M2[:], Dx[:],
                                        Bs[2][:, 1:21, x0:x0 + 79],
                                        op=AL.mult)
                u["M2"] = M2

                if m == 1 and xh == 0:
                    load_k(k + 1)

            last = len(units) - 1
            stage_T1(last)
            stage_ids(last)
            stage_sq(last)

            nc.sync.dma_start(out[:], acc[:])
    nc.compile()
    return nc


def make_zslab(output, b, zc):
    """[4, 44, 162, 160] f32 slab for core (b, zc) from output [2,4,160,...]."""
    slab = np.zeros((4, NZ_SLAB, NY_PAD, NX), dtype=np.float32)
    z0 = 40 * zc
    zn = min(NZ_SLAB, 160 - z0)
    slab[:, :zn, :160, :] = output[b, :, z0 : z0 + zn, :, :]
    return slab


def pack_slab(zslab):
    """Repack [4,44,162,160] -> device layout [4, 3, 128, 22, 160]."""
    out = np.empty((4, NSUP, 128, YROWS, NX), dtype=np.float32)
    for k in range(NSUP):
        zk = zslab[:, 14 * k : 14 * k + 16]          # [4,16,162,160]
        for q in range(NYB):
            out[:, k, 16 * q : 16 * q + 16] = zk[:, :, 20 * q : 20 * q + 22, :]
    return out


def pack_slab_chan(zslab):
    """Repack [4,44,162,160] -> [NSUP, 128, 4, YROWS, NX] (channel inside
    the partition's free dim, one big DMA per supertile)."""
    out = np.empty((NSUP, 128, 4, YROWS, NX), dtype=np.float32)
    for k in range(NSUP):
        zk = zslab[:, 14 * k : 14 * k + 16]          # [4,16,162,160]
        for q in range(NYB):
            # partition p = q*16 + z ; channel axis after partition
            out[k, 16 * q : 16 * q + 16] = np.transpose(
                zk[:, :, 20 * q : 20 * q + 22, :], (1, 0, 2, 3))
    return out


VARIANT = "v11"
_NC_CACHE = {}


_BUILDERS = {"v1": build_program, "v2": build_program_v2,
             "v3": build_program_v3, "v4": build_program_v4,
             "v5": build_program_v5, "v6": build_program_v6,
             "v7": build_program_v7, "v8": build_program_v8,
             "v9": build_program_v9, "v9a": build_program_v9a,
             "v9b": build_program_v9b,
             "v10sq": (lambda: build_program_v10("sqfirst")),
             "v10id": (lambda: build_program_v10("idsfirst")),
             "v11": build_program_v11}


def _get_nc():
    if VARIANT not in _NC_CACHE:
        _NC_CACHE[VARIANT] = _BUILDERS[VARIANT]()
    return _NC_CACHE[VARIANT]


def make_in_maps(output):
    import ml_dtypes
    if VARIANT == "v11":
        w8, wi = _w_bands_v11()
        in_maps = []
        for core in range(8):
            b, zc = core // 4, core % 4
            s = pack_slab_chan(make_zslab(output, b, zc))
            im = {"wp": w8, "wi": wi}
            for c in range(3):
                im[f"b{c}"] = s[:, :, c].astype(ml_dtypes.bfloat16).copy()
            for c in range(4):
                im[f"f{c}"] = s[:, :, c].astype(ml_dtypes.float8_e4m3).copy()
            in_maps.append(im)
        return in_maps
    if VARIANT in ("v7", "v8", "v9", "v9a", "v9b", "v10sq", "v10id"):
        w8, wi = _w_bands_v7()
        in_maps = []
        for core in range(8):
            b, zc = core // 4, core % 4
            s = pack_slab_chan(make_zslab(output, b, zc))
            in_maps.append({
                "slabb": s[:, :, 0:3].astype(ml_dtypes.bfloat16).copy(),
                "slab8": s.astype(ml_dtypes.float8_e4m3),
                "wp": w8, "wi": wi})
        return in_maps
    if VARIANT == "v6":
        dmats = _band_matrices_x2()
        in_maps = []
        for core in range(8):
            b, zc = core // 4, core % 4
            s = pack_slab_chan(make_zslab(output, b, zc))
            in_maps.append({"slab": s.astype(ml_dtypes.bfloat16),
                            "dmats": dmats, "zmask": _zmask(zc)})
        return in_maps
    dmats = _band_matrices() if VARIANT == "v1" else _band_matrices_v2()
    in_maps = []
    for core in range(8):
        b, zc = core // 4, core % 4
        s = pack_slab(make_zslab(output, b, zc))
        if VARIANT != "v1":
            s = s.astype(ml_dtypes.bfloat16)
        im = {"slab": s, "dmats": dmats}
        if VARIANT in ("v3", "v4", "v5"):
            im["amask"] = _amask(zc)
        else:
            im["zmask"] = _zmask(zc)
        in_maps.append(im)
    return in_maps


def kernel(output, inp):
    output = np.asarray(output, dtype=np.float32)
    nc = _get_nc()
    res = run_bass_kernel_spmd(nc, make_in_maps(output),
                               core_ids=list(range(8)))
    total = np.float64(0.0)
    if VARIANT in ("v7", "v8", "v9", "v9a", "v9b", "v10sq", "v10id", "v11"):
        for core, r in enumerate(res.results):
            zc = core % 4
            zm3 = _zmask(zc).astype(np.float64)  # [3, 128]
            o = r["out"].astype(np.float64)      # [128, 36]
            for slot in range(NSLOT7):
                total += (o[:, slot] * zm3[slot // 12]).sum()
        total /= 4.0
    else:
        for r in res.results:
            total += np.float64(r["out"].astype(np.float64).sum())
        if VARIANT == "v6":
            total /= 4.0
    n = 2 * 158 * 158 * 158
    return np.float32(total / n)



# revision 40
# speedup vs baseline: 1.0021x; 1.0021x over previous
"""Navier-Stokes momentum-residual loss on 8 Trainium2 NeuronCores.

Reference computes, per momentum component m in {z,y,x}:
    R_m = rho*(uz_c*d_dz(u_m) + uy_c*d_dy(u_m) + ux_c*d_dx(u_m))
          + d_dm(p) - MU*lap(u_m)
    loss = sum_m mean(R_m^2)   over the interior [2,158,158,158]

Sharding: 8 cores = (batch b in {0,1}) x (z-chunk zc in {0..3}).  Each core
gets a z-slab of 44 planes [4, 44, 162, 160] (z planes 40*zc .. 40*zc+43,
y padded 160->162, zero-padded out of range).

On-core layout: partition p = y_block*16 + z_loc (8 y-blocks of 20 interior
rows, 16 z-planes per supertile).  3 z-supertiles x 2 x-halves per core.
z-direction stencil terms are computed on the TensorEngine with banded
128x128 matrices (PSUM accumulation); y/x stencils on the VectorEngine via
free-dim AP offsets; squared residuals are summed by the ScalarEngine's
activation(Square, accum_out=...) with a per-partition z-validity mask.
Host sums the per-core [128, NSLOT] partials and divides by N.
"""

import numpy as np

import concourse.bass as bass
import concourse.tile as tile
from concourse import bacc, mybir
from concourse.bass_utils import run_bass_kernel_spmd

try:  # persistent XLA/NEFF compile cache across processes (best effort)
    import jax as _jax
    _jax.config.update("jax_compilation_cache_dir", "/tmp/jax_ns_cache")
    _jax.config.update("jax_persistent_cache_min_entry_size_bytes", -1)
    _jax.config.update("jax_persistent_cache_min_compile_time_secs", 0.0)
except Exception:
    pass

MU = 0.01
RHO = 1.0

# geometry
NZ_SLAB = 44          # z planes per core slab
NY_PAD = 162          # y rows (160 + 2 zero pad)
NX = 160
NSUP = 3              # z supertiles per core
ZSUP = 16             # z planes per supertile (14 interior)
ZINT = 14
NYB = 8               # y blocks
YROWS = 22            # input y rows per block (20 interior + 2 halo)
XTW = 82              # x columns per x-half tile
NSLOT = 6 * 3 * 6     # units * momenta * accum slots


def _band_matrices():
    """lhsT matrices for the z-direction banded matmuls.

    out[p, f] = sum_k lhsT[k, p] * rhs[k, f];  p = yblk*16 + z_loc.
    D:  0.5*(u[z+1] - u[z-1]);  VU: -MU*(u[z+1] + u[z-1]) + 6*MU*u
    (only emitted for interior z_loc 1..14; edge columns all-zero).
    """
    D = np.zeros((128, 128), dtype=np.float32)
    VU = np.zeros((128, 128), dtype=np.float32)
    for p in range(128):
        z = p % ZSUP
        if 1 <= z <= ZINT:
            D[p + 1, p] = 0.5
            D[p - 1, p] = -0.5
            VU[p, p] = 6.0 * MU
            VU[p + 1, p] = -MU
            VU[p - 1, p] = -MU
    return np.concatenate([D, VU], axis=1)  # [128, 256]


def _zmask(zc):
    """[3, 128] validity mask per supertile/partition for core z-chunk zc."""
    smax = min(40, 158 - 40 * zc)
    m = np.zeros((3, 128), dtype=np.float32)
    for k in range(3):
        for p in range(128):
            z = p % ZSUP
            s = 14 * k + z
            if 1 <= z <= ZINT and 1 <= s <= smax:
                m[k, p] = 1.0
    return m


def build_program():
    f32 = mybir.dt.float32
    nc = bacc.Bacc("TRN2", target_bir_lowering=False, debug=False,
                   num_devices=8)
    # pre-packed: [channel, supertile, partition(=yblk*16+z), y_row, x]
    slab = nc.declare_dram_parameter("slab", [4, NSUP, 128, YROWS, NX], f32,
                                     isOutput=False)
    dmats = nc.declare_dram_parameter("dmats", [128, 256], f32, isOutput=False)
    zmask = nc.declare_dram_parameter("zmask", [3, 128], f32, isOutput=False)
    out = nc.declare_dram_parameter("out", [128, NSLOT], f32, isOutput=True)

    AL = mybir.AluOpType
    SQ = mybir.ActivationFunctionType.Square

    with tile.TileContext(nc) as tc:
        with (
            tc.tile_pool(name="const", bufs=1) as cpool,
            tc.tile_pool(name="inp", bufs=2) as inpool,
            tc.tile_pool(name="tmp", bufs=1) as tpool,
            tc.tile_pool(name="ctmp", bufs=2) as ctpool,
            tc.tile_pool(name="psA", bufs=3, space=bass.MemorySpace.PSUM) as psa,
            tc.tile_pool(name="psV", bufs=3, space=bass.MemorySpace.PSUM) as psv,
        ):
            dm = cpool.tile([128, 256], f32, tag="dm")
            nc.sync.dma_start(dm[:], dmats[:])
            zm = cpool.tile([128, 3], f32, tag="zm")
            for k in range(3):
                nc.sync.dma_start(zm[:, k : k + 1], zmask[k, :][:, None])
            acc = cpool.tile([128, NSLOT], f32, tag="acc")
            nc.vector.memset(acc[:], 0.0)

            lhs_D = dm[:, 0:128]
            lhs_VU = dm[:, 128:256]

            unit = 0
            for k in range(3):
                for xh in range(2):
                    x0 = 0 if xh == 0 else 78
                    xo = 1 if xh == 0 else 3   # first out col within tile
                    xn = 80 if xh == 0 else 78  # out col count
                    U = []
                    for c in range(4):
                        t = inpool.tile([128, YROWS, XTW], f32, tag=f"U{c}")
                        nc.sync.dma_start(t[:], slab[c, k, :, :, x0 : x0 + XTW])
                        U.append(t)

                    def cen(c, r0=1, nr=20):
                        return U[c][:, r0 : r0 + nr, xo : xo + xn]

                    def yp(c):
                        return U[c][:, 2:22, xo : xo + xn]

                    def ym(c):
                        return U[c][:, 0:20, xo : xo + xn]

                    def xp(c):
                        return U[c][:, 1:21, xo + 1 : xo + 1 + xn]

                    def xm(c):
                        return U[c][:, 1:21, xo - 1 : xo - 1 + xn]

                    for m in range(3):
                        Dy = tpool.tile([128, 20, 80], f32, tag="dy")
                        nc.vector.tensor_tensor(Dy[:, :, :xn], yp(m), ym(m),
                                                op=AL.subtract)
                        Dx = tpool.tile([128, 20, 80], f32, tag="dx")
                        nc.vector.tensor_tensor(Dx[:, :, :xn], xp(m), xm(m),
                                                op=AL.subtract)
                        NYt = tpool.tile([128, 20, 80], f32, tag="ny")
                        nc.vector.tensor_tensor(NYt[:, :, :xn], yp(m), ym(m),
                                                op=AL.add)
                        NXt = tpool.tile([128, 20, 80], f32, tag="nx")
                        nc.vector.tensor_tensor(NXt[:, :, :xn], xp(m), xm(m),
                                                op=AL.add)
                        T1 = tpool.tile([128, 20, 80], f32, tag="t1")
                        nc.vector.scalar_tensor_tensor(
                            T1[:, :, :xn], Dy[:, :, :xn], 0.5 * RHO, cen(1),
                            op0=AL.mult, op1=AL.mult)
                        T2 = tpool.tile([128, 20, 80], f32, tag="t2")
                        nc.vector.scalar_tensor_tensor(
                            T2[:, :, :xn], Dx[:, :, :xn], 0.5 * RHO, cen(2),
                            op0=AL.mult, op1=AL.mult)
                        S1 = tpool.tile([128, 20, 80], f32, tag="s1")
                        nc.vector.tensor_tensor(S1[:, :, :xn], T1[:, :, :xn],
                                                T2[:, :, :xn], op=AL.add)
                        NS = tpool.tile([128, 20, 80], f32, tag="ns")
                        nc.vector.tensor_tensor(NS[:, :, :xn], NYt[:, :, :xn],
                                                NXt[:, :, :xn], op=AL.add)
                        S2 = tpool.tile([128, 20, 80], f32, tag="s2")
                        nc.vector.scalar_tensor_tensor(
                            S2[:, :, :xn], NS[:, :, :xn], -MU, S1[:, :, :xn],
                            op0=AL.mult, op1=AL.add)
                        Dp = None
                        if m == 1:
                            Dp = tpool.tile([128, 20, 80], f32, tag="dp")
                            nc.vector.tensor_tensor(Dp[:, :, :xn], yp(3), ym(3),
                                                    op=AL.subtract)
                        elif m == 2:
                            Dp = tpool.tile([128, 20, 80], f32, tag="dp")
                            nc.vector.tensor_tensor(Dp[:, :, :xn], xp(3), xm(3),
                                                    op=AL.subtract)

                        for ch in range(4):
                            r0 = 1 + 5 * ch          # input-row of chunk start
                            L = 5 * xn
                            pA = psa.tile([128, 512], f32, tag="psA")
                            nc.tensor.matmul(pA[:, :L], lhs_D, cen(m, r0, 5),
                                             start=True, stop=True)
                            pV = psv.tile([128, 512], f32, tag="psV")
                            if m == 0:
                                nc.tensor.matmul(pV[:, :L], lhs_VU,
                                                 cen(0, r0, 5),
                                                 start=True, stop=False)
                                nc.tensor.matmul(pV[:, :L], lhs_D,
                                                 cen(3, r0, 5),
                                                 start=False, stop=True)
                            else:
                                nc.tensor.matmul(pV[:, :L], lhs_VU,
                                                 cen(m, r0, 5),
                                                 start=True, stop=True)

                            T3 = ctpool.tile([128, 5, 80], f32, tag="t3")
                            nc.vector.tensor_tensor(
                                T3[:, :, :xn], pA[:, :L], cen(0, r0, 5),
                                op=AL.mult)
                            S3 = ctpool.tile([128, 5, 80], f32, tag="s3")
                            nc.vector.tensor_tensor(
                                S3[:, :, :xn],
                                S2[:, 5 * ch : 5 * ch + 5, :xn],
                                T3[:, :, :xn], op=AL.add)
                            R = ctpool.tile([128, 5, 80], f32, tag="s4")
                            if m == 0:
                                nc.vector.tensor_tensor(
                                    R[:, :, :xn], S3[:, :, :xn], pV[:, :L],
                                    op=AL.add)
                            else:
                                S4 = ctpool.tile([128, 5, 80], f32, tag="s4b")
                                nc.vector.tensor_tensor(
                                    S4[:, :, :xn], S3[:, :, :xn], pV[:, :L],
                                    op=AL.add)
                                nc.vector.scalar_tensor_tensor(
                                    R[:, :, :xn],
                                    Dp[:, 5 * ch : 5 * ch + 5, :xn], 0.5,
                                    S4[:, :, :xn], op0=AL.mult, op1=AL.add)

                            sq = ctpool.tile([128, 5, 80], f32, tag="sq")
                            base = (unit * 3 + m) * 6
                            if ch < 3:
                                nc.scalar.activation(
                                    sq[:, :, :xn], R[:, :, :xn], SQ,
                                    scale=zm[:, k : k + 1],
                                    accum_out=acc[:, base + ch : base + ch + 1])
                            else:
                                # rows 16..20: y rows 159,160 are garbage on
                                # y-block 7 (partitions 112..127)
                                nc.scalar.activation(
                                    sq[0:96, :, :xn], R[0:96, :, :xn], SQ,
                                    scale=zm[0:96, k : k + 1],
                                    accum_out=acc[0:96, base + 3 : base + 4])
                                nc.scalar.activation(
                                    sq[96:128, 0:3, :xn], R[96:128, 0:3, :xn],
                                    SQ, scale=zm[96:128, k : k + 1],
                                    accum_out=acc[96:128, base + 4 : base + 5])
                                nc.scalar.activation(
                                    sq[96:112, 3:5, :xn], R[96:112, 3:5, :xn],
                                    SQ, scale=zm[96:112, k : k + 1],
                                    accum_out=acc[96:112, base + 5 : base + 6])
                    unit += 1

            nc.sync.dma_start(out[:], acc[:])
    nc.compile()
    return nc


def _band_matrices_v2():
    """bf16 lhsT matrices, packed [128, 5*128]: D, VU, IP(0.5I), IM(-0.5I),
    IMU(-MU*I)."""
    import ml_dtypes
    D = np.zeros((128, 128), dtype=np.float32)
    VU = np.zeros((128, 128), dtype=np.float32)
    for p in range(128):
        z = p % ZSUP
        if 1 <= z <= ZINT:
            D[p + 1, p] = 0.5
            D[p - 1, p] = -0.5
            VU[p, p] = 6.0 * MU
            VU[p + 1, p] = -MU
            VU[p - 1, p] = -MU
    eye = np.eye(128, dtype=np.float32)
    packed = np.concatenate([D, VU, 0.5 * eye, -0.5 * eye, -MU * eye], axis=1)
    return packed.astype(ml_dtypes.bfloat16)


def _band_matrices_v2():
    """bf16 lhsT matrices packed [128, 5*128]: D, VU, IP(0.5I), IM(-0.5I),
    IMU(-MU*I)."""
    import ml_dtypes
    D = np.zeros((128, 128), dtype=np.float32)
    VU = np.zeros((128, 128), dtype=np.float32)
    for p in range(128):
        z = p % ZSUP
        if 1 <= z <= ZINT:
            D[p + 1, p] = 0.5
            D[p - 1, p] = -0.5
            VU[p, p] = 6.0 * MU
            VU[p + 1, p] = -MU
            VU[p - 1, p] = -MU
    eye = np.eye(128, dtype=np.float32)
    packed = np.concatenate([D, VU, 0.5 * eye, -0.5 * eye, -MU * eye], axis=1)
    return packed.astype(ml_dtypes.bfloat16)


NSLOT2 = 3 * 3 * 8
NRC = 7  # row chunks: six of 3 rows + one of 2


def build_program_v2():
    """bf16 non-conservative variant, engine-balanced.

    Per momentum m the TensorEngine accumulates into PSUM:
      A_m = 0.5*dz(u_m)                                  [banded D]
      V_m = -MU*lap(u_m) + 0.5*d_m(p)   (z-lap banded VU + 6MU center;
            y/x neighbors via -MU*I shifted; dp via D band or +-0.5I shifts)
    The ScalarEngine copies A_m/V_m to bf16 SBUF and does the masked R^2
    accumulation; the VectorEngine (all-bf16 2x ops) does
      Dy, Dx subs; T1=A*uzc; T2=0.5*Dy*uyc; T3=0.5*Dx*uxc;
      S=T1+T2; S2=S+T3; R=S2+V.
    """
    f32 = mybir.dt.float32
    bf16 = mybir.dt.bfloat16
    nc = bacc.Bacc("TRN2", target_bir_lowering=False, debug=False,
                   num_devices=8)
    slab = nc.declare_dram_parameter("slab", [4, NSUP, 128, YROWS, NX], bf16,
                                     isOutput=False)
    dmats = nc.declare_dram_parameter("dmats", [128, 5 * 128], bf16,
                                      isOutput=False)
    zmask = nc.declare_dram_parameter("zmask", [3, 128], f32, isOutput=False)
    out = nc.declare_dram_parameter("out", [128, NSLOT2], f32, isOutput=True)

    AL = mybir.AluOpType
    SQ = mybir.ActivationFunctionType.Square

    with tile.TileContext(nc) as tc:
        with (
            tc.tile_pool(name="const", bufs=1) as cpool,
            tc.tile_pool(name="inp", bufs=2) as inpool,
            tc.tile_pool(name="ctmp", bufs=3) as ctpool,
            tc.tile_pool(name="psA", bufs=1, space=bass.MemorySpace.PSUM) as psa,
            tc.tile_pool(name="psV", bufs=1, space=bass.MemorySpace.PSUM) as psv,
        ):
            dm = cpool.tile([128, 5 * 128], bf16, tag="dm")
            nc.sync.dma_start(dm[:], dmats[:])
            zm = cpool.tile([128, 3], f32, tag="zm")
            for k in range(3):
                nc.sync.dma_start(zm[:, k : k + 1], zmask[k, :][:, None])
            acc = cpool.tile([128, NSLOT2], f32, tag="acc")
            nc.vector.memset(acc[:], 0.0)

            M_D = dm[:, 0:128]
            M_VU = dm[:, 128:256]
            M_IP = dm[:, 256:384]
            M_IM = dm[:, 384:512]
            M_IMU = dm[:, 512:640]

            for k in range(3):
                U = []
                for c in range(4):
                    t = inpool.tile([128, YROWS, NX], bf16, tag=f"U{c}")
                    nc.sync.dma_start(t[:], slab[c, k])
                    U.append(t)

                for rc in range(NRC):
                    r0 = 1 + 3 * rc
                    nr = 3 if rc < 6 else 2
                    NCH = nr * 158

                    def ap(c, dy=0, dx=0):
                        return U[c][:, r0 + dy : r0 + dy + nr,
                                    1 + dx : 159 + dx]

                    # ---- PE ----
                    A = [psa.tile([128, 512], f32, tag=f"psA{m}",
                                  name=f"A{m}_{k}_{rc}", bufs=1)
                         for m in range(3)]
                    V = [psv.tile([128, 512], f32, tag=f"psV{m}",
                                  name=f"V{m}_{k}_{rc}", bufs=1)
                         for m in range(3)]
                    # D group: A_m and dp_z
                    for m in range(3):
                        nc.tensor.matmul(A[m][:, :NCH], M_D, ap(m),
                                         start=True, stop=True)
                    nc.tensor.matmul(V[0][:, :NCH], M_D, ap(3),
                                     start=True, stop=False)
                    # VU group: z-lap + 6MU center
                    for m in range(3):
                        nc.tensor.matmul(V[m][:, :NCH], M_VU, ap(m),
                                         start=(m != 0), stop=False)
                    # IMU group: -MU * (y and x neighbors)
                    for m in range(3):
                        nc.tensor.matmul(V[m][:, :NCH], M_IMU, ap(m, dy=1),
                                         start=False, stop=False)
                        nc.tensor.matmul(V[m][:, :NCH], M_IMU, ap(m, dy=-1),
                                         start=False, stop=False)
                        nc.tensor.matmul(V[m][:, :NCH], M_IMU, ap(m, dx=1),
                                         start=False, stop=False)
                        nc.tensor.matmul(V[m][:, :NCH], M_IMU, ap(m, dx=-1),
                                         start=False, stop=(m == 0))
                    # IP/IM: dp_y, dp_x
                    nc.tensor.matmul(V[1][:, :NCH], M_IP, ap(3, dy=1),
                                     start=False, stop=False)
                    nc.tensor.matmul(V[2][:, :NCH], M_IP, ap(3, dx=1),
                                     start=False, stop=False)
                    nc.tensor.matmul(V[1][:, :NCH], M_IM, ap(3, dy=-1),
                                     start=False, stop=True)
                    nc.tensor.matmul(V[2][:, :NCH], M_IM, ap(3, dx=-1),
                                     start=False, stop=True)

                    # ---- ACT: copy PSUM -> bf16 SBUF ----
                    Ab, Vb = [], []
                    for m in range(3):
                        ab = ctpool.tile([128, 512], bf16, tag=f"ab{m}",
                                         name=f"Ab{m}_{k}_{rc}")
                        nc.scalar.copy(ab[:, :NCH], A[m][:, :NCH])
                        Ab.append(ab)
                        vb = ctpool.tile([128, 512], bf16, tag=f"vb{m}",
                                         name=f"Vb{m}_{k}_{rc}")
                        nc.scalar.copy(vb[:, :NCH], V[m][:, :NCH])
                        Vb.append(vb)

                    # ---- DVE (bf16) ----
                    for m in range(3):
                        Dy = ctpool.tile([128, 3, 158], bf16, tag="dy",
                                         name=f"Dy{m}_{k}_{rc}")
                        nc.vector.tensor_tensor(Dy[:, :nr, :], ap(m, dy=1),
                                                ap(m, dy=-1), op=AL.subtract)
                        Dx = ctpool.tile([128, 3, 158], bf16, tag="dx",
                                         name=f"Dx{m}_{k}_{rc}")
                        nc.vector.tensor_tensor(Dx[:, :nr, :], ap(m, dx=1),
                                                ap(m, dx=-1), op=AL.subtract)
                        T1 = ctpool.tile([128, 512], bf16, tag="t1",
                                         name=f"T1{m}_{k}_{rc}")
                        nc.vector.tensor_tensor(T1[:, :NCH], Ab[m][:, :NCH],
                                                ap(0), op=AL.mult)
                        T2 = ctpool.tile([128, 3, 158], bf16, tag="t2",
                                         name=f"T2{m}_{k}_{rc}")
                        nc.vector.scalar_tensor_tensor(
                            T2[:, :nr, :], Dy[:, :nr, :], 0.5 * RHO, ap(1),
                            op0=AL.mult, op1=AL.mult)
                        T3 = ctpool.tile([128, 3, 158], bf16, tag="t3",
                                         name=f"T3{m}_{k}_{rc}")
                        nc.vector.scalar_tensor_tensor(
                            T3[:, :nr, :], Dx[:, :nr, :], 0.5 * RHO, ap(2),
                            op0=AL.mult, op1=AL.mult)
                        S = ctpool.tile([128, 512], bf16, tag="s",
                                        name=f"S{m}_{k}_{rc}")
                        nc.vector.tensor_tensor(S[:, :NCH], T1[:, :NCH],
                                                T2[:, :nr, :], op=AL.add)
                        S2 = ctpool.tile([128, 512], bf16, tag="s2",
                                         name=f"S2{m}_{k}_{rc}")
                        nc.vector.tensor_tensor(S2[:, :NCH], S[:, :NCH],
                                                T3[:, :nr, :], op=AL.add)
                        R = ctpool.tile([128, 512], bf16, tag="r",
                                        name=f"R{m}_{k}_{rc}")
                        nc.vector.tensor_tensor(R[:, :NCH], S2[:, :NCH],
                                                Vb[m][:, :NCH], op=AL.add)

                        # ---- ACT: masked square-accumulate ----
                        sq = ctpool.tile([128, 512], bf16, tag="sq",
                                         name=f"sq{m}_{k}_{rc}")
                        base = (k * 3 + m) * 8
                        if rc < 6:
                            nc.scalar.activation(
                                sq[:, :NCH], R[:, :NCH], SQ,
                                scale=zm[:, k : k + 1],
                                accum_out=acc[:, base + rc : base + rc + 1])
                        else:
                            # rows 19,20: garbage on y-block 7 (parts 112-127)
                            nc.scalar.activation(
                                sq[0:96, :NCH], R[0:96, :NCH], SQ,
                                scale=zm[0:96, k : k + 1],
                                accum_out=acc[0:96, base + 6 : base + 7])
                            nc.scalar.activation(
                                sq[96:112, :NCH], R[96:112, :NCH], SQ,
                                scale=zm[96:112, k : k + 1],
                                accum_out=acc[96:112, base + 7 : base + 8])

            nc.sync.dma_start(out[:], acc[:])
    nc.compile()
    return nc


NSLOT3 = 3 * 3 * 2


def _amask(zc):
    """[128, NSLOT3] end-mask: slot = (k*3+m)*2 + j; j=0 rows 1-18, j=1 rows
    19-20 (garbage on y-block 7 = partitions 112..127)."""
    zm = _zmask(zc)  # [3, 128]
    m = np.zeros((128, NSLOT3), dtype=np.float32)
    for k in range(3):
        for mm in range(3):
            for j in range(2):
                s = (k * 3 + mm) * 2 + j
                col = zm[k].copy()
                if j == 1:
                    col[112:] = 0.0
                m[:, s] = col
    return m


def build_program_v3():
    """Like v2 but with full-supertile DVE ops (amortizes the per-op pipeline
    bubble), in-place tile reuse, tensor_scalar pre-scales instead of
    scalar_tensor_tensor, ACT squares without per-op masks, and one end-mask
    multiply on the [128, NSLOT3] partial sums."""
    f32 = mybir.dt.float32
    bf16 = mybir.dt.bfloat16
    nc = bacc.Bacc("TRN2", target_bir_lowering=False, debug=False,
                   num_devices=8)
    slab = nc.declare_dram_parameter("slab", [4, NSUP, 128, YROWS, NX], bf16,
                                     isOutput=False)
    dmats = nc.declare_dram_parameter("dmats", [128, 5 * 128], bf16,
                                      isOutput=False)
    amask = nc.declare_dram_parameter("amask", [128, NSLOT3], f32,
                                      isOutput=False)
    out = nc.declare_dram_parameter("out", [128, NSLOT3], f32, isOutput=True)

    AL = mybir.AluOpType
    SQ = mybir.ActivationFunctionType.Square

    with tile.TileContext(nc) as tc:
        with (
            tc.tile_pool(name="const", bufs=1) as cpool,
            tc.tile_pool(name="inp", bufs=2) as inpool,
            tc.tile_pool(name="fld", bufs=2) as fpool,
            tc.tile_pool(name="psA", bufs=1, space=bass.MemorySpace.PSUM) as psa,
            tc.tile_pool(name="psV", bufs=1, space=bass.MemorySpace.PSUM) as psv,
        ):
            dm = cpool.tile([128, 5 * 128], bf16, tag="dm")
            nc.sync.dma_start(dm[:], dmats[:])
            am = cpool.tile([128, NSLOT3], f32, tag="am")
            nc.sync.dma_start(am[:], amask[:])
            acc = cpool.tile([128, NSLOT3], f32, tag="acc")

            M_D = dm[:, 0:128]
            M_VU = dm[:, 128:256]
            M_IP = dm[:, 256:384]
            M_IM = dm[:, 384:512]
            M_IMU = dm[:, 512:640]

            for k in range(3):
                U = []
                for c in range(4):
                    t = inpool.tile([128, YROWS, NX], bf16, tag=f"U{c}")
                    nc.sync.dma_start(t[:], slab[c, k])
                    U.append(t)

                # pre-scaled center factors 0.5*uy, 0.5*ux (full interior)
                HUY = fpool.tile([128, 20, 158], bf16, tag="huy")
                nc.vector.tensor_scalar_mul(HUY[:], U[1][:, 1:21, 1:159],
                                            0.5 * RHO)
                HUX = fpool.tile([128, 20, 158], bf16, tag="hux")
                nc.vector.tensor_scalar_mul(HUX[:], U[2][:, 1:21, 1:159],
                                            0.5 * RHO)

                Ab, Vb = [], []
                for m in range(3):
                    ab = fpool.tile([128, 20, 158], bf16, tag=f"ab{m}",
                                    name=f"Ab{m}_{k}")
                    Ab.append(ab)
                    vb = fpool.tile([128, 20, 158], bf16, tag=f"vb{m}",
                                    name=f"Vb{m}_{k}")
                    Vb.append(vb)

                for rc in range(NRC):
                    r0 = 1 + 3 * rc
                    nr = 3 if rc < 6 else 2
                    NCH = nr * 158

                    def ap(c, dy=0, dx=0):
                        return U[c][:, r0 + dy : r0 + dy + nr,
                                    1 + dx : 159 + dx]

                    A = [psa.tile([128, 512], f32, tag=f"psA{m}",
                                  name=f"A{m}_{k}_{rc}")
                         for m in range(3)]
                    V = [psv.tile([128, 512], f32, tag=f"psV{m}",
                                  name=f"V{m}_{k}_{rc}")
                         for m in range(3)]
                    for m in range(3):
                        nc.tensor.matmul(A[m][:, :NCH], M_D, ap(m),
                                         start=True, stop=True)
                    nc.tensor.matmul(V[0][:, :NCH], M_D, ap(3),
                                     start=True, stop=False)
                    for m in range(3):
                        nc.tensor.matmul(V[m][:, :NCH], M_VU, ap(m),
                                         start=(m != 0), stop=False)
                    for m in range(3):
                        nc.tensor.matmul(V[m][:, :NCH], M_IMU, ap(m, dy=1),
                                         start=False, stop=False)
                        nc.tensor.matmul(V[m][:, :NCH], M_IMU, ap(m, dy=-1),
                                         start=False, stop=False)
                        nc.tensor.matmul(V[m][:, :NCH], M_IMU, ap(m, dx=1),
                                         start=False, stop=False)
                        nc.tensor.matmul(V[m][:, :NCH], M_IMU, ap(m, dx=-1),
                                         start=False, stop=(m == 0))
                    nc.tensor.matmul(V[1][:, :NCH], M_IP, ap(3, dy=1),
                                     start=False, stop=False)
                    nc.tensor.matmul(V[2][:, :NCH], M_IP, ap(3, dx=1),
                                     start=False, stop=False)
                    nc.tensor.matmul(V[1][:, :NCH], M_IM, ap(3, dy=-1),
                                     start=False, stop=True)
                    nc.tensor.matmul(V[2][:, :NCH], M_IM, ap(3, dx=-1),
                                     start=False, stop=True)

                    # ACT: drain PSUM chunks into the full-supertile tiles
                    rows = slice(r0 - 1, r0 - 1 + nr)
                    for m in range(3):
                        nc.scalar.copy(Ab[m][:, rows, :], A[m][:, :NCH])
                        nc.scalar.copy(Vb[m][:, rows, :], V[m][:, :NCH])

                # DVE: full-supertile assembly (in-place chains)
                for m in range(3):
                    Dy = fpool.tile([128, 20, 158], bf16, tag="dy",
                                    name=f"Dy{m}_{k}")
                    nc.vector.tensor_tensor(Dy[:], U[m][:, 2:22, 1:159],
                                            U[m][:, 0:20, 1:159],
                                            op=AL.subtract)
                    Dx = fpool.tile([128, 20, 158], bf16, tag="dx",
                                    name=f"Dx{m}_{k}")
                    nc.vector.tensor_tensor(Dx[:], U[m][:, 1:21, 2:160],
                                            U[m][:, 1:21, 0:158],
                                            op=AL.subtract)
                    # T1 = Ab*uzc (in place over Ab)
                    nc.vector.tensor_tensor(Ab[m][:], Ab[m][:],
                                            U[0][:, 1:21, 1:159], op=AL.mult)
                    # T2 = Dy*0.5uy (in place over Dy); T3 likewise
                    nc.vector.tensor_tensor(Dy[:], Dy[:], HUY[:], op=AL.mult)
                    nc.vector.tensor_tensor(Dx[:], Dx[:], HUX[:], op=AL.mult)
                    # S = T1+T2 -> Ab; S2 = S+T3 -> Ab; R = S2+Vb -> Vb
                    nc.vector.tensor_tensor(Ab[m][:], Ab[m][:], Dy[:],
                                            op=AL.add)
                    nc.vector.tensor_tensor(Ab[m][:], Ab[m][:], Dx[:],
                                            op=AL.add)
                    nc.vector.tensor_tensor(Vb[m][:], Ab[m][:], Vb[m][:],
                                            op=AL.add)

                    # ACT: plain square-accumulate, split rows 1-18 / 19-20
                    s = (k * 3 + m) * 2
                    sq = fpool.tile([128, 20, 158], bf16, tag="sq",
                                    name=f"sq{m}_{k}")
                    nc.scalar.activation(sq[:, 0:18, :], Vb[m][:, 0:18, :],
                                         SQ, accum_out=acc[:, s : s + 1])
                    nc.scalar.activation(sq[:, 18:20, :], Vb[m][:, 18:20, :],
                                         SQ, accum_out=acc[:, s + 1 : s + 2])

            # end-mask and ship
            nc.vector.tensor_tensor(acc[:], acc[:], am[:], op=AL.mult)
            nc.sync.dma_start(out[:], acc[:])
    nc.compile()
    return nc


def build_program_v5():
    """Like v2 but with full-supertile DVE ops (amortizes the per-op pipeline
    bubble), in-place tile reuse, tensor_scalar pre-scales instead of
    scalar_tensor_tensor, ACT squares without per-op masks, and one end-mask
    multiply on the [128, NSLOT3] partial sums."""
    f32 = mybir.dt.float32
    bf16 = mybir.dt.bfloat16
    nc = bacc.Bacc("TRN2", target_bir_lowering=False, debug=False,
                   num_devices=8)
    slab = nc.declare_dram_parameter("slab", [4, NSUP, 128, YROWS, NX], bf16,
                                     isOutput=False)
    dmats = nc.declare_dram_parameter("dmats", [128, 5 * 128], bf16,
                                      isOutput=False)
    amask = nc.declare_dram_parameter("amask", [128, NSLOT3], f32,
                                      isOutput=False)
    out = nc.declare_dram_parameter("out", [128, NSLOT3], f32, isOutput=True)

    AL = mybir.AluOpType
    SQ = mybir.ActivationFunctionType.Square

    with tile.TileContext(nc) as tc:
        with (
            tc.tile_pool(name="const", bufs=1) as cpool,
            tc.tile_pool(name="inp", bufs=2) as inpool,
            tc.tile_pool(name="fld", bufs=2) as fpool,
            tc.tile_pool(name="psA", bufs=1, space=bass.MemorySpace.PSUM) as psa,
            tc.tile_pool(name="psV", bufs=1, space=bass.MemorySpace.PSUM) as psv,
        ):
            dm = cpool.tile([128, 5 * 128], bf16, tag="dm")
            nc.sync.dma_start(dm[:], dmats[:])
            am = cpool.tile([128, NSLOT3], f32, tag="am")
            nc.sync.dma_start(am[:], amask[:])
            acc = cpool.tile([128, NSLOT3], f32, tag="acc")

            M_D = dm[:, 0:128]
            M_VU = dm[:, 128:256]
            M_IP = dm[:, 256:384]
            M_IM = dm[:, 384:512]
            M_IMU = dm[:, 512:640]

            for k in range(3):
                U = []
                for c in range(4):
                    t = inpool.tile([128, YROWS, NX], bf16, tag=f"U{c}")
                    nc.sync.dma_start(t[:], slab[c, k])
                    U.append(t)

                # pre-scaled center factors 0.5*uy, 0.5*ux (full interior)
                HUY = fpool.tile([128, 20, 158], bf16, tag="huy")
                nc.vector.tensor_scalar_mul(HUY[:], U[1][:, 1:21, 1:159],
                                            0.5 * RHO)
                HUX = fpool.tile([128, 20, 158], bf16, tag="hux")
                nc.vector.tensor_scalar_mul(HUX[:], U[2][:, 1:21, 1:159],
                                            0.5 * RHO)

                Ab, Vb = [], []
                for m in range(3):
                    ab = fpool.tile([128, 20, 158], bf16, tag=f"ab{m}",
                                    name=f"Ab{m}_{k}")
                    Ab.append(ab)
                    vb = fpool.tile([128, 20, 158], bf16, tag=f"vb{m}",
                                    name=f"Vb{m}_{k}")
                    Vb.append(vb)

                for rc in range(NRC):
                    r0 = 1 + 3 * rc
                    nr = 3 if rc < 6 else 2
                    NCH = nr * 158

                    def ap(c, dy=0, dx=0):
                        return U[c][:, r0 + dy : r0 + dy + nr,
                                    1 + dx : 159 + dx]

                    A = [psa.tile([128, 512], f32, tag=f"psA{m}",
                                  name=f"A{m}_{k}_{rc}", bufs=1)
                         for m in range(3)]
                    V = [psv.tile([128, 512], f32, tag=f"psV{m}",
                                  name=f"V{m}_{k}_{rc}",
                                  bufs=(2 if m < 2 else 1))
                         for m in range(3)]
                    for m in range(3):
                        nc.tensor.matmul(A[m][:, :NCH], M_D, ap(m),
                                         start=True, stop=True)
                    nc.tensor.matmul(V[0][:, :NCH], M_D, ap(3),
                                     start=True, stop=False)
                    for m in range(3):
                        nc.tensor.matmul(V[m][:, :NCH], M_VU, ap(m),
                                         start=(m != 0), stop=False)
                    for m in range(3):
                        nc.tensor.matmul(V[m][:, :NCH], M_IMU, ap(m, dy=1),
                                         start=False, stop=False)
                        nc.tensor.matmul(V[m][:, :NCH], M_IMU, ap(m, dy=-1),
                                         start=False, stop=False)
                        nc.tensor.matmul(V[m][:, :NCH], M_IMU, ap(m, dx=1),
                                         start=False, stop=False)
                        nc.tensor.matmul(V[m][:, :NCH], M_IMU, ap(m, dx=-1),
                                         start=False, stop=(m == 0))
                    nc.tensor.matmul(V[1][:, :NCH], M_IP, ap(3, dy=1),
                                     start=False, stop=False)
                    nc.tensor.matmul(V[2][:, :NCH], M_IP, ap(3, dx=1),
                                     start=False, stop=False)
                    nc.tensor.matmul(V[1][:, :NCH], M_IM, ap(3, dy=-1),
                                     start=False, stop=True)
                    nc.tensor.matmul(V[2][:, :NCH], M_IM, ap(3, dx=-1),
                                     start=False, stop=True)

                    # ACT: drain PSUM chunks into the full-supertile tiles
                    rows = slice(r0 - 1, r0 - 1 + nr)
                    for m in range(3):
                        nc.scalar.copy(Ab[m][:, rows, :], A[m][:, :NCH])
                        nc.scalar.copy(Vb[m][:, rows, :], V[m][:, :NCH])

                # DVE: full-supertile assembly (in-place chains)
                for m in range(3):
                    Dy = fpool.tile([128, 20, 158], bf16, tag="dy",
                                    name=f"Dy{m}_{k}")
                    nc.vector.tensor_tensor(Dy[:], U[m][:, 2:22, 1:159],
                                            U[m][:, 0:20, 1:159],
                                            op=AL.subtract)
                    Dx = fpool.tile([128, 20, 158], bf16, tag="dx",
                                    name=f"Dx{m}_{k}")
                    nc.vector.tensor_tensor(Dx[:], U[m][:, 1:21, 2:160],
                                            U[m][:, 1:21, 0:158],
                                            op=AL.subtract)
                    # T1 = Ab*uzc (in place over Ab)
                    nc.vector.tensor_tensor(Ab[m][:], Ab[m][:],
                                            U[0][:, 1:21, 1:159], op=AL.mult)
                    # T2 = Dy*0.5uy (in place over Dy); T3 likewise
                    nc.vector.tensor_tensor(Dy[:], Dy[:], HUY[:], op=AL.mult)
                    nc.vector.tensor_tensor(Dx[:], Dx[:], HUX[:], op=AL.mult)
                    # S = T1+T2 -> Ab; S2 = S+T3 -> Ab; R = S2+Vb -> Vb
                    nc.vector.tensor_tensor(Ab[m][:], Ab[m][:], Dy[:],
                                            op=AL.add)
                    nc.vector.tensor_tensor(Ab[m][:], Ab[m][:], Dx[:],
                                            op=AL.add)
                    nc.vector.tensor_tensor(Vb[m][:], Ab[m][:], Vb[m][:],
                                            op=AL.add)

                    # ACT: plain square-accumulate, split rows 1-18 / 19-20
                    s = (k * 3 + m) * 2
                    sq = fpool.tile([128, 20, 158], bf16, tag="sq",
                                    name=f"sq{m}_{k}")
                    nc.scalar.activation(sq[:, 0:18, :], Vb[m][:, 0:18, :],
                                         SQ, accum_out=acc[:, s : s + 1])
                    nc.scalar.activation(sq[:, 18:20, :], Vb[m][:, 18:20, :],
                                         SQ, accum_out=acc[:, s + 1 : s + 2])

            # end-mask and ship
            nc.vector.tensor_tensor(acc[:], acc[:], am[:], op=AL.mult)
            nc.sync.dma_start(out[:], acc[:])
    nc.compile()
    return nc




def build_program_v4():
    """Like v2 but with full-supertile DVE ops (amortizes the per-op pipeline
    bubble), in-place tile reuse, tensor_scalar pre-scales instead of
    scalar_tensor_tensor, ACT squares without per-op masks, and one end-mask
    multiply on the [128, NSLOT3] partial sums."""
    f32 = mybir.dt.float32
    bf16 = mybir.dt.bfloat16
    nc = bacc.Bacc("TRN2", target_bir_lowering=False, debug=False,
                   num_devices=8)
    slab = nc.declare_dram_parameter("slab", [4, NSUP, 128, YROWS, NX], bf16,
                                     isOutput=False)
    dmats = nc.declare_dram_parameter("dmats", [128, 5 * 128], bf16,
                                      isOutput=False)
    amask = nc.declare_dram_parameter("amask", [128, NSLOT3], f32,
                                      isOutput=False)
    out = nc.declare_dram_parameter("out", [128, NSLOT3], f32, isOutput=True)

    AL = mybir.AluOpType
    SQ = mybir.ActivationFunctionType.Square

    with tile.TileContext(nc) as tc:
        with (
            tc.tile_pool(name="const", bufs=1) as cpool,
            tc.tile_pool(name="inp", bufs=2) as inpool,
            tc.tile_pool(name="fld", bufs=2) as fpool,
            tc.tile_pool(name="psAV", bufs=1, space=bass.MemorySpace.PSUM) as psav,
        ):
            dm = cpool.tile([128, 5 * 128], bf16, tag="dm")
            nc.sync.dma_start(dm[:], dmats[:])
            am = cpool.tile([128, NSLOT3], f32, tag="am")
            nc.sync.dma_start(am[:], amask[:])
            acc = cpool.tile([128, NSLOT3], f32, tag="acc")

            M_D = dm[:, 0:128]
            M_VU = dm[:, 128:256]
            M_IP = dm[:, 256:384]
            M_IM = dm[:, 384:512]
            M_IMU = dm[:, 512:640]

            for k in range(3):
                U = []
                for c in range(4):
                    t = inpool.tile([128, YROWS, NX], bf16, tag=f"U{c}")
                    nc.sync.dma_start(t[:], slab[c, k])
                    U.append(t)

                # pre-scaled center factors 0.5*uy, 0.5*ux (full interior)
                HUY = fpool.tile([128, 20, 158], bf16, tag="huy")
                nc.vector.tensor_scalar_mul(HUY[:], U[1][:, 1:21, 1:159],
                                            0.5 * RHO)
                HUX = fpool.tile([128, 20, 158], bf16, tag="hux")
                nc.vector.tensor_scalar_mul(HUX[:], U[2][:, 1:21, 1:159],
                                            0.5 * RHO)

                AVb = [fpool.tile([128, 2, 20, 158], bf16, tag=f"avb{m}",
                                  name=f"AVb{m}_{k}") for m in range(3)]
                Ab = [t[:, 0] for t in AVb]
                Vb = [t[:, 1] for t in AVb]

                for rc in range(NRC):
                    r0 = 1 + 3 * rc
                    nr = 3 if rc < 6 else 2
                    NCH = nr * 158

                    def ap(c, dy=0, dx=0):
                        return U[c][:, r0 + dy : r0 + dy + nr,
                                    1 + dx : 159 + dx]

                    AV = [psav.tile([128, 1024], f32, tag=f"psAV{m}",
                                    name=f"AV{m}_{k}_{rc}")
                          for m in range(3)]
                    A = [t[:, 0:512] for t in AV]
                    V = [t[:, 512:1024] for t in AV]
                    for m in range(3):
                        nc.tensor.matmul(A[m][:, :NCH], M_D, ap(m),
                                         start=True, stop=True)
                    nc.tensor.matmul(V[0][:, :NCH], M_D, ap(3),
                                     start=True, stop=False)
                    for m in range(3):
                        nc.tensor.matmul(V[m][:, :NCH], M_VU, ap(m),
                                         start=(m != 0), stop=False)
                    for m in range(3):
                        nc.tensor.matmul(V[m][:, :NCH], M_IMU, ap(m, dy=1),
                                         start=False, stop=False)
                        nc.tensor.matmul(V[m][:, :NCH], M_IMU, ap(m, dy=-1),
                                         start=False, stop=False)
                        nc.tensor.matmul(V[m][:, :NCH], M_IMU, ap(m, dx=1),
                                         start=False, stop=False)
                        nc.tensor.matmul(V[m][:, :NCH], M_IMU, ap(m, dx=-1),
                                         start=False, stop=(m == 0))
                    nc.tensor.matmul(V[1][:, :NCH], M_IP, ap(3, dy=1),
                                     start=False, stop=False)
                    nc.tensor.matmul(V[2][:, :NCH], M_IP, ap(3, dx=1),
                                     start=False, stop=False)
                    nc.tensor.matmul(V[1][:, :NCH], M_IM, ap(3, dy=-1),
                                     start=False, stop=True)
                    nc.tensor.matmul(V[2][:, :NCH], M_IM, ap(3, dx=-1),
                                     start=False, stop=True)

                    # ACT: drain PSUM chunks into the full-supertile tiles
                    rows = slice(r0 - 1, r0 - 1 + nr)
                    for m in range(3):
                        src2 = AV[m].rearrange("p (b n) -> p b n", b=2)
                        nc.scalar.copy(AVb[m][:, :, rows, :],
                                       src2[:, :, :NCH])

                # DVE: full-supertile assembly (in-place chains)
                for m in range(3):
                    Dy = fpool.tile([128, 20, 158], bf16, tag="dy",
                                    name=f"Dy{m}_{k}")
                    nc.vector.tensor_tensor(Dy[:], U[m][:, 2:22, 1:159],
                                            U[m][:, 0:20, 1:159],
                                            op=AL.subtract)
                    Dx = fpool.tile([128, 20, 158], bf16, tag="dx",
                                    name=f"Dx{m}_{k}")
                    nc.vector.tensor_tensor(Dx[:], U[m][:, 1:21, 2:160],
                                            U[m][:, 1:21, 0:158],
                                            op=AL.subtract)
                    # T1 = Ab*uzc (in place over Ab)
                    nc.vector.tensor_tensor(Ab[m][:], Ab[m][:],
                                            U[0][:, 1:21, 1:159], op=AL.mult)
                    # T2 = Dy*0.5uy (in place over Dy); T3 likewise
                    nc.vector.tensor_tensor(Dy[:], Dy[:], HUY[:], op=AL.mult)
                    nc.vector.tensor_tensor(Dx[:], Dx[:], HUX[:], op=AL.mult)
                    # S = T1+T2 -> Ab; S2 = S+T3 -> Ab; R = S2+Vb -> Vb
                    nc.vector.tensor_tensor(Ab[m][:], Ab[m][:], Dy[:],
                                            op=AL.add)
                    nc.vector.tensor_tensor(Ab[m][:], Ab[m][:], Dx[:],
                                            op=AL.add)
                    nc.vector.tensor_tensor(Vb[m][:], Ab[m][:], Vb[m][:],
                                            op=AL.add)

                    # ACT: plain square-accumulate, split rows 1-18 / 19-20
                    s = (k * 3 + m) * 2
                    sq = fpool.tile([128, 20, 158], bf16, tag="sq",
                                    name=f"sq{m}_{k}")
                    nc.scalar.activation(sq[:, 0:18, :], Vb[m][:, 0:18, :],
                                         SQ, accum_out=acc[:, s : s + 1])
                    nc.scalar.activation(sq[:, 18:20, :], Vb[m][:, 18:20, :],
                                         SQ, accum_out=acc[:, s + 1 : s + 2])

            # end-mask and ship
            nc.vector.tensor_tensor(acc[:], acc[:], am[:], op=AL.mult)
            nc.sync.dma_start(out[:], acc[:])
    nc.compile()
    return nc




def _band_matrices_x2():
    """bf16 matrices packed [128, 5*128], all scaled x2 vs _band_matrices_v2:
    D2 (dz band, +-1), VU2 (12MU diag / -2MU off), IP2 (+I), IM2 (-I),
    IMU2 (-2MU*I).  Kernel computes R' = 2R; host divides the loss by 4."""
    import ml_dtypes
    D = np.zeros((128, 128), dtype=np.float32)
    VU = np.zeros((128, 128), dtype=np.float32)
    for p in range(128):
        z = p % ZSUP
        if 1 <= z <= ZINT:
            D[p + 1, p] = 1.0
            D[p - 1, p] = -1.0
            VU[p, p] = 12.0 * MU
            VU[p + 1, p] = -2.0 * MU
            VU[p - 1, p] = -2.0 * MU
    eye = np.eye(128, dtype=np.float32)
    packed = np.concatenate([D, VU, eye, -eye, -2.0 * MU * eye], axis=1)
    return packed.astype(ml_dtypes.bfloat16)


NSLOT6 = 3 * 3 * 2  # (supertile x momentum) x 2 row-groups


def build_program_v6():
    """All-STT DVE chain (4x mode) + fused masked square-accum on DVE;
    ACT only drains PSUM->SBUF with multi-bank strided copies; PE is the
    v5 banded bf16 scheme with x2 weights (loss /4 on host)."""
    f32 = mybir.dt.float32
    bf16 = mybir.dt.bfloat16
    nc = bacc.Bacc("TRN2", target_bir_lowering=False, debug=False,
                   num_devices=8)
    # host repacks channel-inside-partition: [NSUP, 128, 4, YROWS, NX]
    slab = nc.declare_dram_parameter("slab", [NSUP, 128, 4, YROWS, NX], bf16,
                                     isOutput=False)
    dmats = nc.declare_dram_parameter("dmats", [128, 5 * 128], bf16,
                                      isOutput=False)
    zmask = nc.declare_dram_parameter("zmask", [3, 128], f32, isOutput=False)
    out = nc.declare_dram_parameter("out", [128, NSLOT6], f32, isOutput=True)

    AL = mybir.AluOpType

    with tile.TileContext(nc) as tc:
        with (
            tc.tile_pool(name="const", bufs=1) as cpool,
            tc.tile_pool(name="inp", bufs=2) as inpool,
            tc.tile_pool(name="drn", bufs=2) as dpool,
            tc.tile_pool(name="tmp", bufs=2) as tpool,
            tc.tile_pool(name="plo", bufs=1, space=bass.MemorySpace.PSUM) as plo,
            tc.tile_pool(name="phi", bufs=1, space=bass.MemorySpace.PSUM) as phi,
        ):
            dm = cpool.tile([128, 5 * 128], bf16, tag="dm")
            nc.sync.dma_start(dm[:], dmats[:])
            zm = cpool.tile([128, 3], f32, tag="zm")
            for k in range(3):
                nc.sync.dma_start(zm[:, k : k + 1], zmask[k, :][:, None])
            acc = cpool.tile([128, NSLOT6], f32, tag="acc")
            nc.vector.memset(acc[:], 0.0)

            M_D = dm[:, 0:128]
            M_VU = dm[:, 128:256]
            M_IP = dm[:, 256:384]
            M_IM = dm[:, 384:512]
            M_IMU = dm[:, 512:640]

            for k in range(3):
                U = inpool.tile([128, 4, YROWS, NX], bf16, tag="U",
                                name=f"U_{k}")
                nc.sync.dma_start(U[:], slab[k])

                def ap(c, rc, dy=0, dx=0):
                    r0 = 1 + 3 * rc
                    nr = 3 if rc < 6 else 2
                    return U[:, c, r0 + dy : r0 + dy + nr, 1 + dx : 159 + dx]

                def cen(c):
                    return U[:, c, 1:21, 1:159]

                for m in range(3):
                    A2b = dpool.tile([128, 20, 158], bf16, tag="a2b",
                                     name=f"A2b{m}_{k}")
                    Vb = dpool.tile([128, 20, 158], bf16, tag="vb",
                                    name=f"Vb{m}_{k}")
                    # ---- PE: A then V, chunked into lo(0-3)/hi(4-6) banks
                    for half, rng, ptag in ((0, range(0, 4), "alo"),
                                            (1, range(4, 7), "ahi")):
                        pool_ = plo if half == 0 else phi
                        nb = 4 if half == 0 else 3
                        At = pool_.tile([128, nb, 512], f32, tag=f"p{half}",
                                        name=f"A{m}_{k}_{half}")
                        for c in rng:
                            nr = 3 if c < 6 else 2
                            NCH = nr * 158
                            nc.tensor.matmul(At[:, c - (0 if half == 0 else 4),
                                                :NCH],
                                             M_D, ap(m, c),
                                             start=True, stop=True)
                        # drain this half
                        if half == 0:
                            nc.scalar.copy(A2b[:, 0:12, :], At[:, :, 0:474])
                        else:
                            nc.scalar.copy(A2b[:, 12:18, :],
                                           At[:, 0:2, 0:474])
                            nc.scalar.copy(A2b[:, 18:20, :],
                                           At[:, 2:3, 0:316])
                    for half, rng in ((0, range(0, 4)), (1, range(4, 7))):
                        pool_ = plo if half == 0 else phi
                        nb = 4 if half == 0 else 3
                        Vt = pool_.tile([128, nb, 512], f32, tag=f"p{half}",
                                        name=f"V{m}_{k}_{half}")
                        for c in rng:
                            nr = 3 if c < 6 else 2
                            NCH = nr * 158
                            vt = Vt[:, c - (0 if half == 0 else 4), :NCH]
                            nc.tensor.matmul(vt, M_VU, ap(m, c),
                                             start=True, stop=False)
                            nc.tensor.matmul(vt, M_IMU, ap(m, c, dy=1),
                                             start=False, stop=False)
                            nc.tensor.matmul(vt, M_IMU, ap(m, c, dy=-1),
                                             start=False, stop=False)
                            nc.tensor.matmul(vt, M_IMU, ap(m, c, dx=1),
                                             start=False, stop=False)
                            nc.tensor.matmul(vt, M_IMU, ap(m, c, dx=-1),
                                             start=False, stop=False)
                            if m == 0:
                                nc.tensor.matmul(vt, M_D, ap(3, c),
                                                 start=False, stop=True)
                            elif m == 1:
                                nc.tensor.matmul(vt, M_IP, ap(3, c, dy=1),
                                                 start=False, stop=False)
                                nc.tensor.matmul(vt, M_IM, ap(3, c, dy=-1),
                                                 start=False, stop=True)
                            else:
                                nc.tensor.matmul(vt, M_IP, ap(3, c, dx=1),
                                                 start=False, stop=False)
                                nc.tensor.matmul(vt, M_IM, ap(3, c, dx=-1),
                                                 start=False, stop=True)
                        if half == 0:
                            nc.scalar.copy(Vb[:, 0:12, :], Vt[:, :, 0:474])
                        else:
                            nc.scalar.copy(Vb[:, 12:18, :], Vt[:, 0:2, 0:474])
                            nc.scalar.copy(Vb[:, 18:20, :],
                                           Vt[:, 2:3, 0:316])

                    # ---- DVE: STT chain, all 4x ----
                    def t20(tag):
                        return tpool.tile([128, 20, 158], bf16, tag=tag,
                                          name=f"{tag}{m}_{k}")

                    yp = U[:, m, 2:22, 1:159]
                    ym = U[:, m, 0:20, 1:159]
                    xp = U[:, m, 1:21, 2:160]
                    xm = U[:, m, 1:21, 0:158]
                    Dy = t20("dy")
                    nc.vector.scalar_tensor_tensor(Dy[:], yp, 1.0, ym,
                                                   op0=AL.mult,
                                                   op1=AL.subtract)
                    Dx = t20("dx")
                    nc.vector.scalar_tensor_tensor(Dx[:], xp, 1.0, xm,
                                                   op0=AL.mult,
                                                   op1=AL.subtract)
                    M1 = t20("m1")
                    nc.vector.scalar_tensor_tensor(M1[:], Dy[:], 1.0, cen(1),
                                                   op0=AL.mult, op1=AL.mult)
                    M2 = t20("m2")
                    nc.vector.scalar_tensor_tensor(M2[:], Dx[:], 1.0, cen(2),
                                                   op0=AL.mult, op1=AL.mult)
                    T1 = t20("t1")
                    nc.vector.scalar_tensor_tensor(T1[:], A2b[:], 1.0, cen(0),
                                                   op0=AL.mult, op1=AL.mult)
                    S1 = t20("s1")
                    nc.vector.scalar_tensor_tensor(S1[:], M1[:], 1.0, M2[:],
                                                   op0=AL.mult, op1=AL.add)
                    S2 = t20("s2")
                    nc.vector.scalar_tensor_tensor(S2[:], S1[:], 1.0, T1[:],
                                                   op0=AL.mult, op1=AL.add)
                    R = t20("r")
                    nc.vector.scalar_tensor_tensor(R[:], S2[:], 1.0, Vb[:],
                                                   op0=AL.mult, op1=AL.add)
                    # fused masked square + accumulate (zm scales once)
                    sq = t20("sq")
                    s = (k * 3 + m) * 2
                    nc.vector.scalar_tensor_tensor(
                        sq[:, 0:18, :], R[:, 0:18, :], zm[:, k : k + 1],
                        R[:, 0:18, :], op0=AL.mult, op1=AL.mult,
                        accum_out=acc[:, s : s + 1])
                    nc.vector.scalar_tensor_tensor(
                        sq[0:112, 18:20, :], R[0:112, 18:20, :],
                        zm[0:112, k : k + 1], R[0:112, 18:20, :],
                        op0=AL.mult, op1=AL.mult,
                        accum_out=acc[0:112, s + 1 : s + 2])

            nc.sync.dma_start(out[:], acc[:])
    nc.compile()
    return nc


# ---------------------------------------------------------------------------
# v7: fp8 DoubleRow PE stencils + bf16 identity-matmul adds into V-PSUM,
# DVE tensor_tensor products at 2x, gpsimd M2 product, ACT drains + squares.
# x-half units, 4-bank PSUM regions rotated 2-deep.

NSLOT7 = 3 * 3 * 2 * 2  # k x m x xh x rowgroup


def _w_bands_v7():
    """fp8 weight pair tensor [128, 14, 2, 128] + bf16 identity [128,128].

    All stencil weights are x2 (kernel computes R' = 2R; host divides by 4).
    Pair table (slot: (channel, dy, dx, matrix)):
      pair 3*m+?? -> see _PAIRS7 below.
    """
    import ml_dtypes
    D2 = np.zeros((128, 128), dtype=np.float32)
    VU2 = np.zeros((128, 128), dtype=np.float32)
    for p in range(128):
        z = p % ZSUP
        if 1 <= z <= ZINT:
            D2[p + 1, p] = 1.0
            D2[p - 1, p] = -1.0
            VU2[p, p] = 12.0 * MU
            VU2[p + 1, p] = -2.0 * MU
            VU2[p - 1, p] = -2.0 * MU
    eye = np.eye(128, dtype=np.float32)
    mats = {"D2": D2, "VU2": VU2, "IP2": eye, "IM2": -eye,
            "IMU2": -2.0 * MU * eye, "Z": np.zeros((128, 128), np.float32)}
    W = np.zeros((14, 128, 2, 128), dtype=np.float32)
    for pi, pair in enumerate(_PAIRS7):
        for sl in range(2):
            W[pi, :, sl, :] = mats[pair[sl][3]]
    # -> [128, 14, 2, 128]
    W = np.transpose(W, (1, 0, 2, 3)).copy()
    return (W.astype(ml_dtypes.float8_e4m3),
            eye.astype(ml_dtypes.bfloat16))


# pair index layout: 0-2 = A pairs for m=0,1,2; then V pairs:
# m=0: 3,4,5   m=1: 6,7,8,9   m=2: 10,11,12,13
_PAIRS7 = [
    [(0, 0, 0, "D2"), (0, 0, 1, "Z")],
    [(1, 0, 0, "D2"), (1, 0, 1, "Z")],
    [(2, 0, 0, "D2"), (2, 0, 1, "Z")],
    # V m=0
    [(0, 0, 0, "VU2"), (3, 0, 0, "D2")],
    [(0, 1, 0, "IMU2"), (0, -1, 0, "IMU2")],
    [(0, 0, 1, "IMU2"), (0, 0, -1, "IMU2")],
    # V m=1
    [(1, 0, 0, "VU2"), (3, 1, 0, "IP2")],
    [(1, 1, 0, "IMU2"), (1, -1, 0, "IMU2")],
    [(1, 0, 1, "IMU2"), (1, 0, -1, "IMU2")],
    [(3, -1, 0, "IM2"), (3, -1, 1, "Z")],
    # V m=2
    [(2, 0, 0, "VU2"), (3, 0, 1, "IP2")],
    [(2, 1, 0, "IMU2"), (2, -1, 0, "IMU2")],
    [(2, 0, 1, "IMU2"), (2, 0, -1, "IMU2")],
    [(3, 0, -1, "IM2"), (3, 0, 0, "Z")],
]

_VPAIRS7 = {0: [3, 4, 5], 1: [6, 7, 8, 9], 2: [10, 11, 12, 13]}

# row chunks per x-half unit: (out_row0, nrows); out rows 0..19
_CH7 = [(0, 6), (6, 6), (12, 6), (18, 2)]


def build_program_v7():
    from concourse.ap import AP as _AP
    f32 = mybir.dt.float32
    bf16 = mybir.dt.bfloat16
    f8 = mybir.dt.float8e4
    DRm = mybir.MatmulPerfMode.DoubleRow
    AL = mybir.AluOpType
    SQf = mybir.ActivationFunctionType.Square

    nc = bacc.Bacc("TRN2", target_bir_lowering=False, debug=False,
                   num_devices=8)
    slabb = nc.declare_dram_parameter("slabb", [NSUP, 128, 3, YROWS, NX],
                                      bf16, isOutput=False)
    slab8 = nc.declare_dram_parameter("slab8", [NSUP, 128, 4, YROWS, NX],
                                      f8, isOutput=False)
    wp = nc.declare_dram_parameter("wp", [128, 14, 2, 128], f8,
                                   isOutput=False)
    wi = nc.declare_dram_parameter("wi", [128, 128], bf16, isOutput=False)
    out = nc.declare_dram_parameter("out", [128, NSLOT7], f32, isOutput=True)

    with tile.TileContext(nc) as tc:
        with (
            tc.tile_pool(name="const", bufs=1) as cpool,
            tc.tile_pool(name="inp", bufs=2) as inpool,
            tc.tile_pool(name="drn", bufs=3) as dpool,
            tc.tile_pool(name="tmp", bufs=3) as tpool,
            tc.tile_pool(name="pv", bufs=2, space=bass.MemorySpace.PSUM) as pv,
        ):
            W = cpool.tile([128, 14, 2, 128], f8, tag="W")
            nc.sync.dma_start(W[:], wp[:])
            WI = cpool.tile([128, 128], bf16, tag="WI")
            nc.sync.dma_start(WI[:], wi[:])
            acc = cpool.tile([128, NSLOT7], f32, tag="acc")
            nc.vector.memset(acc[:], 0.0)

            pending_sq = []

            def flush_sq():
                while pending_sq:
                    pending_sq.pop(0)()

            for k in range(3):
                B = inpool.tile([128, 3, YROWS, NX], bf16, tag="B",
                                name=f"B_{k}")
                nc.sync.dma_start(B[:], slabb[k])
                F = inpool.tile([128, 4, YROWS, NX], f8, tag="F",
                                name=f"F_{k}")
                nc.sync.dma_start(F[:], slab8[k])

                fp_stride = F[:].ap[0][0]

                def pairAP(pi, r0, nr, x0):
                    (c0, dy0, dx0, _), (c1, dy1, dx1, _) = _PAIRS7[pi]
                    s0 = F[:, c0, 1 + r0 + dy0 : 1 + r0 + dy0 + nr,
                           x0 + dx0 : x0 + dx0 + 79]
                    s1 = F[:, c1, 1 + r0 + dy1 : 1 + r0 + dy1 + nr,
                           x0 + dx1 : x0 + dx1 + 79]
                    return _AP(s0.tensor, s0.offset,
                               [[fp_stride, 128], [s1.offset - s0.offset, 2],
                                [NX, nr], [1, 79]])

                for m in range(3):
                    for xh in range(2):
                        x0 = 1 + 79 * xh
                        kk, mm = k, m  # capture
                        P = pv.tile([128, 4, 512], f32, tag="pv",
                                    name=f"P_{k}_{m}_{xh}")
                        # ---- A fill ----
                        for c, (r0, nr) in enumerate(_CH7):
                            nc.tensor.matmul(P[:, c, : nr * 79],
                                             W[:, m], pairAP(m, r0, nr, x0),
                                             start=True, stop=True,
                                             perf_mode=DRm)
                        # ---- A drain (one strided op; rows 20-23 junk) ----
                        A2b = dpool.tile([128, 24, 79], bf16, tag="a2b",
                                         name=f"A2b_{k}_{m}_{xh}")
                        nc.scalar.copy(A2b[:], P[:, :, 0:474])
                        # ---- V fill (group stays open; ids close it) ----
                        for c, (r0, nr) in enumerate(_CH7):
                            vps = _VPAIRS7[m]
                            for j, pi in enumerate(vps):
                                nc.tensor.matmul(P[:, c, : nr * 79],
                                                 W[:, pi],
                                                 pairAP(pi, r0, nr, x0),
                                                 start=(j == 0), stop=False,
                                                 perf_mode=DRm)
                        # ---- DVE products ----
                        def t20(tag):
                            return tpool.tile([128, 20, 79], bf16, tag=tag,
                                              name=f"{tag}_{k}_{m}_{xh}")

                        Dy = t20("dy")
                        nc.vector.tensor_tensor(Dy[:], B[:, m, 2:22, x0:x0 + 79],
                                                B[:, m, 0:20, x0:x0 + 79],
                                                op=AL.subtract)
                        Dx = t20("dx")
                        nc.vector.tensor_tensor(Dx[:],
                                                B[:, m, 1:21, x0 + 1:x0 + 80],
                                                B[:, m, 1:21, x0 - 1:x0 + 78],
                                                op=AL.subtract)
                        M1 = t20("m1")
                        nc.vector.tensor_tensor(M1[:], Dy[:],
                                                B[:, 1, 1:21, x0:x0 + 79],
                                                op=AL.mult)
                        M2 = t20("m2")
                        nc.gpsimd.tensor_tensor(M2[:], Dx[:],
                                                B[:, 2, 1:21, x0:x0 + 79],
                                                op=AL.mult)
                        T1 = t20("t1")
                        nc.vector.tensor_tensor(T1[:], A2b[:, 0:20, :],
                                                B[:, 0, 1:21, x0:x0 + 79],
                                                op=AL.mult)
                        # ---- ids: accumulate products into V ----
                        for fi, Ft in enumerate((M1, M2, T1)):
                            for c, (r0, nr) in enumerate(_CH7):
                                nc.tensor.matmul(
                                    P[:, c, : nr * 79], WI[:],
                                    Ft[:, r0 : r0 + nr, :],
                                    start=False,
                                    stop=(fi == 2))
                        # ---- SQ (deferred one xunit for pipelining) ----
                        s = ((k * 3 + m) * 2 + xh) * 2
                        Pq = _AP(P[:].tensor, P[:].offset,
                                 [[P[:].ap[0][0], 128], [512, 3], [1, 474]])

                        def do_sq(P=P, Pq=Pq, s=s):
                            sq1 = tpool.tile([128, 3, 474], bf16, tag="sq1",
                                             name=f"sq1_{s}")
                            nc.scalar.activation(sq1[:], Pq, SQf,
                                                 accum_out=acc[:, s : s + 1])
                            sq2 = tpool.tile([128, 158], bf16, tag="sq2",
                                             name=f"sq2_{s}")
                            nc.scalar.activation(
                                sq2[0:112], P[0:112, 3, 0:158], SQf,
                                accum_out=acc[0:112, s + 1 : s + 2])

                        pending_sq.append(do_sq)
                        if len(pending_sq) > 1:
                            pending_sq.pop(0)()
            flush_sq()
            nc.sync.dma_start(out[:], acc[:])
    nc.compile()
    return nc


def build_program_v8():
    """v7 with 2-stage software-pipelined emission: per xunit i the engine
    queues carry [PE: A-fill(i), ids(i-1), V-fill(i)], [ACT: drain(i),
    SQ(i-1)], [DVE: T1(i-1), Dy/Dx/M1(i)], [Pool: M2(i)] so no engine
    head-of-line blocks on another engine's latency."""
    from concourse.ap import AP as _AP
    f32 = mybir.dt.float32
    bf16 = mybir.dt.bfloat16
    f8 = mybir.dt.float8e4
    DRm = mybir.MatmulPerfMode.DoubleRow
    AL = mybir.AluOpType
    SQf = mybir.ActivationFunctionType.Square

    nc = bacc.Bacc("TRN2", target_bir_lowering=False, debug=False,
                   num_devices=8)
    slabb = nc.declare_dram_parameter("slabb", [NSUP, 128, 3, YROWS, NX],
                                      bf16, isOutput=False)
    slab8 = nc.declare_dram_parameter("slab8", [NSUP, 128, 4, YROWS, NX],
                                      f8, isOutput=False)
    wp = nc.declare_dram_parameter("wp", [128, 14, 2, 128], f8,
                                   isOutput=False)
    wi = nc.declare_dram_parameter("wi", [128, 128], bf16, isOutput=False)
    out = nc.declare_dram_parameter("out", [128, NSLOT7], f32, isOutput=True)

    units = [(k, m, xh) for k in range(3) for m in range(3)
             for xh in range(2)]

    with tile.TileContext(nc) as tc:
        with (
            tc.tile_pool(name="const", bufs=1) as cpool,
            tc.tile_pool(name="inp", bufs=2) as inpool,
            tc.tile_pool(name="drn", bufs=3) as dpool,
            tc.tile_pool(name="tmp", bufs=3) as tpool,
            tc.tile_pool(name="pv", bufs=2, space=bass.MemorySpace.PSUM) as pv,
        ):
            W = cpool.tile([128, 14, 2, 128], f8, tag="W")
            nc.sync.dma_start(W[:], wp[:])
            WI = cpool.tile([128, 128], bf16, tag="WI")
            nc.sync.dma_start(WI[:], wi[:])
            acc = cpool.tile([128, NSLOT7], f32, tag="acc")
            nc.vector.memset(acc[:], 0.0)

            BF = {}  # k -> (B tile, F tile)

            def load_k(k):
                if k in BF or k >= 3:
                    return
                B = inpool.tile([128, 3, YROWS, NX], bf16, tag="B",
                                name=f"B_{k}")
                F = inpool.tile([128, 4, YROWS, NX], f8, tag="F",
                                name=f"F_{k}")
                nc.sync.dma_start(B[:], slabb[k])
                nc.sync.dma_start(F[:], slab8[k])
                BF[k] = (B, F)

            load_k(0)

            def pairAP(F, pi, r0, nr, x0):
                fp_stride = F[:].ap[0][0]
                (c0, dy0, dx0, _), (c1, dy1, dx1, _) = _PAIRS7[pi]
                s0 = F[:, c0, 1 + r0 + dy0 : 1 + r0 + dy0 + nr,
                       x0 + dx0 : x0 + dx0 + 79]
                s1 = F[:, c1, 1 + r0 + dy1 : 1 + r0 + dy1 + nr,
                       x0 + dx1 : x0 + dx1 + 79]
                return _AP(s0.tensor, s0.offset,
                           [[fp_stride, 128], [s1.offset - s0.offset, 2],
                            [NX, nr], [1, 79]])

            st = {}  # unit index -> state dict

            def stage_T1(j):
                u = st[j]
                T1 = tpool.tile([128, 20, 79], bf16, tag="t1",
                                name=f"t1_{j}")
                nc.vector.tensor_tensor(
                    T1[:], u["A2b"][:, 0:20, :],
                    u["B"][:, 0, 1:21, u["x0"]:u["x0"] + 79], op=AL.mult)
                u["T1"] = T1

            def stage_ids(j):
                u = st[j]
                for fi, Ft in enumerate((u["M1"], u["M2"], u["T1"])):
                    for c, (r0, nr) in enumerate(_CH7):
                        nc.tensor.matmul(u["P"][:, c, : nr * 79], WI[:],
                                         Ft[:, r0 : r0 + nr, :],
                                         start=False, stop=(fi == 2))

            def stage_sq(j):
                u = st[j]
                P, s = u["P"], u["s"]
                Pq = _AP(P[:].tensor, P[:].offset,
                         [[P[:].ap[0][0], 128], [512, 3], [1, 474]])
                sq2 = tpool.tile([128, 158], bf16, tag="sq2",
                                 name=f"sq2_{j}")
                nc.scalar.activation(sq2[0:112], P[0:112, 3, 0:158], SQf,
                                     accum_out=acc[0:112, s + 1 : s + 2])
                sq1 = tpool.tile([128, 3, 474], bf16, tag="sq1",
                                 name=f"sq1_{j}")
                nc.scalar.activation(sq1[:], Pq, SQf,
                                     accum_out=acc[:, s : s + 1])
                del st[j]

            for i, (k, m, xh) in enumerate(units):
                B, F = BF[k]
                x0 = 1 + 79 * xh
                u = {"B": B, "x0": x0,
                     "s": ((k * 3 + m) * 2 + xh) * 2}
                st[i] = u

                # DVE: finish previous unit's T1 first (A2b ready long ago)
                if i - 1 in st:
                    stage_T1(i - 1)

                # PE: A fill
                P = pv.tile([128, 4, 512], f32, tag="pv", name=f"P_{i}")
                u["P"] = P
                for c in (3, 0, 1, 2):
                    r0, nr = _CH7[c]
                    nc.tensor.matmul(P[:, c, : nr * 79], W[:, m],
                                     pairAP(F, m, r0, nr, x0),
                                     start=True, stop=True, perf_mode=DRm)
                # ACT: A drain
                A2b = dpool.tile([128, 24, 79], bf16, tag="a2b",
                                 name=f"a2b_{i}")
                nc.scalar.copy(A2b[:], P[:, :, 0:474])
                u["A2b"] = A2b

                # PE: previous unit's ids; ACT: previous unit's SQ
                if i - 1 in st:
                    stage_ids(i - 1)
                    stage_sq(i - 1)

                # PE: V fill
                for c, (r0, nr) in enumerate(_CH7):
                    for j, pi in enumerate(_VPAIRS7[m]):
                        nc.tensor.matmul(P[:, c, : nr * 79], W[:, pi],
                                         pairAP(F, pi, r0, nr, x0),
                                         start=(j == 0), stop=False,
                                         perf_mode=DRm)

                # DVE: this unit's Dy/Dx/M1 ; Pool: M2
                Dy = tpool.tile([128, 20, 79], bf16, tag="dy", name=f"dy_{i}")
                nc.vector.tensor_tensor(Dy[:], B[:, m, 2:22, x0:x0 + 79],
                                        B[:, m, 0:20, x0:x0 + 79],
                                        op=AL.subtract)
                Dx = tpool.tile([128, 20, 79], bf16, tag="dx", name=f"dx_{i}")
                nc.vector.tensor_tensor(Dx[:], B[:, m, 1:21, x0 + 1:x0 + 80],
                                        B[:, m, 1:21, x0 - 1:x0 + 78],
                                        op=AL.subtract)
                M1 = tpool.tile([128, 20, 79], bf16, tag="m1", name=f"m1_{i}")
                nc.vector.tensor_tensor(M1[:], Dy[:],
                                        B[:, 1, 1:21, x0:x0 + 79],
                                        op=AL.mult)
                u["M1"] = M1
                M2 = tpool.tile([128, 20, 79], bf16, tag="m2", name=f"m2_{i}")
                nc.gpsimd.tensor_tensor(M2[:], Dx[:],
                                        B[:, 2, 1:21, x0:x0 + 79],
                                        op=AL.mult)
                u["M2"] = M2

                # prefetch next supertile mid-way through this one
                if m == 0 and xh == 1:
                    load_k(k + 1)

            # drain the pipeline
            last = len(units) - 1
            stage_T1(last)
            stage_ids(last)
            stage_sq(last)

            nc.sync.dma_start(out[:], acc[:])
    nc.compile()
    return nc


def build_program_v10(order="v8"):
    """v7 with 2-stage software-pipelined emission: per xunit i the engine
    queues carry [PE: A-fill(i), ids(i-1), V-fill(i)], [ACT: drain(i),
    SQ(i-1)], [DVE: T1(i-1), Dy/Dx/M1(i)], [Pool: M2(i)] so no engine
    head-of-line blocks on another engine's latency."""
    from concourse.ap import AP as _AP
    f32 = mybir.dt.float32
    bf16 = mybir.dt.bfloat16
    f8 = mybir.dt.float8e4
    DRm = mybir.MatmulPerfMode.DoubleRow
    AL = mybir.AluOpType
    SQf = mybir.ActivationFunctionType.Square

    nc = bacc.Bacc("TRN2", target_bir_lowering=False, debug=False,
                   num_devices=8)
    slabb = nc.declare_dram_parameter("slabb", [NSUP, 128, 3, YROWS, NX],
                                      bf16, isOutput=False)
    slab8 = nc.declare_dram_parameter("slab8", [NSUP, 128, 4, YROWS, NX],
                                      f8, isOutput=False)
    wp = nc.declare_dram_parameter("wp", [128, 14, 2, 128], f8,
                                   isOutput=False)
    wi = nc.declare_dram_parameter("wi", [128, 128], bf16, isOutput=False)
    out = nc.declare_dram_parameter("out", [128, NSLOT7], f32, isOutput=True)

    units = [(k, m, xh) for k in range(3) for m in range(3)
             for xh in range(2)]

    with tile.TileContext(nc) as tc:
        with (
            tc.tile_pool(name="const", bufs=1) as cpool,
            tc.tile_pool(name="inp", bufs=2) as inpool,
            tc.tile_pool(name="drn", bufs=3) as dpool,
            tc.tile_pool(name="tmp", bufs=3) as tpool,
            tc.tile_pool(name="pv", bufs=2, space=bass.MemorySpace.PSUM) as pv,
        ):
            W = cpool.tile([128, 14, 2, 128], f8, tag="W")
            nc.sync.dma_start(W[:], wp[:])
            WI = cpool.tile([128, 128], bf16, tag="WI")
            nc.sync.dma_start(WI[:], wi[:])
            acc = cpool.tile([128, NSLOT7], f32, tag="acc")
            nc.vector.memset(acc[:], 0.0)

            BF = {}  # k -> (B tile, F tile)

            def load_k(k):
                if k in BF or k >= 3:
                    return
                B = inpool.tile([128, 3, YROWS, NX], bf16, tag="B",
                                name=f"B_{k}")
                nc.sync.dma_start(B[:], slabb[k])
                F = inpool.tile([128, 4, YROWS, NX], f8, tag="F",
                                name=f"F_{k}")
                nc.sync.dma_start(F[:], slab8[k])
                BF[k] = (B, F)

            load_k(0)

            def pairAP(F, pi, r0, nr, x0):
                fp_stride = F[:].ap[0][0]
                (c0, dy0, dx0, _), (c1, dy1, dx1, _) = _PAIRS7[pi]
                s0 = F[:, c0, 1 + r0 + dy0 : 1 + r0 + dy0 + nr,
                       x0 + dx0 : x0 + dx0 + 79]
                s1 = F[:, c1, 1 + r0 + dy1 : 1 + r0 + dy1 + nr,
                       x0 + dx1 : x0 + dx1 + 79]
                return _AP(s0.tensor, s0.offset,
                           [[fp_stride, 128], [s1.offset - s0.offset, 2],
                            [NX, nr], [1, 79]])

            st = {}  # unit index -> state dict

            def stage_T1(j):
                u = st[j]
                T1 = tpool.tile([128, 20, 79], bf16, tag="t1",
                                name=f"t1_{j}")
                nc.vector.tensor_tensor(
                    T1[:], u["A2b"][:, 0:20, :],
                    u["B"][:, 0, 1:21, u["x0"]:u["x0"] + 79], op=AL.mult)
                u["T1"] = T1

            def stage_ids(j):
                u = st[j]
                for fi, Ft in enumerate((u["M1"], u["M2"], u["T1"])):
                    for c, (r0, nr) in enumerate(_CH7):
                        nc.tensor.matmul(u["P"][:, c, : nr * 79], WI[:],
                                         Ft[:, r0 : r0 + nr, :],
                                         start=False, stop=(fi == 2))

            def stage_sq(j):
                u = st[j]
                P, s = u["P"], u["s"]
                Pq = _AP(P[:].tensor, P[:].offset,
                         [[P[:].ap[0][0], 128], [512, 3], [1, 474]])
                sq1 = tpool.tile([128, 3, 474], bf16, tag="sq1",
                                 name=f"sq1_{j}")
                nc.scalar.activation(sq1[:], Pq, SQf,
                                     accum_out=acc[:, s : s + 1])
                sq2 = tpool.tile([128, 158], bf16, tag="sq2",
                                 name=f"sq2_{j}")
                nc.scalar.activation(sq2[0:112], P[0:112, 3, 0:158], SQf,
                                     accum_out=acc[0:112, s + 1 : s + 2])
                del st[j]

            for i, (k, m, xh) in enumerate(units):
                B, F = BF[k]
                x0 = 1 + 79 * xh
                u = {"B": B, "x0": x0,
                     "s": ((k * 3 + m) * 2 + xh) * 2}
                st[i] = u

                # DVE: finish previous unit's T1 first (A2b ready long ago)
                if i - 1 in st:
                    stage_T1(i - 1)

                # PE: A fill
                P = pv.tile([128, 4, 512], f32, tag="pv", name=f"P_{i}")
                u["P"] = P
                for c, (r0, nr) in enumerate(_CH7):
                    nc.tensor.matmul(P[:, c, : nr * 79], W[:, m],
                                     pairAP(F, m, r0, nr, x0),
                                     start=True, stop=True, perf_mode=DRm)
                A2b = dpool.tile([128, 24, 79], bf16, tag="a2b",
                                 name=f"a2b_{i}")
                u["A2b"] = A2b

                def drain(P=P, A2b=A2b):
                    nc.scalar.copy(A2b[:], P[:, :, 0:474])

                prev = i - 1 in st
                if order == "v8":
                    drain()
                    if prev:
                        stage_ids(i - 1)
                        stage_sq(i - 1)
                elif order == "sqfirst":
                    if prev:
                        stage_ids(i - 1)
                        stage_sq(i - 1)
                    drain()
                elif order == "idsfirst":
                    if prev:
                        stage_ids(i - 1)
                    drain()
                    if prev:
                        stage_sq(i - 1)

                # PE: V fill
                for c, (r0, nr) in enumerate(_CH7):
                    for j, pi in enumerate(_VPAIRS7[m]):
                        nc.tensor.matmul(P[:, c, : nr * 79], W[:, pi],
                                         pairAP(F, pi, r0, nr, x0),
                                         start=(j == 0), stop=False,
                                         perf_mode=DRm)

                # DVE: this unit's Dy/Dx/M1 ; Pool: M2
                Dy = tpool.tile([128, 20, 79], bf16, tag="dy", name=f"dy_{i}")
                nc.vector.tensor_tensor(Dy[:], B[:, m, 2:22, x0:x0 + 79],
                                        B[:, m, 0:20, x0:x0 + 79],
                                        op=AL.subtract)
                Dx = tpool.tile([128, 20, 79], bf16, tag="dx", name=f"dx_{i}")
                nc.vector.tensor_tensor(Dx[:], B[:, m, 1:21, x0 + 1:x0 + 80],
                                        B[:, m, 1:21, x0 - 1:x0 + 78],
                                        op=AL.subtract)
                M1 = tpool.tile([128, 20, 79], bf16, tag="m1", name=f"m1_{i}")
                nc.vector.tensor_tensor(M1[:], Dy[:],
                                        B[:, 1, 1:21, x0:x0 + 79],
                                        op=AL.mult)
                u["M1"] = M1
                M2 = tpool.tile([128, 20, 79], bf16, tag="m2", name=f"m2_{i}")
                nc.gpsimd.tensor_tensor(M2[:], Dx[:],
                                        B[:, 2, 1:21, x0:x0 + 79],
                                        op=AL.mult)
                u["M2"] = M2

                # prefetch next supertile mid-way through this one
                if m == 0 and xh == 1:
                    load_k(k + 1)

            # drain the pipeline
            last = len(units) - 1
            stage_T1(last)
            stage_ids(last)
            stage_sq(last)

            nc.sync.dma_start(out[:], acc[:])
    nc.compile()
    return nc




def build_program_v9():
    """v8 + split A-drain (bank pairs) so V-fill chunks 0-1 start early,
    ACT queue [drain-a, drain-b, SQ1, SQ2], Dx emitted first so Pool's M2
    starts sooner."""
    from concourse.ap import AP as _AP
    f32 = mybir.dt.float32
    bf16 = mybir.dt.bfloat16
    f8 = mybir.dt.float8e4
    DRm = mybir.MatmulPerfMode.DoubleRow
    AL = mybir.AluOpType
    SQf = mybir.ActivationFunctionType.Square

    nc = bacc.Bacc("TRN2", target_bir_lowering=False, debug=False,
                   num_devices=8)
    slabb = nc.declare_dram_parameter("slabb", [NSUP, 128, 3, YROWS, NX],
                                      bf16, isOutput=False)
    slab8 = nc.declare_dram_parameter("slab8", [NSUP, 128, 4, YROWS, NX],
                                      f8, isOutput=False)
    wp = nc.declare_dram_parameter("wp", [128, 14, 2, 128], f8,
                                   isOutput=False)
    wi = nc.declare_dram_parameter("wi", [128, 128], bf16, isOutput=False)
    out = nc.declare_dram_parameter("out", [128, NSLOT7], f32, isOutput=True)

    units = [(k, m, xh) for k in range(3) for m in range(3)
             for xh in range(2)]

    with tile.TileContext(nc) as tc:
        with (
            tc.tile_pool(name="const", bufs=1) as cpool,
            tc.tile_pool(name="inp", bufs=2) as inpool,
            tc.tile_pool(name="drn", bufs=3) as dpool,
            tc.tile_pool(name="tmp", bufs=3) as tpool,
            tc.tile_pool(name="pv", bufs=2, space=bass.MemorySpace.PSUM) as pv,
        ):
            W = cpool.tile([128, 14, 2, 128], f8, tag="W")
            nc.sync.dma_start(W[:], wp[:])
            WI = cpool.tile([128, 128], bf16, tag="WI")
            nc.sync.dma_start(WI[:], wi[:])
            acc = cpool.tile([128, NSLOT7], f32, tag="acc")
            nc.vector.memset(acc[:], 0.0)

            BF = {}  # k -> (B tile, F tile)

            def load_k(k):
                if k in BF or k >= 3:
                    return
                B = inpool.tile([128, 3, YROWS, NX], bf16, tag="B",
                                name=f"B_{k}")
                nc.sync.dma_start(B[:], slabb[k])
                F = inpool.tile([128, 4, YROWS, NX], f8, tag="F",
                                name=f"F_{k}")
                nc.sync.dma_start(F[:], slab8[k])
                BF[k] = (B, F)

            load_k(0)

            def pairAP(F, pi, r0, nr, x0):
                fp_stride = F[:].ap[0][0]
                (c0, dy0, dx0, _), (c1, dy1, dx1, _) = _PAIRS7[pi]
                s0 = F[:, c0, 1 + r0 + dy0 : 1 + r0 + dy0 + nr,
                       x0 + dx0 : x0 + dx0 + 79]
                s1 = F[:, c1, 1 + r0 + dy1 : 1 + r0 + dy1 + nr,
                       x0 + dx1 : x0 + dx1 + 79]
                return _AP(s0.tensor, s0.offset,
                           [[fp_stride, 128], [s1.offset - s0.offset, 2],
                            [NX, nr], [1, 79]])

            st = {}  # unit index -> state dict

            def stage_T1(j):
                u = st[j]
                T1 = tpool.tile([128, 20, 79], bf16, tag="t1",
                                name=f"t1_{j}")
                nc.vector.tensor_tensor(
                    T1[:], u["A2b"][:, 0:20, :],
                    u["B"][:, 0, 1:21, u["x0"]:u["x0"] + 79], op=AL.mult)
                u["T1"] = T1

            def stage_ids(j):
                u = st[j]
                for fi, Ft in enumerate((u["M1"], u["M2"], u["T1"])):
                    for c, (r0, nr) in enumerate(_CH7):
                        nc.tensor.matmul(u["P"][:, c, : nr * 79], WI[:],
                                         Ft[:, r0 : r0 + nr, :],
                                         start=False, stop=(fi == 2))

            def stage_sq(j):
                u = st[j]
                P, s = u["P"], u["s"]
                Pq = _AP(P[:].tensor, P[:].offset,
                         [[P[:].ap[0][0], 128], [512, 3], [1, 474]])
                sq1 = tpool.tile([128, 3, 474], bf16, tag="sq1",
                                 name=f"sq1_{j}")
                nc.scalar.activation(sq1[:], Pq, SQf,
                                     accum_out=acc[:, s : s + 1])
                sq2 = tpool.tile([128, 158], bf16, tag="sq2",
                                 name=f"sq2_{j}")
                nc.scalar.activation(sq2[0:112], P[0:112, 3, 0:158], SQf,
                                     accum_out=acc[0:112, s + 1 : s + 2])
                del st[j]

            for i, (k, m, xh) in enumerate(units):
                B, F = BF[k]
                x0 = 1 + 79 * xh
                u = {"B": B, "x0": x0,
                     "s": ((k * 3 + m) * 2 + xh) * 2}
                st[i] = u

                # DVE: finish previous unit's T1 first (A2b ready long ago)
                if i - 1 in st:
                    stage_T1(i - 1)

                # PE: A fill
                P = pv.tile([128, 4, 512], f32, tag="pv", name=f"P_{i}")
                u["P"] = P
                for c, (r0, nr) in enumerate(_CH7):
                    nc.tensor.matmul(P[:, c, : nr * 79], W[:, m],
                                     pairAP(F, m, r0, nr, x0),
                                     start=True, stop=True, perf_mode=DRm)
                # ACT: A drain, split into bank pairs so V chunks 0-1 can
                # start as soon as the first half lands
                A2b = dpool.tile([128, 24, 79], bf16, tag="a2b",
                                 name=f"a2b_{i}")
                nc.scalar.copy(A2b[:, 0:12, :], P[:, 0:2, 0:474])
                nc.scalar.copy(A2b[:, 12:24, :], P[:, 2:4, 0:474])
                u["A2b"] = A2b

                # PE: previous unit's ids; ACT: previous unit's SQ
                if i - 1 in st:
                    stage_ids(i - 1)
                    stage_sq(i - 1)

                # PE: V fill
                for c, (r0, nr) in enumerate(_CH7):
                    for j, pi in enumerate(_VPAIRS7[m]):
                        nc.tensor.matmul(P[:, c, : nr * 79], W[:, pi],
                                         pairAP(F, pi, r0, nr, x0),
                                         start=(j == 0), stop=False,
                                         perf_mode=DRm)

                # DVE: Dx first so Pool's M2 can start early
                Dx = tpool.tile([128, 20, 79], bf16, tag="dx", name=f"dx_{i}")
                nc.vector.tensor_tensor(Dx[:], B[:, m, 1:21, x0 + 1:x0 + 80],
                                        B[:, m, 1:21, x0 - 1:x0 + 78],
                                        op=AL.subtract)
                M2 = tpool.tile([128, 20, 79], bf16, tag="m2", name=f"m2_{i}")
                nc.gpsimd.tensor_tensor(M2[:], Dx[:],
                                        B[:, 2, 1:21, x0:x0 + 79],
                                        op=AL.mult)
                u["M2"] = M2
                Dy = tpool.tile([128, 20, 79], bf16, tag="dy", name=f"dy_{i}")
                nc.vector.tensor_tensor(Dy[:], B[:, m, 2:22, x0:x0 + 79],
                                        B[:, m, 0:20, x0:x0 + 79],
                                        op=AL.subtract)
                M1 = tpool.tile([128, 20, 79], bf16, tag="m1", name=f"m1_{i}")
                nc.vector.tensor_tensor(M1[:], Dy[:],
                                        B[:, 1, 1:21, x0:x0 + 79],
                                        op=AL.mult)
                u["M1"] = M1

                # prefetch next supertile mid-way through this one
                if m == 0 and xh == 1:
                    load_k(k + 1)

            # drain the pipeline
            last = len(units) - 1
            stage_T1(last)
            stage_ids(last)
            stage_sq(last)

            nc.sync.dma_start(out[:], acc[:])
    nc.compile()
    return nc


def build_program_v9a():
    """v8 + split A-drain (bank pairs) so V-fill chunks 0-1 start early,
    ACT queue [drain-a, drain-b, SQ1, SQ2], Dx emitted first so Pool's M2
    starts sooner."""
    from concourse.ap import AP as _AP
    f32 = mybir.dt.float32
    bf16 = mybir.dt.bfloat16
    f8 = mybir.dt.float8e4
    DRm = mybir.MatmulPerfMode.DoubleRow
    AL = mybir.AluOpType
    SQf = mybir.ActivationFunctionType.Square

    nc = bacc.Bacc("TRN2", target_bir_lowering=False, debug=False,
                   num_devices=8)
    slabb = nc.declare_dram_parameter("slabb", [NSUP, 128, 3, YROWS, NX],
                                      bf16, isOutput=False)
    slab8 = nc.declare_dram_parameter("slab8", [NSUP, 128, 4, YROWS, NX],
                                      f8, isOutput=False)
    wp = nc.declare_dram_parameter("wp", [128, 14, 2, 128], f8,
                                   isOutput=False)
    wi = nc.declare_dram_parameter("wi", [128, 128], bf16, isOutput=False)
    out = nc.declare_dram_parameter("out", [128, NSLOT7], f32, isOutput=True)

    units = [(k, m, xh) for k in range(3) for m in range(3)
             for xh in range(2)]

    with tile.TileContext(nc) as tc:
        with (
            tc.tile_pool(name="const", bufs=1) as cpool,
            tc.tile_pool(name="inp", bufs=2) as inpool,
            tc.tile_pool(name="drn", bufs=3) as dpool,
            tc.tile_pool(name="tmp", bufs=3) as tpool,
            tc.tile_pool(name="pv", bufs=2, space=bass.MemorySpace.PSUM) as pv,
        ):
            W = cpool.tile([128, 14, 2, 128], f8, tag="W")
            nc.sync.dma_start(W[:], wp[:])
            WI = cpool.tile([128, 128], bf16, tag="WI")
            nc.sync.dma_start(WI[:], wi[:])
            acc = cpool.tile([128, NSLOT7], f32, tag="acc")
            nc.vector.memset(acc[:], 0.0)

            BF = {}  # k -> (B tile, F tile)

            def load_k(k):
                if k in BF or k >= 3:
                    return
                B = inpool.tile([128, 3, YROWS, NX], bf16, tag="B",
                                name=f"B_{k}")
                nc.sync.dma_start(B[:], slabb[k])
                F = inpool.tile([128, 4, YROWS, NX], f8, tag="F",
                                name=f"F_{k}")
                nc.sync.dma_start(F[:], slab8[k])
                BF[k] = (B, F)

            load_k(0)

            def pairAP(F, pi, r0, nr, x0):
                fp_stride = F[:].ap[0][0]
                (c0, dy0, dx0, _), (c1, dy1, dx1, _) = _PAIRS7[pi]
                s0 = F[:, c0, 1 + r0 + dy0 : 1 + r0 + dy0 + nr,
                       x0 + dx0 : x0 + dx0 + 79]
                s1 = F[:, c1, 1 + r0 + dy1 : 1 + r0 + dy1 + nr,
                       x0 + dx1 : x0 + dx1 + 79]
                return _AP(s0.tensor, s0.offset,
                           [[fp_stride, 128], [s1.offset - s0.offset, 2],
                            [NX, nr], [1, 79]])

            st = {}  # unit index -> state dict

            def stage_T1(j):
                u = st[j]
                T1 = tpool.tile([128, 20, 79], bf16, tag="t1",
                                name=f"t1_{j}")
                nc.vector.tensor_tensor(
                    T1[:], u["A2b"][:, 0:20, :],
                    u["B"][:, 0, 1:21, u["x0"]:u["x0"] + 79], op=AL.mult)
                u["T1"] = T1

            def stage_ids(j):
                u = st[j]
                for fi, Ft in enumerate((u["M1"], u["M2"], u["T1"])):
                    for c, (r0, nr) in enumerate(_CH7):
                        nc.tensor.matmul(u["P"][:, c, : nr * 79], WI[:],
                                         Ft[:, r0 : r0 + nr, :],
                                         start=False, stop=(fi == 2))

            def stage_sq(j):
                u = st[j]
                P, s = u["P"], u["s"]
                Pq = _AP(P[:].tensor, P[:].offset,
                         [[P[:].ap[0][0], 128], [512, 3], [1, 474]])
                sq1 = tpool.tile([128, 3, 474], bf16, tag="sq1",
                                 name=f"sq1_{j}")
                nc.scalar.activation(sq1[:], Pq, SQf,
                                     accum_out=acc[:, s : s + 1])
                sq2 = tpool.tile([128, 158], bf16, tag="sq2",
                                 name=f"sq2_{j}")
                nc.scalar.activation(sq2[0:112], P[0:112, 3, 0:158], SQf,
                                     accum_out=acc[0:112, s + 1 : s + 2])
                del st[j]

            for i, (k, m, xh) in enumerate(units):
                B, F = BF[k]
                x0 = 1 + 79 * xh
                u = {"B": B, "x0": x0,
                     "s": ((k * 3 + m) * 2 + xh) * 2}
                st[i] = u

                # DVE: finish previous unit's T1 first (A2b ready long ago)
                if i - 1 in st:
                    stage_T1(i - 1)

                # PE: A fill
                P = pv.tile([128, 4, 512], f32, tag="pv", name=f"P_{i}")
                u["P"] = P
                for c, (r0, nr) in enumerate(_CH7):
                    nc.tensor.matmul(P[:, c, : nr * 79], W[:, m],
                                     pairAP(F, m, r0, nr, x0),
                                     start=True, stop=True, perf_mode=DRm)
                # ACT: A drain, split into bank pairs so V chunks 0-1 can
                # start as soon as the first half lands
                A2b = dpool.tile([128, 24, 79], bf16, tag="a2b",
                                 name=f"a2b_{i}")
                nc.scalar.copy(A2b[:, 0:12, :], P[:, 0:2, 0:474])
                nc.scalar.copy(A2b[:, 12:24, :], P[:, 2:4, 0:474])
                u["A2b"] = A2b

                # PE: previous unit's ids; ACT: previous unit's SQ
                if i - 1 in st:
                    stage_ids(i - 1)
                    stage_sq(i - 1)

                # PE: V fill
                for c, (r0, nr) in enumerate(_CH7):
                    for j, pi in enumerate(_VPAIRS7[m]):
                        nc.tensor.matmul(P[:, c, : nr * 79], W[:, pi],
                                         pairAP(F, pi, r0, nr, x0),
                                         start=(j == 0), stop=False,
                                         perf_mode=DRm)

                Dy = tpool.tile([128, 20, 79], bf16, tag="dy", name=f"dy_{i}")
                nc.vector.tensor_tensor(Dy[:], B[:, m, 2:22, x0:x0 + 79],
                                        B[:, m, 0:20, x0:x0 + 79],
                                        op=AL.subtract)
                Dx = tpool.tile([128, 20, 79], bf16, tag="dx", name=f"dx_{i}")
                nc.vector.tensor_tensor(Dx[:], B[:, m, 1:21, x0 + 1:x0 + 80],
                                        B[:, m, 1:21, x0 - 1:x0 + 78],
                                        op=AL.subtract)
                M1 = tpool.tile([128, 20, 79], bf16, tag="m1", name=f"m1_{i}")
                nc.vector.tensor_tensor(M1[:], Dy[:],
                                        B[:, 1, 1:21, x0:x0 + 79],
                                        op=AL.mult)
                u["M1"] = M1
                M2 = tpool.tile([128, 20, 79], bf16, tag="m2", name=f"m2_{i}")
                nc.gpsimd.tensor_tensor(M2[:], Dx[:],
                                        B[:, 2, 1:21, x0:x0 + 79],
                                        op=AL.mult)
                u["M2"] = M2

                # prefetch next supertile mid-way through this one
                if m == 0 and xh == 1:
                    load_k(k + 1)

            # drain the pipeline
            last = len(units) - 1
            stage_T1(last)
            stage_ids(last)
            stage_sq(last)

            nc.sync.dma_start(out[:], acc[:])
    nc.compile()
    return nc




def build_program_v9b():
    """v8 + split A-drain (bank pairs) so V-fill chunks 0-1 start early,
    ACT queue [drain-a, drain-b, SQ1, SQ2], Dx emitted first so Pool's M2
    starts sooner."""
    from concourse.ap import AP as _AP
    f32 = mybir.dt.float32
    bf16 = mybir.dt.bfloat16
    f8 = mybir.dt.float8e4
    DRm = mybir.MatmulPerfMode.DoubleRow
    AL = mybir.AluOpType
    SQf = mybir.ActivationFunctionType.Square

    nc = bacc.Bacc("TRN2", target_bir_lowering=False, debug=False,
                   num_devices=8)
    slabb = nc.declare_dram_parameter("slabb", [NSUP, 128, 3, YROWS, NX],
                                      bf16, isOutput=False)
    slab8 = nc.declare_dram_parameter("slab8", [NSUP, 128, 4, YROWS, NX],
                                      f8, isOutput=False)
    wp = nc.declare_dram_parameter("wp", [128, 14, 2, 128], f8,
                                   isOutput=False)
    wi = nc.declare_dram_parameter("wi", [128, 128], bf16, isOutput=False)
    out = nc.declare_dram_parameter("out", [128, NSLOT7], f32, isOutput=True)

    units = [(k, m, xh) for k in range(3) for m in range(3)
             for xh in range(2)]

    with tile.TileContext(nc) as tc:
        with (
            tc.tile_pool(name="const", bufs=1) as cpool,
            tc.tile_pool(name="inp", bufs=2) as inpool,
            tc.tile_pool(name="drn", bufs=3) as dpool,
            tc.tile_pool(name="tmp", bufs=3) as tpool,
            tc.tile_pool(name="pv", bufs=2, space=bass.MemorySpace.PSUM) as pv,
        ):
            W = cpool.tile([128, 14, 2, 128], f8, tag="W")
            nc.sync.dma_start(W[:], wp[:])
            WI = cpool.tile([128, 128], bf16, tag="WI")
            nc.sync.dma_start(WI[:], wi[:])
            acc = cpool.tile([128, NSLOT7], f32, tag="acc")
            nc.vector.memset(acc[:], 0.0)

            BF = {}  # k -> (B tile, F tile)

            def load_k(k):
                if k in BF or k >= 3:
                    return
                B = inpool.tile([128, 3, YROWS, NX], bf16, tag="B",
                                name=f"B_{k}")
                nc.sync.dma_start(B[:], slabb[k])
                F = inpool.tile([128, 4, YROWS, NX], f8, tag="F",
                                name=f"F_{k}")
                nc.sync.dma_start(F[:], slab8[k])
                BF[k] = (B, F)

            load_k(0)

            def pairAP(F, pi, r0, nr, x0):
                fp_stride = F[:].ap[0][0]
                (c0, dy0, dx0, _), (c1, dy1, dx1, _) = _PAIRS7[pi]
                s0 = F[:, c0, 1 + r0 + dy0 : 1 + r0 + dy0 + nr,
                       x0 + dx0 : x0 + dx0 + 79]
                s1 = F[:, c1, 1 + r0 + dy1 : 1 + r0 + dy1 + nr,
                       x0 + dx1 : x0 + dx1 + 79]
                return _AP(s0.tensor, s0.offset,
                           [[fp_stride, 128], [s1.offset - s0.offset, 2],
                            [NX, nr], [1, 79]])

            st = {}  # unit index -> state dict

            def stage_T1(j):
                u = st[j]
                T1 = tpool.tile([128, 20, 79], bf16, tag="t1",
                                name=f"t1_{j}")
                nc.vector.tensor_tensor(
                    T1[:], u["A2b"][:, 0:20, :],
                    u["B"][:, 0, 1:21, u["x0"]:u["x0"] + 79], op=AL.mult)
                u["T1"] = T1

            def stage_ids(j):
                u = st[j]
                for fi, Ft in enumerate((u["M1"], u["M2"], u["T1"])):
                    for c, (r0, nr) in enumerate(_CH7):
                        nc.tensor.matmul(u["P"][:, c, : nr * 79], WI[:],
                                         Ft[:, r0 : r0 + nr, :],
                                         start=False, stop=(fi == 2))

            def stage_sq(j):
                u = st[j]
                P, s = u["P"], u["s"]
                Pq = _AP(P[:].tensor, P[:].offset,
                         [[P[:].ap[0][0], 128], [512, 3], [1, 474]])
                sq1 = tpool.tile([128, 3, 474], bf16, tag="sq1",
                                 name=f"sq1_{j}")
                nc.scalar.activation(sq1[:], Pq, SQf,
                                     accum_out=acc[:, s : s + 1])
                sq2 = tpool.tile([128, 158], bf16, tag="sq2",
                                 name=f"sq2_{j}")
                nc.scalar.activation(sq2[0:112], P[0:112, 3, 0:158], SQf,
                                     accum_out=acc[0:112, s + 1 : s + 2])
                del st[j]

            for i, (k, m, xh) in enumerate(units):
                B, F = BF[k]
                x0 = 1 + 79 * xh
                u = {"B": B, "x0": x0,
                     "s": ((k * 3 + m) * 2 + xh) * 2}
                st[i] = u

                # DVE: finish previous unit's T1 first (A2b ready long ago)
                if i - 1 in st:
                    stage_T1(i - 1)

                # PE: A fill
                P = pv.tile([128, 4, 512], f32, tag="pv", name=f"P_{i}")
                u["P"] = P
                for c, (r0, nr) in enumerate(_CH7):
                    nc.tensor.matmul(P[:, c, : nr * 79], W[:, m],
                                     pairAP(F, m, r0, nr, x0),
                                     start=True, stop=True, perf_mode=DRm)
                A2b = dpool.tile([128, 24, 79], bf16, tag="a2b",
                                 name=f"a2b_{i}")
                nc.scalar.copy(A2b[:], P[:, :, 0:474])
                u["A2b"] = A2b

                # PE: previous unit's ids; ACT: previous unit's SQ
                if i - 1 in st:
                    stage_ids(i - 1)
                    stage_sq(i - 1)

                # PE: V fill
                for c, (r0, nr) in enumerate(_CH7):
                    for j, pi in enumerate(_VPAIRS7[m]):
                        nc.tensor.matmul(P[:, c, : nr * 79], W[:, pi],
                                         pairAP(F, pi, r0, nr, x0),
                                         start=(j == 0), stop=False,
                                         perf_mode=DRm)

                # DVE: Dx first so Pool's M2 can start early
                Dx = tpool.tile([128, 20, 79], bf16, tag="dx", name=f"dx_{i}")
                nc.vector.tensor_tensor(Dx[:], B[:, m, 1:21, x0 + 1:x0 + 80],
                                        B[:, m, 1:21, x0 - 1:x0 + 78],
                                        op=AL.subtract)
                M2 = tpool.tile([128, 20, 79], bf16, tag="m2", name=f"m2_{i}")
                nc.gpsimd.tensor_tensor(M2[:], Dx[:],
                                        B[:, 2, 1:21, x0:x0 + 79],
                                        op=AL.mult)
                u["M2"] = M2
                Dy = tpool.tile([128, 20, 79], bf16, tag="dy", name=f"dy_{i}")
                nc.vector.tensor_tensor(Dy[:], B[:, m, 2:22, x0:x0 + 79],
                                        B[:, m, 0:20, x0:x0 + 79],
                                        op=AL.subtract)
                M1 = tpool.tile([128, 20, 79], bf16, tag="m1", name=f"m1_{i}")
                nc.vector.tensor_tensor(M1[:], Dy[:],
                                        B[:, 1, 1:21, x0:x0 + 79],
                                        op=AL.mult)
                u["M1"] = M1

                # prefetch next supertile mid-way through this one
                if m == 0 and xh == 1:
                    load_k(k + 1)

            # drain the pipeline
            last = len(units) - 1
            stage_T1(last)
            stage_ids(last)
            stage_sq(last)

            nc.sync.dma_start(out[:], acc[:])
    nc.compile()
    return nc





# ---------------------------------------------------------------------------
# v11: v8 with single-channel DoubleRow pairs so every DMA is a whole-tile
# transfer (per-channel DRAM params); channel-priority DMA order shrinks the
# startup head without the partial-slice NEFF crash.

_PAIRS11 = [
    # A pairs (m = 0,1,2)
    (0, (0, 0, "D2"), (0, 1, "Z")),
    (1, (0, 0, "D2"), (0, 1, "Z")),
    (2, (0, 0, "D2"), (0, 1, "Z")),
    # V m=0
    (0, (0, 0, "VU2"), (1, 0, "IMU2")),
    (0, (-1, 0, "IMU2"), (0, 1, "IMU2")),
    (0, (0, -1, "IMU2"), (0, 0, "Z")),
    (3, (0, 0, "D2"), (0, 1, "Z")),
    # V m=1
    (1, (0, 0, "VU2"), (1, 0, "IMU2")),
    (1, (-1, 0, "IMU2"), (0, 1, "IMU2")),
    (1, (0, -1, "IMU2"), (0, 0, "Z")),
    (3, (1, 0, "IP2"), (-1, 0, "IM2")),
    # V m=2
    (2, (0, 0, "VU2"), (1, 0, "IMU2")),
    (2, (-1, 0, "IMU2"), (0, 1, "IMU2")),
    (2, (0, -1, "IMU2"), (0, 0, "Z")),
    (3, (0, 1, "IP2"), (0, -1, "IM2")),
]

_VPAIRS11 = {0: [3, 4, 5, 6], 1: [7, 8, 9, 10], 2: [11, 12, 13, 14]}


def _w_bands_v11():
    import ml_dtypes
    D2 = np.zeros((128, 128), dtype=np.float32)
    VU2 = np.zeros((128, 128), dtype=np.float32)
    for p in range(128):
        z = p % ZSUP
        if 1 <= z <= ZINT:
            D2[p + 1, p] = 1.0
            D2[p - 1, p] = -1.0
            VU2[p, p] = 12.0 * MU
            VU2[p + 1, p] = -2.0 * MU
            VU2[p - 1, p] = -2.0 * MU
    eye = np.eye(128, dtype=np.float32)
    mats = {"D2": D2, "VU2": VU2, "IP2": eye, "IM2": -eye,
            "IMU2": -2.0 * MU * eye, "Z": np.zeros((128, 128), np.float32)}
    W = np.zeros((15, 128, 2, 128), dtype=np.float32)
    for pi, (ch, s0, s1) in enumerate(_PAIRS11):
        W[pi, :, 0, :] = mats[s0[2]]
        W[pi, :, 1, :] = mats[s1[2]]
    W = np.transpose(W, (1, 0, 2, 3)).copy()
    return (W.astype(ml_dtypes.float8_e4m3), eye.astype(ml_dtypes.bfloat16))


def build_program_v11():
    from concourse.ap import AP as _AP
    f32 = mybir.dt.float32
    bf16 = mybir.dt.bfloat16
    f8 = mybir.dt.float8e4
    DRm = mybir.MatmulPerfMode.DoubleRow
    AL = mybir.AluOpType
    SQf = mybir.ActivationFunctionType.Square

    nc = bacc.Bacc("TRN2", target_bir_lowering=False, debug=False,
                   num_devices=8)
    dbs = [nc.declare_dram_parameter(f"b{c}", [NSUP, 128, YROWS, NX], bf16,
                                     isOutput=False) for c in range(3)]
    dfs = [nc.declare_dram_parameter(f"f{c}", [NSUP, 128, YROWS, NX], f8,
                                     isOutput=False) for c in range(4)]
    wp = nc.declare_dram_parameter("wp", [128, 15, 2, 128], f8,
                                   isOutput=False)
    wi = nc.declare_dram_parameter("wi", [128, 128], bf16, isOutput=False)
    out = nc.declare_dram_parameter("out", [128, NSLOT7], f32, isOutput=True)

    units = [(k, m, xh) for k in range(3) for m in range(3)
             for xh in range(2)]

    with tile.TileContext(nc) as tc:
        with (
            tc.tile_pool(name="const", bufs=1) as cpool,
            tc.tile_pool(name="inp", bufs=2) as inpool,
            tc.tile_pool(name="drn", bufs=3) as dpool,
            tc.tile_pool(name="tmp", bufs=3) as tpool,
            tc.tile_pool(name="pv", bufs=2, space=bass.MemorySpace.PSUM) as pv,
        ):
            W = cpool.tile([128, 15, 2, 128], f8, tag="W")
            nc.sync.dma_start(W[:], wp[:])
            WI = cpool.tile([128, 128], bf16, tag="WI")
            nc.sync.dma_start(WI[:], wi[:])
            acc = cpool.tile([128, NSLOT7], f32, tag="acc")
            nc.vector.memset(acc[:], 0.0)

            BF = {}  # k -> (list of 3 B tiles, list of 4 F tiles)

            def load_k(k):
                if k in BF or k >= 3:
                    return
                Bs = [inpool.tile([128, YROWS, NX], bf16, tag=f"B{c}",
                                  name=f"B{c}_{k}") for c in range(3)]
                Fs = [inpool.tile([128, YROWS, NX], f8, tag=f"F{c}",
                                  name=f"F{c}_{k}") for c in range(4)]
                # dependency-priority order: whole-tile transfers only
                nc.sync.dma_start(Bs[0][:], dbs[0][k])
                nc.sync.dma_start(Fs[0][:], dfs[0][k])
                nc.sync.dma_start(Fs[3][:], dfs[3][k])
                nc.sync.dma_start(Bs[1][:], dbs[1][k])
                nc.sync.dma_start(Bs[2][:], dbs[2][k])
                nc.sync.dma_start(Fs[1][:], dfs[1][k])
                nc.sync.dma_start(Fs[2][:], dfs[2][k])
                BF[k] = (Bs, Fs)

            load_k(0)

            def pairAP(Fs, pi, r0, nr, x0):
                ch, (dy0, dx0, _), (dy1, dx1, _) = _PAIRS11[pi]
                Ft = Fs[ch]
                fp_stride = Ft[:].ap[0][0]
                s0 = Ft[:, 1 + r0 + dy0 : 1 + r0 + dy0 + nr,
                        x0 + dx0 : x0 + dx0 + 79]
                s1 = Ft[:, 1 + r0 + dy1 : 1 + r0 + dy1 + nr,
                        x0 + dx1 : x0 + dx1 + 79]
                return _AP(s0.tensor, s0.offset,
                           [[fp_stride, 128], [s1.offset - s0.offset, 2],
                            [NX, nr], [1, 79]])

            st = {}

            def stage_T1(j):
                u = st[j]
                T1 = tpool.tile([128, 20, 79], bf16, tag="t1",
                                name=f"t1_{j}")
                nc.vector.tensor_tensor(
                    T1[:], u["A2b"][:, 0:20, :],
                    u["Bs"][0][:, 1:21, u["x0"]:u["x0"] + 79], op=AL.mult)
                u["T1"] = T1

            def stage_ids(j):
                u = st[j]
                for fi, Ft in enumerate((u["M1"], u["M2"], u["T1"])):
                    order = (3, 0, 1, 2) if fi == 2 else (0, 1, 2, 3)
                    for c in order:
                        r0, nr = _CH7[c]
                        nc.tensor.matmul(u["P"][:, c, : nr * 79], WI[:],
                                         Ft[:, r0 : r0 + nr, :],
                                         start=False, stop=(fi == 2))

            def stage_sq(j):
                u = st[j]
                P, s = u["P"], u["s"]
                Pq = _AP(P[:].tensor, P[:].offset,
                         [[P[:].ap[0][0], 128], [512, 3], [1, 474]])
                sq2 = tpool.tile([128, 158], bf16, tag="sq2",
                                 name=f"sq2_{j}")
                nc.scalar.activation(sq2[0:112], P[0:112, 3, 0:158], SQf,
                                     accum_out=acc[0:112, s + 1 : s + 2])
                sq1 = tpool.tile([128, 3, 474], bf16, tag="sq1",
                                 name=f"sq1_{j}")
                nc.scalar.activation(sq1[:], Pq, SQf,
                                     accum_out=acc[:, s : s + 1])
                del st[j]

            for i, (k, m, xh) in enumerate(units):
                Bs, Fs = BF[k]
                x0 = 1 + 79 * xh
                u = {"Bs": Bs, "x0": x0,
                     "s": ((k * 3 + m) * 2 + xh) * 2}
                st[i] = u

                if i - 1 in st:
                    stage_T1(i - 1)

                P = pv.tile([128, 4, 512], f32, tag="pv", name=f"P_{i}")
                u["P"] = P
                for c in (3, 0, 1, 2):
                    r0, nr = _CH7[c]
                    nc.tensor.matmul(P[:, c, : nr * 79], W[:, m],
                                     pairAP(Fs, m, r0, nr, x0),
                                     start=True, stop=True, perf_mode=DRm)
                A2b = dpool.tile([128, 24, 79], bf16, tag="a2b",
                                 name=f"a2b_{i}")
                nc.scalar.copy(A2b[:], P[:, :, 0:474])
                u["A2b"] = A2b

                if i - 1 in st:
                    stage_ids(i - 1)
                    stage_sq(i - 1)

                for c, (r0, nr) in enumerate(_CH7):
                    for j, pi in enumerate(_VPAIRS11[m]):
                        nc.tensor.matmul(P[:, c, : nr * 79], W[:, pi],
                                         pairAP(Fs, pi, r0, nr, x0),
                                         start=(j == 0), stop=False,
                                         perf_mode=DRm)

                Dy = tpool.tile([128, 20, 79], bf16, tag="dy", name=f"dy_{i}")
                nc.vector.tensor_tensor(Dy[:], Bs[m][:, 2:22, x0:x0 + 79],
                                        Bs[m][:, 0:20, x0:x0 + 79],
                                        op=AL.subtract)
                Dx = tpool.tile([128, 20, 79], bf16, tag="dx", name=f"dx_{i}")
                nc.vector.tensor_tensor(Dx[:],
                                        Bs[m][:, 1:21, x0 + 1:x0 + 80],
                                        Bs[m][:, 1:21, x0 - 1:x0 + 78],
                                        op=AL.subtract)
                M1 = tpool.tile([128, 20, 79], bf16, tag="m1", name=f"m1_{i}")
                nc.vector.tensor_tensor(M1[:], Dy[:],
                                        Bs[1][:, 1:21, x0:x0 + 79],
                                        op=AL.mult)
                u["M1"] = M1
                M2 = tpool.tile([128, 20, 79], bf16, tag="m2", name=f"m2_{i}")
                nc.gpsimd.tensor_tensor(M2[:], Dx[:],
                                        Bs[2][:, 1:21, x0:x0 + 79],
                                        op=AL.mult)
                u["M2"] = M2

                if m == 1 and xh == 0:
                    load_k(k + 1)

            last = len(units) - 1
            stage_T1(last)
            stage_ids(last)
            stage_sq(last)

            nc.sync.dma_start(out[:], acc[:])
    nc.compile()
    return nc


def build_program_v12():
    from concourse.ap import AP as _AP
    f32 = mybir.dt.float32
    bf16 = mybir.dt.bfloat16
    f8 = mybir.dt.float8e4
    DRm = mybir.MatmulPerfMode.DoubleRow
    AL = mybir.AluOpType
    SQf = mybir.ActivationFunctionType.Square

    nc = bacc.Bacc("TRN2", target_bir_lowering=False, debug=False,
                   num_devices=8)
    dbs = [nc.declare_dram_parameter(f"b{c}", [NSUP, 128, YROWS, NX], bf16,
                                     isOutput=False) for c in range(3)]
    dfs = [nc.declare_dram_parameter(f"f{c}", [NSUP, 128, YROWS, NX], f8,
                                     isOutput=False) for c in range(4)]
    wp = nc.declare_dram_parameter("wp", [128, 15, 2, 128], f8,
                                   isOutput=False)
    wi = nc.declare_dram_parameter("wi", [128, 128], bf16, isOutput=False)
    out = nc.declare_dram_parameter("out", [128, NSLOT7], f32, isOutput=True)

    units = [(k, m, xh) for k in range(3) for m in range(3)
             for xh in range(2)]

    with tile.TileContext(nc) as tc:
        with (
            tc.tile_pool(name="const", bufs=1) as cpool,
            tc.tile_pool(name="inp", bufs=2) as inpool,
            tc.tile_pool(name="drn", bufs=3) as dpool,
            tc.tile_pool(name="tmp", bufs=3) as tpool,
            tc.tile_pool(name="pv", bufs=2, space=bass.MemorySpace.PSUM) as pv,
        ):
            W = cpool.tile([128, 15, 2, 128], f8, tag="W")
            nc.sync.dma_start(W[:], wp[:])
            WI = cpool.tile([128, 128], bf16, tag="WI")
            nc.sync.dma_start(WI[:], wi[:])
            acc = cpool.tile([128, NSLOT7], f32, tag="acc")
            nc.vector.memset(acc[:], 0.0)

            BF = {}  # k -> (list of 3 B tiles, list of 4 F tiles)

            def load_k(k):
                if k in BF or k >= 3:
                    return
                Bs = [inpool.tile([128, YROWS, NX], bf16, tag=f"B{c}",
                                  name=f"B{c}_{k}") for c in range(3)]
                Fs = [inpool.tile([128, YROWS, NX], f8, tag=f"F{c}",
                                  name=f"F{c}_{k}") for c in range(4)]
                # dependency-priority order: whole-tile transfers only
                nc.sync.dma_start(Bs[0][:], dbs[0][k])
                nc.sync.dma_start(Fs[0][:], dfs[0][k])
                nc.sync.dma_start(Fs[3][:], dfs[3][k])
                nc.sync.dma_start(Bs[1][:], dbs[1][k])
                nc.sync.dma_start(Bs[2][:], dbs[2][k])
                nc.sync.dma_start(Fs[1][:], dfs[1][k])
                nc.sync.dma_start(Fs[2][:], dfs[2][k])
                BF[k] = (Bs, Fs)

            load_k(0)

            def pairAP(Fs, pi, r0, nr, x0):
                ch, (dy0, dx0, _), (dy1, dx1, _) = _PAIRS11[pi]
                Ft = Fs[ch]
                fp_stride = Ft[:].ap[0][0]
                s0 = Ft[:, 1 + r0 + dy0 : 1 + r0 + dy0 + nr,
                        x0 + dx0 : x0 + dx0 + 79]
                s1 = Ft[:, 1 + r0 + dy1 : 1 + r0 + dy1 + nr,
                        x0 + dx1 : x0 + dx1 + 79]
                return _AP(s0.tensor, s0.offset,
                           [[fp_stride, 128], [s1.offset - s0.offset, 2],
                            [NX, nr], [1, 79]])

            st = {}

            def stage_T1(j):
                u = st[j]
                T1 = tpool.tile([128, 20, 79], bf16, tag="t1",
                                name=f"t1_{j}")
                nc.vector.tensor_tensor(
                    T1[:], u["A2b"][:, 0:20, :],
                    u["Bs"][0][:, 1:21, u["x0"]:u["x0"] + 79], op=AL.mult)
                u["T1"] = T1

            def stage_ids(j):
                u = st[j]
                for fi, Ft in enumerate((u["M1"], u["M2"], u["T1"])):
                    for c, (r0, nr) in enumerate(_CH7):
                        nc.tensor.matmul(u["P"][:, c, : nr * 79], WI[:],
                                         Ft[:, r0 : r0 + nr, :],
                                         start=False, stop=(fi == 2))

            def stage_sq(j):
                u = st[j]
                P, s = u["P"], u["s"]
                Pq = _AP(P[:].tensor, P[:].offset,
                         [[P[:].ap[0][0], 128], [512, 3], [1, 474]])
                sq2 = tpool.tile([128, 158], bf16, tag="sq2",
                                 name=f"sq2_{j}")
                nc.scalar.activation(sq2[0:112], P[0:112, 3, 0:158], SQf,
                                     accum_out=acc[0:112, s + 1 : s + 2])
                sq1 = tpool.tile([128, 3, 474], bf16, tag="sq1",
                                 name=f"sq1_{j}")
                nc.scalar.activation(sq1[:], Pq, SQf,
                                     accum_out=acc[:, s : s + 1])
                del st[j]

            for i, (k, m, xh) in enumerate(units):
                Bs, Fs = BF[k]
                x0 = 1 + 79 * xh
                u = {"Bs": Bs, "x0": x0,
                     "s": ((k * 3 + m) * 2 + xh) * 2}
                st[i] = u

                # previous unit's ids + SQ lead this iteration: all their
                # inputs (T1 emitted last iter after the drain) are ready, so
                # the region frees as early as possible
                if i - 1 in st:
                    stage_ids(i - 1)
                    stage_sq(i - 1)

                P = pv.tile([128, 4, 512], f32, tag="pv", name=f"P_{i}")
                u["P"] = P
                for c in (3, 0, 1, 2):
                    r0, nr = _CH7[c]
                    nc.tensor.matmul(P[:, c, : nr * 79], W[:, m],
                                     pairAP(Fs, m, r0, nr, x0),
                                     start=True, stop=True, perf_mode=DRm)
                A2b = dpool.tile([128, 24, 79], bf16, tag="a2b",
                                 name=f"a2b_{i}")
                nc.scalar.copy(A2b[:], P[:, :, 0:474])
                u["A2b"] = A2b

                for c, (r0, nr) in enumerate(_CH7):
                    for j, pi in enumerate(_VPAIRS11[m]):
                        nc.tensor.matmul(P[:, c, : nr * 79], W[:, pi],
                                         pairAP(Fs, pi, r0, nr, x0),
                                         start=(j == 0), stop=False,
                                         perf_mode=DRm)

                Dy = tpool.tile([128, 20, 79], bf16, tag="dy", name=f"dy_{i}")
                nc.vector.tensor_tensor(Dy[:], Bs[m][:, 2:22, x0:x0 + 79],
                                        Bs[m][:, 0:20, x0:x0 + 79],
                                        op=AL.subtract)
                Dx = tpool.tile([128, 20, 79], bf16, tag="dx", name=f"dx_{i}")
                nc.vector.tensor_tensor(Dx[:],
                                        Bs[m][:, 1:21, x0 + 1:x0 + 80],
                                        Bs[m][:, 1:21, x0 - 1:x0 + 78],
                                        op=AL.subtract)
                M1 = tpool.tile([128, 20, 79], bf16, tag="m1", name=f"m1_{i}")
                nc.vector.tensor_tensor(M1[:], Dy[:],
                                        Bs[1][:, 1:21, x0:x0 + 79],
                                        op=AL.mult)
                u["M1"] = M1
                M2 = tpool.tile([128, 20, 79], bf16, tag="m2", name=f"m2_{i}")
                nc.gpsimd.tensor_tensor(M2[:], Dx[:],
                                        Bs[2][:, 1:21, x0:x0 + 79],
                                        op=AL.mult)
                u["M2"] = M2
                stage_T1(i)

                if m == 1 and xh == 0:
                    load_k(k + 1)

            last = len(units) - 1
            stage_ids(last)
            stage_sq(last)

            nc.sync.dma_start(out[:], acc[:])
    nc.compile()
    return nc




def make_zslab(output, b, zc):
    """[4, 44, 162, 160] f32 slab for core (b, zc) from output [2,4,160,...]."""
    slab = np.zeros((4, NZ_SLAB, NY_PAD, NX), dtype=np.float32)
    z0 = 40 * zc
    zn = min(NZ_SLAB, 160 - z0)
    slab[:, :zn, :160, :] = output[b, :, z0 : z0 + zn, :, :]
    return slab


def pack_slab(zslab):
    """Repack [4,44,162,160] -> device layout [4, 3, 128, 22, 160]."""
    out = np.empty((4, NSUP, 128, YROWS, NX), dtype=np.float32)
    for k in range(NSUP):
        zk = zslab[:, 14 * k : 14 * k + 16]          # [4,16,162,160]
        for q in range(NYB):
            out[:, k, 16 * q : 16 * q + 16] = zk[:, :, 20 * q : 20 * q + 22, :]
    return out


def pack_slab_chan(zslab):
    """Repack [4,44,162,160] -> [NSUP, 128, 4, YROWS, NX] (channel inside
    the partition's free dim, one big DMA per supertile)."""
    out = np.empty((NSUP, 128, 4, YROWS, NX), dtype=np.float32)
    for k in range(NSUP):
        zk = zslab[:, 14 * k : 14 * k + 16]          # [4,16,162,160]
        for q in range(NYB):
            # partition p = q*16 + z ; channel axis after partition
            out[k, 16 * q : 16 * q + 16] = np.transpose(
                zk[:, :, 20 * q : 20 * q + 22, :], (1, 0, 2, 3))
    return out


VARIANT = "v11"
_NC_CACHE = {}


_BUILDERS = {"v1": build_program, "v2": build_program_v2,
             "v3": build_program_v3, "v4": build_program_v4,
             "v5": build_program_v5, "v6": build_program_v6,
             "v7": build_program_v7, "v8": build_program_v8,
             "v9": build_program_v9, "v9a": build_program_v9a,
             "v9b": build_program_v9b,
             "v10sq": (lambda: build_program_v10("sqfirst")),
             "v10id": (lambda: build_program_v10("idsfirst")),
             "v11": build_program_v11, "v12": build_program_v12}


def _get_nc():
    if VARIANT not in _NC_CACHE:
        _NC_CACHE[VARIANT] = _BUILDERS[VARIANT]()
    return _NC_CACHE[VARIANT]


def make_in_maps(output):
    import ml_dtypes
    if VARIANT in ("v11", "v12"):
        w8, wi = _w_bands_v11()
        in_maps = []
        for core in range(8):
            b, zc = core // 4, core % 4
            s = pack_slab_chan(make_zslab(output, b, zc))
            im = {"wp": w8, "wi": wi}
            for c in range(3):
                im[f"b{c}"] = s[:, :, c].astype(ml_dtypes.bfloat16).copy()
            for c in range(4):
                im[f"f{c}"] = s[:, :, c].astype(ml_dtypes.float8_e4m3).copy()
            in_maps.append(im)
        return in_maps
    if VARIANT in ("v7", "v8", "v9", "v9a", "v9b", "v10sq", "v10id"):
        w8, wi = _w_bands_v7()
        in_maps = []
        for core in range(8):
            b, zc = core // 4, core % 4
            s = pack_slab_chan(make_zslab(output, b, zc))
            in_maps.append({
                "slabb": s[:, :, 0:3].astype(ml_dtypes.bfloat16).copy(),
                "slab8": s.astype(ml_dtypes.float8_e4m3),
                "wp": w8, "wi": wi})
        return in_maps
    if VARIANT == "v6":
        dmats = _band_matrices_x2()
        in_maps = []
        for core in range(8):
            b, zc = core // 4, core % 4
            s = pack_slab_chan(make_zslab(output, b, zc))
            in_maps.append({"slab": s.astype(ml_dtypes.bfloat16),
                            "dmats": dmats, "zmask": _zmask(zc)})
        return in_maps
    dmats = _band_matrices() if VARIANT == "v1" else _band_matrices_v2()
    in_maps = []
    for core in range(8):
        b, zc = core // 4, core % 4
        s = pack_slab(make_zslab(output, b, zc))
        if VARIANT != "v1":
            s = s.astype(ml_dtypes.bfloat16)
        im = {"slab": s, "dmats": dmats}
        if VARIANT in ("v3", "v4", "v5"):
            im["amask"] = _amask(zc)
        else:
            im["zmask"] = _zmask(zc)
        in_maps.append(im)
    return in_maps


def kernel(output, inp):
    output = np.asarray(output, dtype=np.float32)
    nc = _get_nc()
    res = run_bass_kernel_spmd(nc, make_in_maps(output),
                               core_ids=list(range(8)))
    total = np.float64(0.0)
    if VARIANT in ("v7", "v8", "v9", "v9a", "v9b", "v10sq", "v10id", "v11", "v12"):
        for core, r in enumerate(res.results):
            zc = core % 4
            zm3 = _zmask(zc).astype(np.float64)  # [3, 128]
            o = r["out"].astype(np.float64)      # [128, 36]
            for slot in range(NSLOT7):
                total += (o[:, slot] * zm3[slot // 12]).sum()
        total /= 4.0
    else:
        for r in res.results:
            total += np.float64(r["out"].astype(np.float64).sum())
        if VARIANT == "v6":
            total /= 4.0
    n = 2 * 158 * 158 * 158
    return np.float32(total / n)



# revision 46
# speedup vs baseline: 1.0300x; 1.0278x over previous
"""Navier-Stokes momentum-residual loss on 8 Trainium2 NeuronCores.

Reference computes, per momentum component m in {z,y,x}:
    R_m = rho*(uz_c*d_dz(u_m) + uy_c*d_dy(u_m) + ux_c*d_dx(u_m))
          + d_dm(p) - MU*lap(u_m)
    loss = sum_m mean(R_m^2)   over the interior [2,158,158,158]

Sharding: 8 cores = (batch b in {0,1}) x (z-chunk zc in {0..3}).  Each core
gets a z-slab of 44 planes [4, 44, 162, 160] (z planes 40*zc .. 40*zc+43,
y padded 160->162, zero-padded out of range).

On-core layout: partition p = y_block*16 + z_loc (8 y-blocks of 20 interior
rows, 16 z-planes per supertile).  3 z-supertiles x 2 x-halves per core.
z-direction stencil terms are computed on the TensorEngine with banded
128x128 matrices (PSUM accumulation); y/x stencils on the VectorEngine via
free-dim AP offsets; squared residuals are summed by the ScalarEngine's
activation(Square, accum_out=...) with a per-partition z-validity mask.
Host sums the per-core [128, NSLOT] partials and divides by N.
"""

import numpy as np

import concourse.bass as bass
import concourse.tile as tile
from concourse import bacc, mybir
from concourse.bass_utils import run_bass_kernel_spmd

try:  # persistent XLA/NEFF compile cache across processes (best effort)
    import jax as _jax
    _jax.config.update("jax_compilation_cache_dir", "/tmp/jax_ns_cache")
    _jax.config.update("jax_persistent_cache_min_entry_size_bytes", -1)
    _jax.config.update("jax_persistent_cache_min_compile_time_secs", 0.0)
except Exception:
    pass

MU = 0.01
RHO = 1.0

# geometry
NZ_SLAB = 44          # z planes per core slab
NY_PAD = 162          # y rows (160 + 2 zero pad)
NX = 160
NSUP = 3              # z supertiles per core
ZSUP = 16             # z planes per supertile (14 interior)
ZINT = 14
NYB = 8               # y blocks
YROWS = 22            # input y rows per block (20 interior + 2 halo)
XTW = 82              # x columns per x-half tile
NSLOT = 6 * 3 * 6     # units * momenta * accum slots


def _band_matrices():
    """lhsT matrices for the z-direction banded matmuls.

    out[p, f] = sum_k lhsT[k, p] * rhs[k, f];  p = yblk*16 + z_loc.
    D:  0.5*(u[z+1] - u[z-1]);  VU: -MU*(u[z+1] + u[z-1]) + 6*MU*u
    (only emitted for interior z_loc 1..14; edge columns all-zero).
    """
    D = np.zeros((128, 128), dtype=np.float32)
    VU = np.zeros((128, 128), dtype=np.float32)
    for p in range(128):
        z = p % ZSUP
        if 1 <= z <= ZINT:
            D[p + 1, p] = 0.5
            D[p - 1, p] = -0.5
            VU[p, p] = 6.0 * MU
            VU[p + 1, p] = -MU
            VU[p - 1, p] = -MU
    return np.concatenate([D, VU], axis=1)  # [128, 256]


def _zmask(zc):
    """[3, 128] validity mask per supertile/partition for core z-chunk zc."""
    smax = min(40, 158 - 40 * zc)
    m = np.zeros((3, 128), dtype=np.float32)
    for k in range(3):
        for p in range(128):
            z = p % ZSUP
            s = 14 * k + z
            if 1 <= z <= ZINT and 1 <= s <= smax:
                m[k, p] = 1.0
    return m


def build_program():
    f32 = mybir.dt.float32
    nc = bacc.Bacc("TRN2", target_bir_lowering=False, debug=False,
                   num_devices=8)
    # pre-packed: [channel, supertile, partition(=yblk*16+z), y_row, x]
    slab = nc.declare_dram_parameter("slab", [4, NSUP, 128, YROWS, NX], f32,
                                     isOutput=False)
    dmats = nc.declare_dram_parameter("dmats", [128, 256], f32, isOutput=False)
    zmask = nc.declare_dram_parameter("zmask", [3, 128], f32, isOutput=False)
    out = nc.declare_dram_parameter("out", [128, NSLOT], f32, isOutput=True)

    AL = mybir.AluOpType
    SQ = mybir.ActivationFunctionType.Square

    with tile.TileContext(nc) as tc:
        with (
            tc.tile_pool(name="const", bufs=1) as cpool,
            tc.tile_pool(name="inp", bufs=2) as inpool,
            tc.tile_pool(name="tmp", bufs=1) as tpool,
            tc.tile_pool(name="ctmp", bufs=2) as ctpool,
            tc.tile_pool(name="psA", bufs=3, space=bass.MemorySpace.PSUM) as psa,
            tc.tile_pool(name="psV", bufs=3, space=bass.MemorySpace.PSUM) as psv,
        ):
            dm = cpool.tile([128, 256], f32, tag="dm")
            nc.sync.dma_start(dm[:], dmats[:])
            zm = cpool.tile([128, 3], f32, tag="zm")
            for k in range(3):
                nc.sync.dma_start(zm[:, k : k + 1], zmask[k, :][:, None])
            acc = cpool.tile([128, NSLOT], f32, tag="acc")
            nc.vector.memset(acc[:], 0.0)

            lhs_D = dm[:, 0:128]
            lhs_VU = dm[:, 128:256]

            unit = 0
            for k in range(3):
                for xh in range(2):
                    x0 = 0 if xh == 0 else 78
                    xo = 1 if xh == 0 else 3   # first out col within tile
                    xn = 80 if xh == 0 else 78  # out col count
                    U = []
                    for c in range(4):
                        t = inpool.tile([128, YROWS, XTW], f32, tag=f"U{c}")
                        nc.sync.dma_start(t[:], slab[c, k, :, :, x0 : x0 + XTW])
                        U.append(t)

                    def cen(c, r0=1, nr=20):
                        return U[c][:, r0 : r0 + nr, xo : xo + xn]

                    def yp(c):
                        return U[c][:, 2:22, xo : xo + xn]

                    def ym(c):
                        return U[c][:, 0:20, xo : xo + xn]

                    def xp(c):
                        return U[c][:, 1:21, xo + 1 : xo + 1 + xn]

                    def xm(c):
                        return U[c][:, 1:21, xo - 1 : xo - 1 + xn]

                    for m in range(3):
                        Dy = tpool.tile([128, 20, 80], f32, tag="dy")
                        nc.vector.tensor_tensor(Dy[:, :, :xn], yp(m), ym(m),
                                                op=AL.subtract)
                        Dx = tpool.tile([128, 20, 80], f32, tag="dx")
                        nc.vector.tensor_tensor(Dx[:, :, :xn], xp(m), xm(m),
                                                op=AL.subtract)
                        NYt = tpool.tile([128, 20, 80], f32, tag="ny")
                        nc.vector.tensor_tensor(NYt[:, :, :xn], yp(m), ym(m),
                                                op=AL.add)
                        NXt = tpool.tile([128, 20, 80], f32, tag="nx")
                        nc.vector.tensor_tensor(NXt[:, :, :xn], xp(m), xm(m),
                                                op=AL.add)
                        T1 = tpool.tile([128, 20, 80], f32, tag="t1")
                        nc.vector.scalar_tensor_tensor(
                            T1[:, :, :xn], Dy[:, :, :xn], 0.5 * RHO, cen(1),
                            op0=AL.mult, op1=AL.mult)
                        T2 = tpool.tile([128, 20, 80], f32, tag="t2")
                        nc.vector.scalar_tensor_tensor(
                            T2[:, :, :xn], Dx[:, :, :xn], 0.5 * RHO, cen(2),
                            op0=AL.mult, op1=AL.mult)
                        S1 = tpool.tile([128, 20, 80], f32, tag="s1")
                        nc.vector.tensor_tensor(S1[:, :, :xn], T1[:, :, :xn],
                                                T2[:, :, :xn], op=AL.add)
                        NS = tpool.tile([128, 20, 80], f32, tag="ns")
                        nc.vector.tensor_tensor(NS[:, :, :xn], NYt[:, :, :xn],
                                                NXt[:, :, :xn], op=AL.add)
                        S2 = tpool.tile([128, 20, 80], f32, tag="s2")
                        nc.vector.scalar_tensor_tensor(
                            S2[:, :, :xn], NS[:, :, :xn], -MU, S1[:, :, :xn],
                            op0=AL.mult, op1=AL.add)
                        Dp = None
                        if m == 1:
                            Dp = tpool.tile([128, 20, 80], f32, tag="dp")
                            nc.vector.tensor_tensor(Dp[:, :, :xn], yp(3), ym(3),
                                                    op=AL.subtract)
                        elif m == 2:
                            Dp = tpool.tile([128, 20, 80], f32, tag="dp")
                            nc.vector.tensor_tensor(Dp[:, :, :xn], xp(3), xm(3),
                                                    op=AL.subtract)

                        for ch in range(4):
                            r0 = 1 + 5 * ch          # input-row of chunk start
                            L = 5 * xn
                            pA = psa.tile([128, 512], f32, tag="psA")
                            nc.tensor.matmul(pA[:, :L], lhs_D, cen(m, r0, 5),
                                             start=True, stop=True)
                            pV = psv.tile([128, 512], f32, tag="psV")
                            if m == 0:
                                nc.tensor.matmul(pV[:, :L], lhs_VU,
                                                 cen(0, r0, 5),
                                                 start=True, stop=False)
                                nc.tensor.matmul(pV[:, :L], lhs_D,
                                                 cen(3, r0, 5),
                                                 start=False, stop=True)
                            else:
                                nc.tensor.matmul(pV[:, :L], lhs_VU,
                                                 cen(m, r0, 5),
                                                 start=True, stop=True)

                            T3 = ctpool.tile([128, 5, 80], f32, tag="t3")
                            nc.vector.tensor_tensor(
                                T3[:, :, :xn], pA[:, :L], cen(0, r0, 5),
                                op=AL.mult)
                            S3 = ctpool.tile([128, 5, 80], f32, tag="s3")
                            nc.vector.tensor_tensor(
                                S3[:, :, :xn],
                                S2[:, 5 * ch : 5 * ch + 5, :xn],
                                T3[:, :, :xn], op=AL.add)
                            R = ctpool.tile([128, 5, 80], f32, tag="s4")
                            if m == 0:
                                nc.vector.tensor_tensor(
                                    R[:, :, :xn], S3[:, :, :xn], pV[:, :L],
                                    op=AL.add)
                            else:
                                S4 = ctpool.tile([128, 5, 80], f32, tag="s4b")
                                nc.vector.tensor_tensor(
                                    S4[:, :, :xn], S3[:, :, :xn], pV[:, :L],
                                    op=AL.add)
                                nc.vector.scalar_tensor_tensor(
                                    R[:, :, :xn],
                                    Dp[:, 5 * ch : 5 * ch + 5, :xn], 0.5,
                                    S4[:, :, :xn], op0=AL.mult, op1=AL.add)

                            sq = ctpool.tile([128, 5, 80], f32, tag="sq")
                            base = (unit * 3 + m) * 6
                            if ch < 3:
                                nc.scalar.activation(
                                    sq[:, :, :xn], R[:, :, :xn], SQ,
                                    scale=zm[:, k : k + 1],
                                    accum_out=acc[:, base + ch : base + ch + 1])
                            else:
                                # rows 16..20: y rows 159,160 are garbage on
                                # y-block 7 (partitions 112..127)
                                nc.scalar.activation(
                                    sq[0:96, :, :xn], R[0:96, :, :xn], SQ,
                                    scale=zm[0:96, k : k + 1],
                                    accum_out=acc[0:96, base + 3 : base + 4])
                                nc.scalar.activation(
                                    sq[96:128, 0:3, :xn], R[96:128, 0:3, :xn],
                                    SQ, scale=zm[96:128, k : k + 1],
                                    accum_out=acc[96:128, base + 4 : base + 5])
                                nc.scalar.activation(
                                    sq[96:112, 3:5, :xn], R[96:112, 3:5, :xn],
                                    SQ, scale=zm[96:112, k : k + 1],
                                    accum_out=acc[96:112, base + 5 : base + 6])
                    unit += 1

            nc.sync.dma_start(out[:], acc[:])
    nc.compile()
    return nc


def _band_matrices_v2():
    """bf16 lhsT matrices, packed [128, 5*128]: D, VU, IP(0.5I), IM(-0.5I),
    IMU(-MU*I)."""
    import ml_dtypes
    D = np.zeros((128, 128), dtype=np.float32)
    VU = np.zeros((128, 128), dtype=np.float32)
    for p in range(128):
        z = p % ZSUP
        if 1 <= z <= ZINT:
            D[p + 1, p] = 0.5
            D[p - 1, p] = -0.5
            VU[p, p] = 6.0 * MU
            VU[p + 1, p] = -MU
            VU[p - 1, p] = -MU
    eye = np.eye(128, dtype=np.float32)
    packed = np.concatenate([D, VU, 0.5 * eye, -0.5 * eye, -MU * eye], axis=1)
    return packed.astype(ml_dtypes.bfloat16)


def _band_matrices_v2():
    """bf16 lhsT matrices packed [128, 5*128]: D, VU, IP(0.5I), IM(-0.5I),
    IMU(-MU*I)."""
    import ml_dtypes
    D = np.zeros((128, 128), dtype=np.float32)
    VU = np.zeros((128, 128), dtype=np.float32)
    for p in range(128):
        z = p % ZSUP
        if 1 <= z <= ZINT:
            D[p + 1, p] = 0.5
            D[p - 1, p] = -0.5
            VU[p, p] = 6.0 * MU
            VU[p + 1, p] = -MU
            VU[p - 1, p] = -MU
    eye = np.eye(128, dtype=np.float32)
    packed = np.concatenate([D, VU, 0.5 * eye, -0.5 * eye, -MU * eye], axis=1)
    return packed.astype(ml_dtypes.bfloat16)


NSLOT2 = 3 * 3 * 8
NRC = 7  # row chunks: six of 3 rows + one of 2


def build_program_v2():
    """bf16 non-conservative variant, engine-balanced.

    Per momentum m the TensorEngine accumulates into PSUM:
      A_m = 0.5*dz(u_m)                                  [banded D]
      V_m = -MU*lap(u_m) + 0.5*d_m(p)   (z-lap banded VU + 6MU center;
            y/x neighbors via -MU*I shifted; dp via D band or +-0.5I shifts)
    The ScalarEngine copies A_m/V_m to bf16 SBUF and does the masked R^2
    accumulation; the VectorEngine (all-bf16 2x ops) does
      Dy, Dx subs; T1=A*uzc; T2=0.5*Dy*uyc; T3=0.5*Dx*uxc;
      S=T1+T2; S2=S+T3; R=S2+V.
    """
    f32 = mybir.dt.float32
    bf16 = mybir.dt.bfloat16
    nc = bacc.Bacc("TRN2", target_bir_lowering=False, debug=False,
                   num_devices=8)
    slab = nc.declare_dram_parameter("slab", [4, NSUP, 128, YROWS, NX], bf16,
                                     isOutput=False)
    dmats = nc.declare_dram_parameter("dmats", [128, 5 * 128], bf16,
                                      isOutput=False)
    zmask = nc.declare_dram_parameter("zmask", [3, 128], f32, isOutput=False)
    out = nc.declare_dram_parameter("out", [128, NSLOT2], f32, isOutput=True)

    AL = mybir.AluOpType
    SQ = mybir.ActivationFunctionType.Square

    with tile.TileContext(nc) as tc:
        with (
            tc.tile_pool(name="const", bufs=1) as cpool,
            tc.tile_pool(name="inp", bufs=2) as inpool,
            tc.tile_pool(name="ctmp", bufs=3) as ctpool,
            tc.tile_pool(name="psA", bufs=1, space=bass.MemorySpace.PSUM) as psa,
            tc.tile_pool(name="psV", bufs=1, space=bass.MemorySpace.PSUM) as psv,
        ):
            dm = cpool.tile([128, 5 * 128], bf16, tag="dm")
            nc.sync.dma_start(dm[:], dmats[:])
            zm = cpool.tile([128, 3], f32, tag="zm")
            for k in range(3):
                nc.sync.dma_start(zm[:, k : k + 1], zmask[k, :][:, None])
            acc = cpool.tile([128, NSLOT2], f32, tag="acc")
            nc.vector.memset(acc[:], 0.0)

            M_D = dm[:, 0:128]
            M_VU = dm[:, 128:256]
            M_IP = dm[:, 256:384]
            M_IM = dm[:, 384:512]
            M_IMU = dm[:, 512:640]

            for k in range(3):
                U = []
                for c in range(4):
                    t = inpool.tile([128, YROWS, NX], bf16, tag=f"U{c}")
                    nc.sync.dma_start(t[:], slab[c, k])
                    U.append(t)

                for rc in range(NRC):
                    r0 = 1 + 3 * rc
                    nr = 3 if rc < 6 else 2
                    NCH = nr * 158

                    def ap(c, dy=0, dx=0):
                        return U[c][:, r0 + dy : r0 + dy + nr,
                                    1 + dx : 159 + dx]

                    # ---- PE ----
                    A = [psa.tile([128, 512], f32, tag=f"psA{m}",
                                  name=f"A{m}_{k}_{rc}", bufs=1)
                         for m in range(3)]
                    V = [psv.tile([128, 512], f32, tag=f"psV{m}",
                                  name=f"V{m}_{k}_{rc}", bufs=1)
                         for m in range(3)]
                    # D group: A_m and dp_z
                    for m in range(3):
                        nc.tensor.matmul(A[m][:, :NCH], M_D, ap(m),
                                         start=True, stop=True)
                    nc.tensor.matmul(V[0][:, :NCH], M_D, ap(3),
                                     start=True, stop=False)
                    # VU group: z-lap + 6MU center
                    for m in range(3):
                        nc.tensor.matmul(V[m][:, :NCH], M_VU, ap(m),
                                         start=(m != 0), stop=False)
                    # IMU group: -MU * (y and x neighbors)
                    for m in range(3):
                        nc.tensor.matmul(V[m][:, :NCH], M_IMU, ap(m, dy=1),
                                         start=False, stop=False)
                        nc.tensor.matmul(V[m][:, :NCH], M_IMU, ap(m, dy=-1),
                                         start=False, stop=False)
                        nc.tensor.matmul(V[m][:, :NCH], M_IMU, ap(m, dx=1),
                                         start=False, stop=False)
                        nc.tensor.matmul(V[m][:, :NCH], M_IMU, ap(m, dx=-1),
                                         start=False, stop=(m == 0))
                    # IP/IM: dp_y, dp_x
                    nc.tensor.matmul(V[1][:, :NCH], M_IP, ap(3, dy=1),
                                     start=False, stop=False)
                    nc.tensor.matmul(V[2][:, :NCH], M_IP, ap(3, dx=1),
                                     start=False, stop=False)
                    nc.tensor.matmul(V[1][:, :NCH], M_IM, ap(3, dy=-1),
                                     start=False, stop=True)
                    nc.tensor.matmul(V[2][:, :NCH], M_IM, ap(3, dx=-1),
                                     start=False, stop=True)

                    # ---- ACT: copy PSUM -> bf16 SBUF ----
                    Ab, Vb = [], []
                    for m in range(3):
                        ab = ctpool.tile([128, 512], bf16, tag=f"ab{m}",
                                         name=f"Ab{m}_{k}_{rc}")
                        nc.scalar.copy(ab[:, :NCH], A[m][:, :NCH])
                        Ab.append(ab)
                        vb = ctpool.tile([128, 512], bf16, tag=f"vb{m}",
                                         name=f"Vb{m}_{k}_{rc}")
                        nc.scalar.copy(vb[:, :NCH], V[m][:, :NCH])
                        Vb.append(vb)

                    # ---- DVE (bf16) ----
                    for m in range(3):
                        Dy = ctpool.tile([128, 3, 158], bf16, tag="dy",
                                         name=f"Dy{m}_{k}_{rc}")
                        nc.vector.tensor_tensor(Dy[:, :nr, :], ap(m, dy=1),
                                                ap(m, dy=-1), op=AL.subtract)
                        Dx = ctpool.tile([128, 3, 158], bf16, tag="dx",
                                         name=f"Dx{m}_{k}_{rc}")
                        nc.vector.tensor_tensor(Dx[:, :nr, :], ap(m, dx=1),
                                                ap(m, dx=-1), op=AL.subtract)
                        T1 = ctpool.tile([128, 512], bf16, tag="t1",
                                         name=f"T1{m}_{k}_{rc}")
                        nc.vector.tensor_tensor(T1[:, :NCH], Ab[m][:, :NCH],
                                                ap(0), op=AL.mult)
                        T2 = ctpool.tile([128, 3, 158], bf16, tag="t2",
                                         name=f"T2{m}_{k}_{rc}")
                        nc.vector.scalar_tensor_tensor(
                            T2[:, :nr, :], Dy[:, :nr, :], 0.5 * RHO, ap(1),
                            op0=AL.mult, op1=AL.mult)
                        T3 = ctpool.tile([128, 3, 158], bf16, tag="t3",
                                         name=f"T3{m}_{k}_{rc}")
                        nc.vector.scalar_tensor_tensor(
                            T3[:, :nr, :], Dx[:, :nr, :], 0.5 * RHO, ap(2),
                            op0=AL.mult, op1=AL.mult)
                        S = ctpool.tile([128, 512], bf16, tag="s",
                                        name=f"S{m}_{k}_{rc}")
                        nc.vector.tensor_tensor(S[:, :NCH], T1[:, :NCH],
                                                T2[:, :nr, :], op=AL.add)
                        S2 = ctpool.tile([128, 512], bf16, tag="s2",
                                         name=f"S2{m}_{k}_{rc}")
                        nc.vector.tensor_tensor(S2[:, :NCH], S[:, :NCH],
                                                T3[:, :nr, :], op=AL.add)
                        R = ctpool.tile([128, 512], bf16, tag="r",
                                        name=f"R{m}_{k}_{rc}")
                        nc.vector.tensor_tensor(R[:, :NCH], S2[:, :NCH],
                                                Vb[m][:, :NCH], op=AL.add)

                        # ---- ACT: masked square-accumulate ----
                        sq = ctpool.tile([128, 512], bf16, tag="sq",
                                         name=f"sq{m}_{k}_{rc}")
                        base = (k * 3 + m) * 8
                        if rc < 6:
                            nc.scalar.activation(
                                sq[:, :NCH], R[:, :NCH], SQ,
                                scale=zm[:, k : k + 1],
                                accum_out=acc[:, base + rc : base + rc + 1])
                        else:
                            # rows 19,20: garbage on y-block 7 (parts 112-127)
                            nc.scalar.activation(
                                sq[0:96, :NCH], R[0:96, :NCH], SQ,
                                scale=zm[0:96, k : k + 1],
                                accum_out=acc[0:96, base + 6 : base + 7])
                            nc.scalar.activation(
                                sq[96:112, :NCH], R[96:112, :NCH], SQ,
                                scale=zm[96:112, k : k + 1],
                                accum_out=acc[96:112, base + 7 : base + 8])

            nc.sync.dma_start(out[:], acc[:])
    nc.compile()
    return nc


NSLOT3 = 3 * 3 * 2


def _amask(zc):
    """[128, NSLOT3] end-mask: slot = (k*3+m)*2 + j; j=0 rows 1-18, j=1 rows
    19-20 (garbage on y-block 7 = partitions 112..127)."""
    zm = _zmask(zc)  # [3, 128]
    m = np.zeros((128, NSLOT3), dtype=np.float32)
    for k in range(3):
        for mm in range(3):
            for j in range(2):
                s = (k * 3 + mm) * 2 + j
                col = zm[k].copy()
                if j == 1:
                    col[112:] = 0.0
                m[:, s] = col
    return m


def build_program_v3():
    """Like v2 but with full-supertile DVE ops (amortizes the per-op pipeline
    bubble), in-place tile reuse, tensor_scalar pre-scales instead of
    scalar_tensor_tensor, ACT squares without per-op masks, and one end-mask
    multiply on the [128, NSLOT3] partial sums."""
    f32 = mybir.dt.float32
    bf16 = mybir.dt.bfloat16
    nc = bacc.Bacc("TRN2", target_bir_lowering=False, debug=False,
                   num_devices=8)
    slab = nc.declare_dram_parameter("slab", [4, NSUP, 128, YROWS, NX], bf16,
                                     isOutput=False)
    dmats = nc.declare_dram_parameter("dmats", [128, 5 * 128], bf16,
                                      isOutput=False)
    amask = nc.declare_dram_parameter("amask", [128, NSLOT3], f32,
                                      isOutput=False)
    out = nc.declare_dram_parameter("out", [128, NSLOT3], f32, isOutput=True)

    AL = mybir.AluOpType
    SQ = mybir.ActivationFunctionType.Square

    with tile.TileContext(nc) as tc:
        with (
            tc.tile_pool(name="const", bufs=1) as cpool,
            tc.tile_pool(name="inp", bufs=2) as inpool,
            tc.tile_pool(name="fld", bufs=2) as fpool,
            tc.tile_pool(name="psA", bufs=1, space=bass.MemorySpace.PSUM) as psa,
            tc.tile_pool(name="psV", bufs=1, space=bass.MemorySpace.PSUM) as psv,
        ):
            dm = cpool.tile([128, 5 * 128], bf16, tag="dm")
            nc.sync.dma_start(dm[:], dmats[:])
            am = cpool.tile([128, NSLOT3], f32, tag="am")
            nc.sync.dma_start(am[:], amask[:])
            acc = cpool.tile([128, NSLOT3], f32, tag="acc")

            M_D = dm[:, 0:128]
            M_VU = dm[:, 128:256]
            M_IP = dm[:, 256:384]
            M_IM = dm[:, 384:512]
            M_IMU = dm[:, 512:640]

            for k in range(3):
                U = []
                for c in range(4):
                    t = inpool.tile([128, YROWS, NX], bf16, tag=f"U{c}")
                    nc.sync.dma_start(t[:], slab[c, k])
                    U.append(t)

                # pre-scaled center factors 0.5*uy, 0.5*ux (full interior)
                HUY = fpool.tile([128, 20, 158], bf16, tag="huy")
                nc.vector.tensor_scalar_mul(HUY[:], U[1][:, 1:21, 1:159],
                                            0.5 * RHO)
                HUX = fpool.tile([128, 20, 158], bf16, tag="hux")
                nc.vector.tensor_scalar_mul(HUX[:], U[2][:, 1:21, 1:159],
                                            0.5 * RHO)

                Ab, Vb = [], []
                for m in range(3):
                    ab = fpool.tile([128, 20, 158], bf16, tag=f"ab{m}",
                                    name=f"Ab{m}_{k}")
                    Ab.append(ab)
                    vb = fpool.tile([128, 20, 158], bf16, tag=f"vb{m}",
                                    name=f"Vb{m}_{k}")
                    Vb.append(vb)

                for rc in range(NRC):
                    r0 = 1 + 3 * rc
                    nr = 3 if rc < 6 else 2
                    NCH = nr * 158

                    def ap(c, dy=0, dx=0):
                        return U[c][:, r0 + dy : r0 + dy + nr,
                                    1 + dx : 159 + dx]

                    A = [psa.tile([128, 512], f32, tag=f"psA{m}",
                                  name=f"A{m}_{k}_{rc}")
                         for m in range(3)]
                    V = [psv.tile([128, 512], f32, tag=f"psV{m}",
                                  name=f"V{m}_{k}_{rc}")
                         for m in range(3)]
                    for m in range(3):
                        nc.tensor.matmul(A[m][:, :NCH], M_D, ap(m),
                                         start=True, stop=True)
                    nc.tensor.matmul(V[0][:, :NCH], M_D, ap(3),
                                     start=True, stop=False)
                    for m in range(3):
                        nc.tensor.matmul(V[m][:, :NCH], M_VU, ap(m),
                                         start=(m != 0), stop=False)
                    for m in range(3):
                        nc.tensor.matmul(V[m][:, :NCH], M_IMU, ap(m, dy=1),
                                         start=False, stop=False)
                        nc.tensor.matmul(V[m][:, :NCH], M_IMU, ap(m, dy=-1),
                                         start=False, stop=False)
                        nc.tensor.matmul(V[m][:, :NCH], M_IMU, ap(m, dx=1),
                                         start=False, stop=False)
                        nc.tensor.matmul(V[m][:, :NCH], M_IMU, ap(m, dx=-1),
                                         start=False, stop=(m == 0))
                    nc.tensor.matmul(V[1][:, :NCH], M_IP, ap(3, dy=1),
                                     start=False, stop=False)
                    nc.tensor.matmul(V[2][:, :NCH], M_IP, ap(3, dx=1),
                                     start=False, stop=False)
                    nc.tensor.matmul(V[1][:, :NCH], M_IM, ap(3, dy=-1),
                                     start=False, stop=True)
                    nc.tensor.matmul(V[2][:, :NCH], M_IM, ap(3, dx=-1),
                                     start=False, stop=True)

                    # ACT: drain PSUM chunks into the full-supertile tiles
                    rows = slice(r0 - 1, r0 - 1 + nr)
                    for m in range(3):
                        nc.scalar.copy(Ab[m][:, rows, :], A[m][:, :NCH])
                        nc.scalar.copy(Vb[m][:, rows, :], V[m][:, :NCH])

                # DVE: full-supertile assembly (in-place chains)
                for m in range(3):
                    Dy = fpool.tile([128, 20, 158], bf16, tag="dy",
                                    name=f"Dy{m}_{k}")
                    nc.vector.tensor_tensor(Dy[:], U[m][:, 2:22, 1:159],
                                            U[m][:, 0:20, 1:159],
                                            op=AL.subtract)
                    Dx = fpool.tile([128, 20, 158], bf16, tag="dx",
                                    name=f"Dx{m}_{k}")
                    nc.vector.tensor_tensor(Dx[:], U[m][:, 1:21, 2:160],
                                            U[m][:, 1:21, 0:158],
                                            op=AL.subtract)
                    # T1 = Ab*uzc (in place over Ab)
                    nc.vector.tensor_tensor(Ab[m][:], Ab[m][:],
                                            U[0][:, 1:21, 1:159], op=AL.mult)
                    # T2 = Dy*0.5uy (in place over Dy); T3 likewise
                    nc.vector.tensor_tensor(Dy[:], Dy[:], HUY[:], op=AL.mult)
                    nc.vector.tensor_tensor(Dx[:], Dx[:], HUX[:], op=AL.mult)
                    # S = T1+T2 -> Ab; S2 = S+T3 -> Ab; R = S2+Vb -> Vb
                    nc.vector.tensor_tensor(Ab[m][:], Ab[m][:], Dy[:],
                                            op=AL.add)
                    nc.vector.tensor_tensor(Ab[m][:], Ab[m][:], Dx[:],
                                            op=AL.add)
                    nc.vector.tensor_tensor(Vb[m][:], Ab[m][:], Vb[m][:],
                                            op=AL.add)

                    # ACT: plain square-accumulate, split rows 1-18 / 19-20
                    s = (k * 3 + m) * 2
                    sq = fpool.tile([128, 20, 158], bf16, tag="sq",
                                    name=f"sq{m}_{k}")
                    nc.scalar.activation(sq[:, 0:18, :], Vb[m][:, 0:18, :],
                                         SQ, accum_out=acc[:, s : s + 1])
                    nc.scalar.activation(sq[:, 18:20, :], Vb[m][:, 18:20, :],
                                         SQ, accum_out=acc[:, s + 1 : s + 2])

            # end-mask and ship
            nc.vector.tensor_tensor(acc[:], acc[:], am[:], op=AL.mult)
            nc.sync.dma_start(out[:], acc[:])
    nc.compile()
    return nc


def build_program_v5():
    """Like v2 but with full-supertile DVE ops (amortizes the per-op pipeline
    bubble), in-place tile reuse, tensor_scalar pre-scales instead of
    scalar_tensor_tensor, ACT squares without per-op masks, and one end-mask
    multiply on the [128, NSLOT3] partial sums."""
    f32 = mybir.dt.float32
    bf16 = mybir.dt.bfloat16
    nc = bacc.Bacc("TRN2", target_bir_lowering=False, debug=False,
                   num_devices=8)
    slab = nc.declare_dram_parameter("slab", [4, NSUP, 128, YROWS, NX], bf16,
                                     isOutput=False)
    dmats = nc.declare_dram_parameter("dmats", [128, 5 * 128], bf16,
                                      isOutput=False)
    amask = nc.declare_dram_parameter("amask", [128, NSLOT3], f32,
                                      isOutput=False)
    out = nc.declare_dram_parameter("out", [128, NSLOT3], f32, isOutput=True)

    AL = mybir.AluOpType
    SQ = mybir.ActivationFunctionType.Square

    with tile.TileContext(nc) as tc:
        with (
            tc.tile_pool(name="const", bufs=1) as cpool,
            tc.tile_pool(name="inp", bufs=2) as inpool,
            tc.tile_pool(name="fld", bufs=2) as fpool,
            tc.tile_pool(name="psA", bufs=1, space=bass.MemorySpace.PSUM) as psa,
            tc.tile_pool(name="psV", bufs=1, space=bass.MemorySpace.PSUM) as psv,
        ):
            dm = cpool.tile([128, 5 * 128], bf16, tag="dm")
            nc.sync.dma_start(dm[:], dmats[:])
            am = cpool.tile([128, NSLOT3], f32, tag="am")
            nc.sync.dma_start(am[:], amask[:])
            acc = cpool.tile([128, NSLOT3], f32, tag="acc")

            M_D = dm[:, 0:128]
            M_VU = dm[:, 128:256]
            M_IP = dm[:, 256:384]
            M_IM = dm[:, 384:512]
            M_IMU = dm[:, 512:640]

            for k in range(3):
                U = []
                for c in range(4):
                    t = inpool.tile([128, YROWS, NX], bf16, tag=f"U{c}")
                    nc.sync.dma_start(t[:], slab[c, k])
                    U.append(t)

                # pre-scaled center factors 0.5*uy, 0.5*ux (full interior)
                HUY = fpool.tile([128, 20, 158], bf16, tag="huy")
                nc.vector.tensor_scalar_mul(HUY[:], U[1][:, 1:21, 1:159],
                                            0.5 * RHO)
                HUX = fpool.tile([128, 20, 158], bf16, tag="hux")
                nc.vector.tensor_scalar_mul(HUX[:], U[2][:, 1:21, 1:159],
                                            0.5 * RHO)

                Ab, Vb = [], []
                for m in range(3):
                    ab = fpool.tile([128, 20, 158], bf16, tag=f"ab{m}",
                                    name=f"Ab{m}_{k}")
                    Ab.append(ab)
                    vb = fpool.tile([128, 20, 158], bf16, tag=f"vb{m}",
                                    name=f"Vb{m}_{k}")
                    Vb.append(vb)

                for rc in range(NRC):
                    r0 = 1 + 3 * rc
                    nr = 3 if rc < 6 else 2
                    NCH = nr * 158

                    def ap(c, dy=0, dx=0):
                        return U[c][:, r0 + dy : r0 + dy + nr,
                                    1 + dx : 159 + dx]

                    A = [psa.tile([128, 512], f32, tag=f"psA{m}",
                                  name=f"A{m}_{k}_{rc}", bufs=1)
                         for m in range(3)]
                    V = [psv.tile([128, 512], f32, tag=f"psV{m}",
                                  name=f"V{m}_{k}_{rc}",
                                  bufs=(2 if m < 2 else 1))
                         for m in range(3)]
                    for m in range(3):
                        nc.tensor.matmul(A[m][:, :NCH], M_D, ap(m),
                                         start=True, stop=True)
                    nc.tensor.matmul(V[0][:, :NCH], M_D, ap(3),
                                     start=True, stop=False)
                    for m in range(3):
                        nc.tensor.matmul(V[m][:, :NCH], M_VU, ap(m),
                                         start=(m != 0), stop=False)
                    for m in range(3):
                        nc.tensor.matmul(V[m][:, :NCH], M_IMU, ap(m, dy=1),
                                         start=False, stop=False)
                        nc.tensor.matmul(V[m][:, :NCH], M_IMU, ap(m, dy=-1),
                                         start=False, stop=False)
                        nc.tensor.matmul(V[m][:, :NCH], M_IMU, ap(m, dx=1),
                                         start=False, stop=False)
                        nc.tensor.matmul(V[m][:, :NCH], M_IMU, ap(m, dx=-1),
                                         start=False, stop=(m == 0))
                    nc.tensor.matmul(V[1][:, :NCH], M_IP, ap(3, dy=1),
                                     start=False, stop=False)
                    nc.tensor.matmul(V[2][:, :NCH], M_IP, ap(3, dx=1),
                                     start=False, stop=False)
                    nc.tensor.matmul(V[1][:, :NCH], M_IM, ap(3, dy=-1),
                                     start=False, stop=True)
                    nc.tensor.matmul(V[2][:, :NCH], M_IM, ap(3, dx=-1),
                                     start=False, stop=True)

                    # ACT: drain PSUM chunks into the full-supertile tiles
                    rows = slice(r0 - 1, r0 - 1 + nr)
                    for m in range(3):
                        nc.scalar.copy(Ab[m][:, rows, :], A[m][:, :NCH])
                        nc.scalar.copy(Vb[m][:, rows, :], V[m][:, :NCH])

                # DVE: full-supertile assembly (in-place chains)
                for m in range(3):
                    Dy = fpool.tile([128, 20, 158], bf16, tag="dy",
                                    name=f"Dy{m}_{k}")
                    nc.vector.tensor_tensor(Dy[:], U[m][:, 2:22, 1:159],
                                            U[m][:, 0:20, 1:159],
                                            op=AL.subtract)
                    Dx = fpool.tile([128, 20, 158], bf16, tag="dx",
                                    name=f"Dx{m}_{k}")
                    nc.vector.tensor_tensor(Dx[:], U[m][:, 1:21, 2:160],
                                            U[m][:, 1:21, 0:158],
                                            op=AL.subtract)
                    # T1 = Ab*uzc (in place over Ab)
                    nc.vector.tensor_tensor(Ab[m][:], Ab[m][:],
                                            U[0][:, 1:21, 1:159], op=AL.mult)
                    # T2 = Dy*0.5uy (in place over Dy); T3 likewise
                    nc.vector.tensor_tensor(Dy[:], Dy[:], HUY[:], op=AL.mult)
                    nc.vector.tensor_tensor(Dx[:], Dx[:], HUX[:], op=AL.mult)
                    # S = T1+T2 -> Ab; S2 = S+T3 -> Ab; R = S2+Vb -> Vb
                    nc.vector.tensor_tensor(Ab[m][:], Ab[m][:], Dy[:],
                                            op=AL.add)
                    nc.vector.tensor_tensor(Ab[m][:], Ab[m][:], Dx[:],
                                            op=AL.add)
                    nc.vector.tensor_tensor(Vb[m][:], Ab[m][:], Vb[m][:],
                                            op=AL.add)

                    # ACT: plain square-accumulate, split rows 1-18 / 19-20
                    s = (k * 3 + m) * 2
                    sq = fpool.tile([128, 20, 158], bf16, tag="sq",
                                    name=f"sq{m}_{k}")
                    nc.scalar.activation(sq[:, 0:18, :], Vb[m][:, 0:18, :],
                                         SQ, accum_out=acc[:, s : s + 1])
                    nc.scalar.activation(sq[:, 18:20, :], Vb[m][:, 18:20, :],
                                         SQ, accum_out=acc[:, s + 1 : s + 2])

            # end-mask and ship
            nc.vector.tensor_tensor(acc[:], acc[:], am[:], op=AL.mult)
            nc.sync.dma_start(out[:], acc[:])
    nc.compile()
    return nc




def build_program_v4():
    """Like v2 but with full-supertile DVE ops (amortizes the per-op pipeline
    bubble), in-place tile reuse, tensor_scalar pre-scales instead of
    scalar_tensor_tensor, ACT squares without per-op masks, and one end-mask
    multiply on the [128, NSLOT3] partial sums."""
    f32 = mybir.dt.float32
    bf16 = mybir.dt.bfloat16
    nc = bacc.Bacc("TRN2", target_bir_lowering=False, debug=False,
                   num_devices=8)
    slab = nc.declare_dram_parameter("slab", [4, NSUP, 128, YROWS, NX], bf16,
                                     isOutput=False)
    dmats = nc.declare_dram_parameter("dmats", [128, 5 * 128], bf16,
                                      isOutput=False)
    amask = nc.declare_dram_parameter("amask", [128, NSLOT3], f32,
                                      isOutput=False)
    out = nc.declare_dram_parameter("out", [128, NSLOT3], f32, isOutput=True)

    AL = mybir.AluOpType
    SQ = mybir.ActivationFunctionType.Square

    with tile.TileContext(nc) as tc:
        with (
            tc.tile_pool(name="const", bufs=1) as cpool,
            tc.tile_pool(name="inp", bufs=2) as inpool,
            tc.tile_pool(name="fld", bufs=2) as fpool,
            tc.tile_pool(name="psAV", bufs=1, space=bass.MemorySpace.PSUM) as psav,
        ):
            dm = cpool.tile([128, 5 * 128], bf16, tag="dm")
            nc.sync.dma_start(dm[:], dmats[:])
            am = cpool.tile([128, NSLOT3], f32, tag="am")
            nc.sync.dma_start(am[:], amask[:])
            acc = cpool.tile([128, NSLOT3], f32, tag="acc")

            M_D = dm[:, 0:128]
            M_VU = dm[:, 128:256]
            M_IP = dm[:, 256:384]
            M_IM = dm[:, 384:512]
            M_IMU = dm[:, 512:640]

            for k in range(3):
                U = []
                for c in range(4):
                    t = inpool.tile([128, YROWS, NX], bf16, tag=f"U{c}")
                    nc.sync.dma_start(t[:], slab[c, k])
                    U.append(t)

                # pre-scaled center factors 0.5*uy, 0.5*ux (full interior)
                HUY = fpool.tile([128, 20, 158], bf16, tag="huy")
                nc.vector.tensor_scalar_mul(HUY[:], U[1][:, 1:21, 1:159],
                                            0.5 * RHO)
                HUX = fpool.tile([128, 20, 158], bf16, tag="hux")
                nc.vector.tensor_scalar_mul(HUX[:], U[2][:, 1:21, 1:159],
                                            0.5 * RHO)

                AVb = [fpool.tile([128, 2, 20, 158], bf16, tag=f"avb{m}",
                                  name=f"AVb{m}_{k}") for m in range(3)]
                Ab = [t[:, 0] for t in AVb]
                Vb = [t[:, 1] for t in AVb]

                for rc in range(NRC):
                    r0 = 1 + 3 * rc
                    nr = 3 if rc < 6 else 2
                    NCH = nr * 158

                    def ap(c, dy=0, dx=0):
                        return U[c][:, r0 + dy : r0 + dy + nr,
                                    1 + dx : 159 + dx]

                    AV = [psav.tile([128, 1024], f32, tag=f"psAV{m}",
                                    name=f"AV{m}_{k}_{rc}")
                          for m in range(3)]
                    A = [t[:, 0:512] for t in AV]
                    V = [t[:, 512:1024] for t in AV]
                    for m in range(3):
                        nc.tensor.matmul(A[m][:, :NCH], M_D, ap(m),
                                         start=True, stop=True)
                    nc.tensor.matmul(V[0][:, :NCH], M_D, ap(3),
                                     start=True, stop=False)
                    for m in range(3):
                        nc.tensor.matmul(V[m][:, :NCH], M_VU, ap(m),
                                         start=(m != 0), stop=False)
                    for m in range(3):
                        nc.tensor.matmul(V[m][:, :NCH], M_IMU, ap(m, dy=1),
                                         start=False, stop=False)
                        nc.tensor.matmul(V[m][:, :NCH], M_IMU, ap(m, dy=-1),
                                         start=False, stop=False)
                        nc.tensor.matmul(V[m][:, :NCH], M_IMU, ap(m, dx=1),
                                         start=False, stop=False)
                        nc.tensor.matmul(V[m][:, :NCH], M_IMU, ap(m, dx=-1),
                                         start=False, stop=(m == 0))
                    nc.tensor.matmul(V[1][:, :NCH], M_IP, ap(3, dy=1),
                                     start=False, stop=False)
                    nc.tensor.matmul(V[2][:, :NCH], M_IP, ap(3, dx=1),
                                     start=False, stop=False)
                    nc.tensor.matmul(V[1][:, :NCH], M_IM, ap(3, dy=-1),
                                     start=False, stop=True)
                    nc.tensor.matmul(V[2][:, :NCH], M_IM, ap(3, dx=-1),
                                     start=False, stop=True)

                    # ACT: drain PSUM chunks into the full-supertile tiles
                    rows = slice(r0 - 1, r0 - 1 + nr)
                    for m in range(3):
                        src2 = AV[m].rearrange("p (b n) -> p b n", b=2)
                        nc.scalar.copy(AVb[m][:, :, rows, :],
                                       src2[:, :, :NCH])

                # DVE: full-supertile assembly (in-place chains)
                for m in range(3):
                    Dy = fpool.tile([128, 20, 158], bf16, tag="dy",
                                    name=f"Dy{m}_{k}")
                    nc.vector.tensor_tensor(Dy[:], U[m][:, 2:22, 1:159],
                                            U[m][:, 0:20, 1:159],
                                            op=AL.subtract)
                    Dx = fpool.tile([128, 20, 158], bf16, tag="dx",
                                    name=f"Dx{m}_{k}")
                    nc.vector.tensor_tensor(Dx[:], U[m][:, 1:21, 2:160],
                                            U[m][:, 1:21, 0:158],
                                            op=AL.subtract)
                    # T1 = Ab*uzc (in place over Ab)
                    nc.vector.tensor_tensor(Ab[m][:], Ab[m][:],
                                            U[0][:, 1:21, 1:159], op=AL.mult)
                    # T2 = Dy*0.5uy (in place over Dy); T3 likewise
                    nc.vector.tensor_tensor(Dy[:], Dy[:], HUY[:], op=AL.mult)
                    nc.vector.tensor_tensor(Dx[:], Dx[:], HUX[:], op=AL.mult)
                    # S = T1+T2 -> Ab; S2 = S+T3 -> Ab; R = S2+Vb -> Vb
                    nc.vector.tensor_tensor(Ab[m][:], Ab[m][:], Dy[:],
                                            op=AL.add)
                    nc.vector.tensor_tensor(Ab[m][:], Ab[m][:], Dx[:],
                                            op=AL.add)
                    nc.vector.tensor_tensor(Vb[m][:], Ab[m][:], Vb[m][:],
                                            op=AL.add)

                    # ACT: plain square-accumulate, split rows 1-18 / 19-20
                    s = (k * 3 + m) * 2
                    sq = fpool.tile([128, 20, 158], bf16, tag="sq",
                                    name=f"sq{m}_{k}")
                    nc.scalar.activation(sq[:, 0:18, :], Vb[m][:, 0:18, :],
                                         SQ, accum_out=acc[:, s : s + 1])
                    nc.scalar.activation(sq[:, 18:20, :], Vb[m][:, 18:20, :],
                                         SQ, accum_out=acc[:, s + 1 : s + 2])

            # end-mask and ship
            nc.vector.tensor_tensor(acc[:], acc[:], am[:], op=AL.mult)
            nc.sync.dma_start(out[:], acc[:])
    nc.compile()
    return nc




def _band_matrices_x2():
    """bf16 matrices packed [128, 5*128], all scaled x2 vs _band_matrices_v2:
    D2 (dz band, +-1), VU2 (12MU diag / -2MU off), IP2 (+I), IM2 (-I),
    IMU2 (-2MU*I).  Kernel computes R' = 2R; host divides the loss by 4."""
    import ml_dtypes
    D = np.zeros((128, 128), dtype=np.float32)
    VU = np.zeros((128, 128), dtype=np.float32)
    for p in range(128):
        z = p % ZSUP
        if 1 <= z <= ZINT:
            D[p + 1, p] = 1.0
            D[p - 1, p] = -1.0
            VU[p, p] = 12.0 * MU
            VU[p + 1, p] = -2.0 * MU
            VU[p - 1, p] = -2.0 * MU
    eye = np.eye(128, dtype=np.float32)
    packed = np.concatenate([D, VU, eye, -eye, -2.0 * MU * eye], axis=1)
    return packed.astype(ml_dtypes.bfloat16)


NSLOT6 = 3 * 3 * 2  # (supertile x momentum) x 2 row-groups


def build_program_v6():
    """All-STT DVE chain (4x mode) + fused masked square-accum on DVE;
    ACT only drains PSUM->SBUF with multi-bank strided copies; PE is the
    v5 banded bf16 scheme with x2 weights (loss /4 on host)."""
    f32 = mybir.dt.float32
    bf16 = mybir.dt.bfloat16
    nc = bacc.Bacc("TRN2", target_bir_lowering=False, debug=False,
                   num_devices=8)
    # host repacks channel-inside-partition: [NSUP, 128, 4, YROWS, NX]
    slab = nc.declare_dram_parameter("slab", [NSUP, 128, 4, YROWS, NX], bf16,
                                     isOutput=False)
    dmats = nc.declare_dram_parameter("dmats", [128, 5 * 128], bf16,
                                      isOutput=False)
    zmask = nc.declare_dram_parameter("zmask", [3, 128], f32, isOutput=False)
    out = nc.declare_dram_parameter("out", [128, NSLOT6], f32, isOutput=True)

    AL = mybir.AluOpType

    with tile.TileContext(nc) as tc:
        with (
            tc.tile_pool(name="const", bufs=1) as cpool,
            tc.tile_pool(name="inp", bufs=2) as inpool,
            tc.tile_pool(name="drn", bufs=2) as dpool,
            tc.tile_pool(name="tmp", bufs=2) as tpool,
            tc.tile_pool(name="plo", bufs=1, space=bass.MemorySpace.PSUM) as plo,
            tc.tile_pool(name="phi", bufs=1, space=bass.MemorySpace.PSUM) as phi,
        ):
            dm = cpool.tile([128, 5 * 128], bf16, tag="dm")
            nc.sync.dma_start(dm[:], dmats[:])
            zm = cpool.tile([128, 3], f32, tag="zm")
            for k in range(3):
                nc.sync.dma_start(zm[:, k : k + 1], zmask[k, :][:, None])
            acc = cpool.tile([128, NSLOT6], f32, tag="acc")
            nc.vector.memset(acc[:], 0.0)

            M_D = dm[:, 0:128]
            M_VU = dm[:, 128:256]
            M_IP = dm[:, 256:384]
            M_IM = dm[:, 384:512]
            M_IMU = dm[:, 512:640]

            for k in range(3):
                U = inpool.tile([128, 4, YROWS, NX], bf16, tag="U",
                                name=f"U_{k}")
                nc.sync.dma_start(U[:], slab[k])

                def ap(c, rc, dy=0, dx=0):
                    r0 = 1 + 3 * rc
                    nr = 3 if rc < 6 else 2
                    return U[:, c, r0 + dy : r0 + dy + nr, 1 + dx : 159 + dx]

                def cen(c):
                    return U[:, c, 1:21, 1:159]

                for m in range(3):
                    A2b = dpool.tile([128, 20, 158], bf16, tag="a2b",
                                     name=f"A2b{m}_{k}")
                    Vb = dpool.tile([128, 20, 158], bf16, tag="vb",
                                    name=f"Vb{m}_{k}")
                    # ---- PE: A then V, chunked into lo(0-3)/hi(4-6) banks
                    for half, rng, ptag in ((0, range(0, 4), "alo"),
                                            (1, range(4, 7), "ahi")):
                        pool_ = plo if half == 0 else phi
                        nb = 4 if half == 0 else 3
                        At = pool_.tile([128, nb, 512], f32, tag=f"p{half}",
                                        name=f"A{m}_{k}_{half}")
                        for c in rng:
                            nr = 3 if c < 6 else 2
                            NCH = nr * 158
                            nc.tensor.matmul(At[:, c - (0 if half == 0 else 4),
                                                :NCH],
                                             M_D, ap(m, c),
                                             start=True, stop=True)
                        # drain this half
                        if half == 0:
                            nc.scalar.copy(A2b[:, 0:12, :], At[:, :, 0:474])
                        else:
                            nc.scalar.copy(A2b[:, 12:18, :],
                                           At[:, 0:2, 0:474])
                            nc.scalar.copy(A2b[:, 18:20, :],
                                           At[:, 2:3, 0:316])
                    for half, rng in ((0, range(0, 4)), (1, range(4, 7))):
                        pool_ = plo if half == 0 else phi
                        nb = 4 if half == 0 else 3
                        Vt = pool_.tile([128, nb, 512], f32, tag=f"p{half}",
                                        name=f"V{m}_{k}_{half}")
                        for c in rng:
                            nr = 3 if c < 6 else 2
                            NCH = nr * 158
                            vt = Vt[:, c - (0 if half == 0 else 4), :NCH]
                            nc.tensor.matmul(vt, M_VU, ap(m, c),
                                             start=True, stop=False)
                            nc.tensor.matmul(vt, M_IMU, ap(m, c, dy=1),
                                             start=False, stop=False)
                            nc.tensor.matmul(vt, M_IMU, ap(m, c, dy=-1),
                                             start=False, stop=False)
                            nc.tensor.matmul(vt, M_IMU, ap(m, c, dx=1),
                                             start=False, stop=False)
                            nc.tensor.matmul(vt, M_IMU, ap(m, c, dx=-1),
                                             start=False, stop=False)
                            if m == 0:
                                nc.tensor.matmul(vt, M_D, ap(3, c),
                                                 start=False, stop=True)
                            elif m == 1:
                                nc.tensor.matmul(vt, M_IP, ap(3, c, dy=1),
                                                 start=False, stop=False)
                                nc.tensor.matmul(vt, M_IM, ap(3, c, dy=-1),
                                                 start=False, stop=True)
                            else:
                                nc.tensor.matmul(vt, M_IP, ap(3, c, dx=1),
                                                 start=False, stop=False)
                                nc.tensor.matmul(vt, M_IM, ap(3, c, dx=-1),
                                                 start=False, stop=True)
                        if half == 0:
                            nc.scalar.copy(Vb[:, 0:12, :], Vt[:, :, 0:474])
                        else:
                            nc.scalar.copy(Vb[:, 12:18, :], Vt[:, 0:2, 0:474])
                            nc.scalar.copy(Vb[:, 18:20, :],
                                           Vt[:, 2:3, 0:316])

                    # ---- DVE: STT chain, all 4x ----
                    def t20(tag):
                        return tpool.tile([128, 20, 158], bf16, tag=tag,
                                          name=f"{tag}{m}_{k}")

                    yp = U[:, m, 2:22, 1:159]
                    ym = U[:, m, 0:20, 1:159]
                    xp = U[:, m, 1:21, 2:160]
                    xm = U[:, m, 1:21, 0:158]
                    Dy = t20("dy")
                    nc.vector.scalar_tensor_tensor(Dy[:], yp, 1.0, ym,
                                                   op0=AL.mult,
                                                   op1=AL.subtract)
                    Dx = t20("dx")
                    nc.vector.scalar_tensor_tensor(Dx[:], xp, 1.0, xm,
                                                   op0=AL.mult,
                                                   op1=AL.subtract)
                    M1 = t20("m1")
                    nc.vector.scalar_tensor_tensor(M1[:], Dy[:], 1.0, cen(1),
                                                   op0=AL.mult, op1=AL.mult)
                    M2 = t20("m2")
                    nc.vector.scalar_tensor_tensor(M2[:], Dx[:], 1.0, cen(2),
                                                   op0=AL.mult, op1=AL.mult)
                    T1 = t20("t1")
                    nc.vector.scalar_tensor_tensor(T1[:], A2b[:], 1.0, cen(0),
                                                   op0=AL.mult, op1=AL.mult)
                    S1 = t20("s1")
                    nc.vector.scalar_tensor_tensor(S1[:], M1[:], 1.0, M2[:],
                                                   op0=AL.mult, op1=AL.add)
                    S2 = t20("s2")
                    nc.vector.scalar_tensor_tensor(S2[:], S1[:], 1.0, T1[:],
                                                   op0=AL.mult, op1=AL.add)
                    R = t20("r")
                    nc.vector.scalar_tensor_tensor(R[:], S2[:], 1.0, Vb[:],
                                                   op0=AL.mult, op1=AL.add)
                    # fused masked square + accumulate (zm scales once)
                    sq = t20("sq")
                    s = (k * 3 + m) * 2
                    nc.vector.scalar_tensor_tensor(
                        sq[:, 0:18, :], R[:, 0:18, :], zm[:, k : k + 1],
                        R[:, 0:18, :], op0=AL.mult, op1=AL.mult,
                        accum_out=acc[:, s : s + 1])
                    nc.vector.scalar_tensor_tensor(
                        sq[0:112, 18:20, :], R[0:112, 18:20, :],
                        zm[0:112, k : k + 1], R[0:112, 18:20, :],
                        op0=AL.mult, op1=AL.mult,
                        accum_out=acc[0:112, s + 1 : s + 2])

            nc.sync.dma_start(out[:], acc[:])
    nc.compile()
    return nc


# ---------------------------------------------------------------------------
# v7: fp8 DoubleRow PE stencils + bf16 identity-matmul adds into V-PSUM,
# DVE tensor_tensor products at 2x, gpsimd M2 product, ACT drains + squares.
# x-half units, 4-bank PSUM regions rotated 2-deep.

NSLOT7 = 3 * 3 * 2 * 2  # k x m x xh x rowgroup


def _w_bands_v7():
    """fp8 weight pair tensor [128, 14, 2, 128] + bf16 identity [128,128].

    All stencil weights are x2 (kernel computes R' = 2R; host divides by 4).
    Pair table (slot: (channel, dy, dx, matrix)):
      pair 3*m+?? -> see _PAIRS7 below.
    """
    import ml_dtypes
    D2 = np.zeros((128, 128), dtype=np.float32)
    VU2 = np.zeros((128, 128), dtype=np.float32)
    for p in range(128):
        z = p % ZSUP
        if 1 <= z <= ZINT:
            D2[p + 1, p] = 1.0
            D2[p - 1, p] = -1.0
            VU2[p, p] = 12.0 * MU
            VU2[p + 1, p] = -2.0 * MU
            VU2[p - 1, p] = -2.0 * MU
    eye = np.eye(128, dtype=np.float32)
    mats = {"D2": D2, "VU2": VU2, "IP2": eye, "IM2": -eye,
            "IMU2": -2.0 * MU * eye, "Z": np.zeros((128, 128), np.float32)}
    W = np.zeros((14, 128, 2, 128), dtype=np.float32)
    for pi, pair in enumerate(_PAIRS7):
        for sl in range(2):
            W[pi, :, sl, :] = mats[pair[sl][3]]
    # -> [128, 14, 2, 128]
    W = np.transpose(W, (1, 0, 2, 3)).copy()
    return (W.astype(ml_dtypes.float8_e4m3),
            eye.astype(ml_dtypes.bfloat16))


# pair index layout: 0-2 = A pairs for m=0,1,2; then V pairs:
# m=0: 3,4,5   m=1: 6,7,8,9   m=2: 10,11,12,13
_PAIRS7 = [
    [(0, 0, 0, "D2"), (0, 0, 1, "Z")],
    [(1, 0, 0, "D2"), (1, 0, 1, "Z")],
    [(2, 0, 0, "D2"), (2, 0, 1, "Z")],
    # V m=0
    [(0, 0, 0, "VU2"), (3, 0, 0, "D2")],
    [(0, 1, 0, "IMU2"), (0, -1, 0, "IMU2")],
    [(0, 0, 1, "IMU2"), (0, 0, -1, "IMU2")],
    # V m=1
    [(1, 0, 0, "VU2"), (3, 1, 0, "IP2")],
    [(1, 1, 0, "IMU2"), (1, -1, 0, "IMU2")],
    [(1, 0, 1, "IMU2"), (1, 0, -1, "IMU2")],
    [(3, -1, 0, "IM2"), (3, -1, 1, "Z")],
    # V m=2
    [(2, 0, 0, "VU2"), (3, 0, 1, "IP2")],
    [(2, 1, 0, "IMU2"), (2, -1, 0, "IMU2")],
    [(2, 0, 1, "IMU2"), (2, 0, -1, "IMU2")],
    [(3, 0, -1, "IM2"), (3, 0, 0, "Z")],
]

_VPAIRS7 = {0: [3, 4, 5], 1: [6, 7, 8, 9], 2: [10, 11, 12, 13]}

# row chunks per x-half unit: (out_row0, nrows); out rows 0..19
_CH7 = [(0, 6), (6, 6), (12, 6), (18, 2)]


def build_program_v7():
    from concourse.ap import AP as _AP
    f32 = mybir.dt.float32
    bf16 = mybir.dt.bfloat16
    f8 = mybir.dt.float8e4
    DRm = mybir.MatmulPerfMode.DoubleRow
    AL = mybir.AluOpType
    SQf = mybir.ActivationFunctionType.Square

    nc = bacc.Bacc("TRN2", target_bir_lowering=False, debug=False,
                   num_devices=8)
    slabb = nc.declare_dram_parameter("slabb", [NSUP, 128, 3, YROWS, NX],
                                      bf16, isOutput=False)
    slab8 = nc.declare_dram_parameter("slab8", [NSUP, 128, 4, YROWS, NX],
                                      f8, isOutput=False)
    wp = nc.declare_dram_parameter("wp", [128, 14, 2, 128], f8,
                                   isOutput=False)
    wi = nc.declare_dram_parameter("wi", [128, 128], bf16, isOutput=False)
    out = nc.declare_dram_parameter("out", [128, NSLOT7], f32, isOutput=True)

    with tile.TileContext(nc) as tc:
        with (
            tc.tile_pool(name="const", bufs=1) as cpool,
            tc.tile_pool(name="inp", bufs=2) as inpool,
            tc.tile_pool(name="drn", bufs=3) as dpool,
            tc.tile_pool(name="tmp", bufs=3) as tpool,
            tc.tile_pool(name="pv", bufs=2, space=bass.MemorySpace.PSUM) as pv,
        ):
            W = cpool.tile([128, 14, 2, 128], f8, tag="W")
            nc.sync.dma_start(W[:], wp[:])
            WI = cpool.tile([128, 128], bf16, tag="WI")
            nc.sync.dma_start(WI[:], wi[:])
            acc = cpool.tile([128, NSLOT7], f32, tag="acc")
            nc.vector.memset(acc[:], 0.0)

            pending_sq = []

            def flush_sq():
                while pending_sq:
                    pending_sq.pop(0)()

            for k in range(3):
                B = inpool.tile([128, 3, YROWS, NX], bf16, tag="B",
                                name=f"B_{k}")
                nc.sync.dma_start(B[:], slabb[k])
                F = inpool.tile([128, 4, YROWS, NX], f8, tag="F",
                                name=f"F_{k}")
                nc.sync.dma_start(F[:], slab8[k])

                fp_stride = F[:].ap[0][0]

                def pairAP(pi, r0, nr, x0):
                    (c0, dy0, dx0, _), (c1, dy1, dx1, _) = _PAIRS7[pi]
                    s0 = F[:, c0, 1 + r0 + dy0 : 1 + r0 + dy0 + nr,
                           x0 + dx0 : x0 + dx0 + 79]
                    s1 = F[:, c1, 1 + r0 + dy1 : 1 + r0 + dy1 + nr,
                           x0 + dx1 : x0 + dx1 + 79]
                    return _AP(s0.tensor, s0.offset,
                               [[fp_stride, 128], [s1.offset - s0.offset, 2],
                                [NX, nr], [1, 79]])

                for m in range(3):
                    for xh in range(2):
                        x0 = 1 + 79 * xh
                        kk, mm = k, m  # capture
                        P = pv.tile([128, 4, 512], f32, tag="pv",
                                    name=f"P_{k}_{m}_{xh}")
                        # ---- A fill ----
                        for c, (r0, nr) in enumerate(_CH7):
                            nc.tensor.matmul(P[:, c, : nr * 79],
                                             W[:, m], pairAP(m, r0, nr, x0),
                                             start=True, stop=True,
                                             perf_mode=DRm)
                        # ---- A drain (one strided op; rows 20-23 junk) ----
                        A2b = dpool.tile([128, 24, 79], bf16, tag="a2b",
                                         name=f"A2b_{k}_{m}_{xh}")
                        nc.scalar.copy(A2b[:], P[:, :, 0:474])
                        # ---- V fill (group stays open; ids close it) ----
                        for c, (r0, nr) in enumerate(_CH7):
                            vps = _VPAIRS7[m]
                            for j, pi in enumerate(vps):
                                nc.tensor.matmul(P[:, c, : nr * 79],
                                                 W[:, pi],
                                                 pairAP(pi, r0, nr, x0),
                                                 start=(j == 0), stop=False,
                                                 perf_mode=DRm)
                        # ---- DVE products ----
                        def t20(tag):
                            return tpool.tile([128, 20, 79], bf16, tag=tag,
                                              name=f"{tag}_{k}_{m}_{xh}")

                        Dy = t20("dy")
                        nc.vector.tensor_tensor(Dy[:], B[:, m, 2:22, x0:x0 + 79],
                                                B[:, m, 0:20, x0:x0 + 79],
                                                op=AL.subtract)
                        Dx = t20("dx")
                        nc.vector.tensor_tensor(Dx[:],
                                                B[:, m, 1:21, x0 + 1:x0 + 80],
                                                B[:, m, 1:21, x0 - 1:x0 + 78],
                                                op=AL.subtract)
                        M1 = t20("m1")
                        nc.vector.tensor_tensor(M1[:], Dy[:],
                                                B[:, 1, 1:21, x0:x0 + 79],
                                                op=AL.mult)
                        M2 = t20("m2")
                        nc.gpsimd.tensor_tensor(M2[:], Dx[:],
                                                B[:, 2, 1:21, x0:x0 + 79],
                                                op=AL.mult)
                        T1 = t20("t1")
                        nc.vector.tensor_tensor(T1[:], A2b[:, 0:20, :],
                                                B[:, 0, 1:21, x0:x0 + 79],
                                                op=AL.mult)
                        # ---- ids: accumulate products into V ----
                        for fi, Ft in enumerate((M1, M2, T1)):
                            for c, (r0, nr) in enumerate(_CH7):
                                nc.tensor.matmul(
                                    P[:, c, : nr * 79], WI[:],
                                    Ft[:, r0 : r0 + nr, :],
                                    start=False,
                                    stop=(fi == 2))
                        # ---- SQ (deferred one xunit for pipelining) ----
                        s = ((k * 3 + m) * 2 + xh) * 2
                        Pq = _AP(P[:].tensor, P[:].offset,
                                 [[P[:].ap[0][0], 128], [512, 3], [1, 474]])

                        def do_sq(P=P, Pq=Pq, s=s):
                            sq1 = tpool.tile([128, 3, 474], bf16, tag="sq1",
                                             name=f"sq1_{s}")
                            nc.scalar.activation(sq1[:], Pq, SQf,
                                                 accum_out=acc[:, s : s + 1])
                            sq2 = tpool.tile([128, 158], bf16, tag="sq2",
                                             name=f"sq2_{s}")
                            nc.scalar.activation(
                                sq2[0:112], P[0:112, 3, 0:158], SQf,
                                accum_out=acc[0:112, s + 1 : s + 2])

                        pending_sq.append(do_sq)
                        if len(pending_sq) > 1:
                            pending_sq.pop(0)()
            flush_sq()
            nc.sync.dma_start(out[:], acc[:])
    nc.compile()
    return nc


def build_program_v8():
    """v7 with 2-stage software-pipelined emission: per xunit i the engine
    queues carry [PE: A-fill(i), ids(i-1), V-fill(i)], [ACT: drain(i),
    SQ(i-1)], [DVE: T1(i-1), Dy/Dx/M1(i)], [Pool: M2(i)] so no engine
    head-of-line blocks on another engine's latency."""
    from concourse.ap import AP as _AP
    f32 = mybir.dt.float32
    bf16 = mybir.dt.bfloat16
    f8 = mybir.dt.float8e4
    DRm = mybir.MatmulPerfMode.DoubleRow
    AL = mybir.AluOpType
    SQf = mybir.ActivationFunctionType.Square

    nc = bacc.Bacc("TRN2", target_bir_lowering=False, debug=False,
                   num_devices=8)
    slabb = nc.declare_dram_parameter("slabb", [NSUP, 128, 3, YROWS, NX],
                                      bf16, isOutput=False)
    slab8 = nc.declare_dram_parameter("slab8", [NSUP, 128, 4, YROWS, NX],
                                      f8, isOutput=False)
    wp = nc.declare_dram_parameter("wp", [128, 14, 2, 128], f8,
                                   isOutput=False)
    wi = nc.declare_dram_parameter("wi", [128, 128], bf16, isOutput=False)
    out = nc.declare_dram_parameter("out", [128, NSLOT7], f32, isOutput=True)

    units = [(k, m, xh) for k in range(3) for m in range(3)
             for xh in range(2)]

    with tile.TileContext(nc) as tc:
        with (
            tc.tile_pool(name="const", bufs=1) as cpool,
            tc.tile_pool(name="inp", bufs=2) as inpool,
            tc.tile_pool(name="drn", bufs=3) as dpool,
            tc.tile_pool(name="tmp", bufs=3) as tpool,
            tc.tile_pool(name="pv", bufs=2, space=bass.MemorySpace.PSUM) as pv,
        ):
            W = cpool.tile([128, 14, 2, 128], f8, tag="W")
            nc.sync.dma_start(W[:], wp[:])
            WI = cpool.tile([128, 128], bf16, tag="WI")
            nc.sync.dma_start(WI[:], wi[:])
            acc = cpool.tile([128, NSLOT7], f32, tag="acc")
            nc.vector.memset(acc[:], 0.0)

            BF = {}  # k -> (B tile, F tile)

            def load_k(k):
                if k in BF or k >= 3:
                    return
                B = inpool.tile([128, 3, YROWS, NX], bf16, tag="B",
                                name=f"B_{k}")
                F = inpool.tile([128, 4, YROWS, NX], f8, tag="F",
                                name=f"F_{k}")
                nc.sync.dma_start(B[:], slabb[k])
                nc.sync.dma_start(F[:], slab8[k])
                BF[k] = (B, F)

            load_k(0)

            def pairAP(F, pi, r0, nr, x0):
                fp_stride = F[:].ap[0][0]
                (c0, dy0, dx0, _), (c1, dy1, dx1, _) = _PAIRS7[pi]
                s0 = F[:, c0, 1 + r0 + dy0 : 1 + r0 + dy0 + nr,
                       x0 + dx0 : x0 + dx0 + 79]
                s1 = F[:, c1, 1 + r0 + dy1 : 1 + r0 + dy1 + nr,
                       x0 + dx1 : x0 + dx1 + 79]
                return _AP(s0.tensor, s0.offset,
                           [[fp_stride, 128], [s1.offset - s0.offset, 2],
                            [NX, nr], [1, 79]])

            st = {}  # unit index -> state dict

            def stage_T1(j):
                u = st[j]
                T1 = tpool.tile([128, 20, 79], bf16, tag="t1",
                                name=f"t1_{j}")
                nc.vector.tensor_tensor(
                    T1[:], u["A2b"][:, 0:20, :],
                    u["B"][:, 0, 1:21, u["x0"]:u["x0"] + 79], op=AL.mult)
                u["T1"] = T1

            def stage_ids(j):
                u = st[j]
                for fi, Ft in enumerate((u["M1"], u["M2"], u["T1"])):
                    for c, (r0, nr) in enumerate(_CH7):
                        nc.tensor.matmul(u["P"][:, c, : nr * 79], WI[:],
                                         Ft[:, r0 : r0 + nr, :],
                                         start=False, stop=(fi == 2))

            def stage_sq(j):
                u = st[j]
                P, s = u["P"], u["s"]
                Pq = _AP(P[:].tensor, P[:].offset,
                         [[P[:].ap[0][0], 128], [512, 3], [1, 474]])
                sq2 = tpool.tile([128, 158], bf16, tag="sq2",
                                 name=f"sq2_{j}")
                nc.scalar.activation(sq2[0:112], P[0:112, 3, 0:158], SQf,
                                     accum_out=acc[0:112, s + 1 : s + 2])
                sq1 = tpool.tile([128, 3, 474], bf16, tag="sq1",
                                 name=f"sq1_{j}")
                nc.scalar.activation(sq1[:], Pq, SQf,
                                     accum_out=acc[:, s : s + 1])
                del st[j]

            for i, (k, m, xh) in enumerate(units):
                B, F = BF[k]
                x0 = 1 + 79 * xh
                u = {"B": B, "x0": x0,
                     "s": ((k * 3 + m) * 2 + xh) * 2}
                st[i] = u

                # DVE: finish previous unit's T1 first (A2b ready long ago)
                if i - 1 in st:
                    stage_T1(i - 1)

                # PE: A fill
                P = pv.tile([128, 4, 512], f32, tag="pv", name=f"P_{i}")
                u["P"] = P
                for c in (3, 0, 1, 2):
                    r0, nr = _CH7[c]
                    nc.tensor.matmul(P[:, c, : nr * 79], W[:, m],
                                     pairAP(F, m, r0, nr, x0),
                                     start=True, stop=True, perf_mode=DRm)
                # ACT: A drain
                A2b = dpool.tile([128, 24, 79], bf16, tag="a2b",
                                 name=f"a2b_{i}")
                nc.scalar.copy(A2b[:], P[:, :, 0:474])
                u["A2b"] = A2b

                # PE: previous unit's ids; ACT: previous unit's SQ
                if i - 1 in st:
                    stage_ids(i - 1)
                    stage_sq(i - 1)

                # PE: V fill
                for c, (r0, nr) in enumerate(_CH7):
                    for j, pi in enumerate(_VPAIRS7[m]):
                        nc.tensor.matmul(P[:, c, : nr * 79], W[:, pi],
                                         pairAP(F, pi, r0, nr, x0),
                                         start=(j == 0), stop=False,
                                         perf_mode=DRm)

                # DVE: this unit's Dy/Dx/M1 ; Pool: M2
                Dy = tpool.tile([128, 20, 79], bf16, tag="dy", name=f"dy_{i}")
                nc.vector.tensor_tensor(Dy[:], B[:, m, 2:22, x0:x0 + 79],
                                        B[:, m, 0:20, x0:x0 + 79],
                                        op=AL.subtract)
                Dx = tpool.tile([128, 20, 79], bf16, tag="dx", name=f"dx_{i}")
                nc.vector.tensor_tensor(Dx[:], B[:, m, 1:21, x0 + 1:x0 + 80],
                                        B[:, m, 1:21, x0 - 1:x0 + 78],
                                        op=AL.subtract)
                M1 = tpool.tile([128, 20, 79], bf16, tag="m1", name=f"m1_{i}")
                nc.vector.tensor_tensor(M1[:], Dy[:],
                                        B[:, 1, 1:21, x0:x0 + 79],
                                        op=AL.mult)
                u["M1"] = M1
                M2 = tpool.tile([128, 20, 79], bf16, tag="m2", name=f"m2_{i}")
                nc.gpsimd.tensor_tensor(M2[:], Dx[:],
                                        B[:, 2, 1:21, x0:x0 + 79],
                                        op=AL.mult)
                u["M2"] = M2

                # prefetch next supertile mid-way through this one
                if m == 0 and xh == 1:
                    load_k(k + 1)

            # drain the pipeline
            last = len(units) - 1
            stage_T1(last)
            stage_ids(last)
            stage_sq(last)

            nc.sync.dma_start(out[:], acc[:])
    nc.compile()
    return nc


def build_program_v10(order="v8"):
    """v7 with 2-stage software-pipelined emission: per xunit i the engine
    queues carry [PE: A-fill(i), ids(i-1), V-fill(i)], [ACT: drain(i),
    SQ(i-1)], [DVE: T1(i-1), Dy/Dx/M1(i)], [Pool: M2(i)] so no engine
    head-of-line blocks on another engine's latency."""
    from concourse.ap import AP as _AP
    f32 = mybir.dt.float32
    bf16 = mybir.dt.bfloat16
    f8 = mybir.dt.float8e4
    DRm = mybir.MatmulPerfMode.DoubleRow
    AL = mybir.AluOpType
    SQf = mybir.ActivationFunctionType.Square

    nc = bacc.Bacc("TRN2", target_bir_lowering=False, debug=False,
                   num_devices=8)
    slabb = nc.declare_dram_parameter("slabb", [NSUP, 128, 3, YROWS, NX],
                                      bf16, isOutput=False)
    slab8 = nc.declare_dram_parameter("slab8", [NSUP, 128, 4, YROWS, NX],
                                      f8, isOutput=False)
    wp = nc.declare_dram_parameter("wp", [128, 14, 2, 128], f8,
                                   isOutput=False)
    wi = nc.declare_dram_parameter("wi", [128, 128], bf16, isOutput=False)
    out = nc.declare_dram_parameter("out", [128, NSLOT7], f32, isOutput=True)

    units = [(k, m, xh) for k in range(3) for m in range(3)
             for xh in range(2)]

    with tile.TileContext(nc) as tc:
        with (
            tc.tile_pool(name="const", bufs=1) as cpool,
            tc.tile_pool(name="inp", bufs=2) as inpool,
            tc.tile_pool(name="drn", bufs=3) as dpool,
            tc.tile_pool(name="tmp", bufs=3) as tpool,
            tc.tile_pool(name="pv", bufs=2, space=bass.MemorySpace.PSUM) as pv,
        ):
            W = cpool.tile([128, 14, 2, 128], f8, tag="W")
            nc.sync.dma_start(W[:], wp[:])
            WI = cpool.tile([128, 128], bf16, tag="WI")
            nc.sync.dma_start(WI[:], wi[:])
            acc = cpool.tile([128, NSLOT7], f32, tag="acc")
            nc.vector.memset(acc[:], 0.0)

            BF = {}  # k -> (B tile, F tile)

            def load_k(k):
                if k in BF or k >= 3:
                    return
                B = inpool.tile([128, 3, YROWS, NX], bf16, tag="B",
                                name=f"B_{k}")
                nc.sync.dma_start(B[:], slabb[k])
                F = inpool.tile([128, 4, YROWS, NX], f8, tag="F",
                                name=f"F_{k}")
                nc.sync.dma_start(F[:], slab8[k])
                BF[k] = (B, F)

            load_k(0)

            def pairAP(F, pi, r0, nr, x0):
                fp_stride = F[:].ap[0][0]
                (c0, dy0, dx0, _), (c1, dy1, dx1, _) = _PAIRS7[pi]
                s0 = F[:, c0, 1 + r0 + dy0 : 1 + r0 + dy0 + nr,
                       x0 + dx0 : x0 + dx0 + 79]
                s1 = F[:, c1, 1 + r0 + dy1 : 1 + r0 + dy1 + nr,
                       x0 + dx1 : x0 + dx1 + 79]
                return _AP(s0.tensor, s0.offset,
                           [[fp_stride, 128], [s1.offset - s0.offset, 2],
                            [NX, nr], [1, 79]])

            st = {}  # unit index -> state dict

            def stage_T1(j):
                u = st[j]
                T1 = tpool.tile([128, 20, 79], bf16, tag="t1",
                                name=f"t1_{j}")
                nc.vector.tensor_tensor(
                    T1[:], u["A2b"][:, 0:20, :],
                    u["B"][:, 0, 1:21, u["x0"]:u["x0"] + 79], op=AL.mult)
                u["T1"] = T1

            def stage_ids(j):
                u = st[j]
                for fi, Ft in enumerate((u["M1"], u["M2"], u["T1"])):
                    for c, (r0, nr) in enumerate(_CH7):
                        nc.tensor.matmul(u["P"][:, c, : nr * 79], WI[:],
                                         Ft[:, r0 : r0 + nr, :],
                                         start=False, stop=(fi == 2))

            def stage_sq(j):
                u = st[j]
                P, s = u["P"], u["s"]
                Pq = _AP(P[:].tensor, P[:].offset,
                         [[P[:].ap[0][0], 128], [512, 3], [1, 474]])
                sq1 = tpool.tile([128, 3, 474], bf16, tag="sq1",
                                 name=f"sq1_{j}")
                nc.scalar.activation(sq1[:], Pq, SQf,
                                     accum_out=acc[:, s : s + 1])
                sq2 = tpool.tile([128, 158], bf16, tag="sq2",
                                 name=f"sq2_{j}")
                nc.scalar.activation(sq2[0:112], P[0:112, 3, 0:158], SQf,
                                     accum_out=acc[0:112, s + 1 : s + 2])
                del st[j]

            for i, (k, m, xh) in enumerate(units):
                B, F = BF[k]
                x0 = 1 + 79 * xh
                u = {"B": B, "x0": x0,
                     "s": ((k * 3 + m) * 2 + xh) * 2}
                st[i] = u

                # DVE: finish previous unit's T1 first (A2b ready long ago)
                if i - 1 in st:
                    stage_T1(i - 1)

                # PE: A fill
                P = pv.tile([128, 4, 512], f32, tag="pv", name=f"P_{i}")
                u["P"] = P
                for c, (r0, nr) in enumerate(_CH7):
                    nc.tensor.matmul(P[:, c, : nr * 79], W[:, m],
                                     pairAP(F, m, r0, nr, x0),
                                     start=True, stop=True, perf_mode=DRm)
                A2b = dpool.tile([128, 24, 79], bf16, tag="a2b",
                                 name=f"a2b_{i}")
                u["A2b"] = A2b

                def drain(P=P, A2b=A2b):
                    nc.scalar.copy(A2b[:], P[:, :, 0:474])

                prev = i - 1 in st
                if order == "v8":
                    drain()
                    if prev:
                        stage_ids(i - 1)
                        stage_sq(i - 1)
                elif order == "sqfirst":
                    if prev:
                        stage_ids(i - 1)
                        stage_sq(i - 1)
                    drain()
                elif order == "idsfirst":
                    if prev:
                        stage_ids(i - 1)
                    drain()
                    if prev:
                        stage_sq(i - 1)

                # PE: V fill
                for c, (r0, nr) in enumerate(_CH7):
                    for j, pi in enumerate(_VPAIRS7[m]):
                        nc.tensor.matmul(P[:, c, : nr * 79], W[:, pi],
                                         pairAP(F, pi, r0, nr, x0),
                                         start=(j == 0), stop=False,
                                         perf_mode=DRm)

                # DVE: this unit's Dy/Dx/M1 ; Pool: M2
                Dy = tpool.tile([128, 20, 79], bf16, tag="dy", name=f"dy_{i}")
                nc.vector.tensor_tensor(Dy[:], B[:, m, 2:22, x0:x0 + 79],
                                        B[:, m, 0:20, x0:x0 + 79],
                                        op=AL.subtract)
                Dx = tpool.tile([128, 20, 79], bf16, tag="dx", name=f"dx_{i}")
                nc.vector.tensor_tensor(Dx[:], B[:, m, 1:21, x0 + 1:x0 + 80],
                                        B[:, m, 1:21, x0 - 1:x0 + 78],
                                        op=AL.subtract)
                M1 = tpool.tile([128, 20, 79], bf16, tag="m1", name=f"m1_{i}")
                nc.vector.tensor_tensor(M1[:], Dy[:],
                                        B[:, 1, 1:21, x0:x0 + 79],
                                        op=AL.mult)
                u["M1"] = M1
                M2 = tpool.tile([128, 20, 79], bf16, tag="m2", name=f"m2_{i}")
                nc.gpsimd.tensor_tensor(M2[:], Dx[:],
                                        B[:, 2, 1:21, x0:x0 + 79],
                                        op=AL.mult)
                u["M2"] = M2

                # prefetch next supertile mid-way through this one
                if m == 0 and xh == 1:
                    load_k(k + 1)

            # drain the pipeline
            last = len(units) - 1
            stage_T1(last)
            stage_ids(last)
            stage_sq(last)

            nc.sync.dma_start(out[:], acc[:])
    nc.compile()
    return nc




def build_program_v9():
    """v8 + split A-drain (bank pairs) so V-fill chunks 0-1 start early,
    ACT queue [drain-a, drain-b, SQ1, SQ2], Dx emitted first so Pool's M2
    starts sooner."""
    from concourse.ap import AP as _AP
    f32 = mybir.dt.float32
    bf16 = mybir.dt.bfloat16
    f8 = mybir.dt.float8e4
    DRm = mybir.MatmulPerfMode.DoubleRow
    AL = mybir.AluOpType
    SQf = mybir.ActivationFunctionType.Square

    nc = bacc.Bacc("TRN2", target_bir_lowering=False, debug=False,
                   num_devices=8)
    slabb = nc.declare_dram_parameter("slabb", [NSUP, 128, 3, YROWS, NX],
                                      bf16, isOutput=False)
    slab8 = nc.declare_dram_parameter("slab8", [NSUP, 128, 4, YROWS, NX],
                                      f8, isOutput=False)
    wp = nc.declare_dram_parameter("wp", [128, 14, 2, 128], f8,
                                   isOutput=False)
    wi = nc.declare_dram_parameter("wi", [128, 128], bf16, isOutput=False)
    out = nc.declare_dram_parameter("out", [128, NSLOT7], f32, isOutput=True)

    units = [(k, m, xh) for k in range(3) for m in range(3)
             for xh in range(2)]

    with tile.TileContext(nc) as tc:
        with (
            tc.tile_pool(name="const", bufs=1) as cpool,
            tc.tile_pool(name="inp", bufs=2) as inpool,
            tc.tile_pool(name="drn", bufs=3) as dpool,
            tc.tile_pool(name="tmp", bufs=3) as tpool,
            tc.tile_pool(name="pv", bufs=2, space=bass.MemorySpace.PSUM) as pv,
        ):
            W = cpool.tile([128, 14, 2, 128], f8, tag="W")
            nc.sync.dma_start(W[:], wp[:])
            WI = cpool.tile([128, 128], bf16, tag="WI")
            nc.sync.dma_start(WI[:], wi[:])
            acc = cpool.tile([128, NSLOT7], f32, tag="acc")
            nc.vector.memset(acc[:], 0.0)

            BF = {}  # k -> (B tile, F tile)

            def load_k(k):
                if k in BF or k >= 3:
                    return
                B = inpool.tile([128, 3, YROWS, NX], bf16, tag="B",
                                name=f"B_{k}")
                nc.sync.dma_start(B[:], slabb[k])
                F = inpool.tile([128, 4, YROWS, NX], f8, tag="F",
                                name=f"F_{k}")
                nc.sync.dma_start(F[:], slab8[k])
                BF[k] = (B, F)

            load_k(0)

            def pairAP(F, pi, r0, nr, x0):
                fp_stride = F[:].ap[0][0]
                (c0, dy0, dx0, _), (c1, dy1, dx1, _) = _PAIRS7[pi]
                s0 = F[:, c0, 1 + r0 + dy0 : 1 + r0 + dy0 + nr,
                       x0 + dx0 : x0 + dx0 + 79]
                s1 = F[:, c1, 1 + r0 + dy1 : 1 + r0 + dy1 + nr,
                       x0 + dx1 : x0 + dx1 + 79]
                return _AP(s0.tensor, s0.offset,
                           [[fp_stride, 128], [s1.offset - s0.offset, 2],
                            [NX, nr], [1, 79]])

            st = {}  # unit index -> state dict

            def stage_T1(j):
                u = st[j]
                T1 = tpool.tile([128, 20, 79], bf16, tag="t1",
                                name=f"t1_{j}")
                nc.vector.tensor_tensor(
                    T1[:], u["A2b"][:, 0:20, :],
                    u["B"][:, 0, 1:21, u["x0"]:u["x0"] + 79], op=AL.mult)
                u["T1"] = T1

            def stage_ids(j):
                u = st[j]
                for fi, Ft in enumerate((u["M1"], u["M2"], u["T1"])):
                    for c, (r0, nr) in enumerate(_CH7):
                        nc.tensor.matmul(u["P"][:, c, : nr * 79], WI[:],
                                         Ft[:, r0 : r0 + nr, :],
                                         start=False, stop=(fi == 2))

            def stage_sq(j):
                u = st[j]
                P, s = u["P"], u["s"]
                Pq = _AP(P[:].tensor, P[:].offset,
                         [[P[:].ap[0][0], 128], [512, 3], [1, 474]])
                sq1 = tpool.tile([128, 3, 474], bf16, tag="sq1",
                                 name=f"sq1_{j}")
                nc.scalar.activation(sq1[:], Pq, SQf,
                                     accum_out=acc[:, s : s + 1])
                sq2 = tpool.tile([128, 158], bf16, tag="sq2",
                                 name=f"sq2_{j}")
                nc.scalar.activation(sq2[0:112], P[0:112, 3, 0:158], SQf,
                                     accum_out=acc[0:112, s + 1 : s + 2])
                del st[j]

            for i, (k, m, xh) in enumerate(units):
                B, F = BF[k]
                x0 = 1 + 79 * xh
                u = {"B": B, "x0": x0,
                     "s": ((k * 3 + m) * 2 + xh) * 2}
                st[i] = u

                # DVE: finish previous unit's T1 first (A2b ready long ago)
                if i - 1 in st:
                    stage_T1(i - 1)

                # PE: A fill
                P = pv.tile([128, 4, 512], f32, tag="pv", name=f"P_{i}")
                u["P"] = P
                for c, (r0, nr) in enumerate(_CH7):
                    nc.tensor.matmul(P[:, c, : nr * 79], W[:, m],
                                     pairAP(F, m, r0, nr, x0),
                                     start=True, stop=True, perf_mode=DRm)
                # ACT: A drain, split into bank pairs so V chunks 0-1 can
                # start as soon as the first half lands
                A2b = dpool.tile([128, 24, 79], bf16, tag="a2b",
                                 name=f"a2b_{i}")
                nc.scalar.copy(A2b[:, 0:12, :], P[:, 0:2, 0:474])
                nc.scalar.copy(A2b[:, 12:24, :], P[:, 2:4, 0:474])
                u["A2b"] = A2b

                # PE: previous unit's ids; ACT: previous unit's SQ
                if i - 1 in st:
                    stage_ids(i - 1)
                    stage_sq(i - 1)

                # PE: V fill
                for c, (r0, nr) in enumerate(_CH7):
                    for j, pi in enumerate(_VPAIRS7[m]):
                        nc.tensor.matmul(P[:, c, : nr * 79], W[:, pi],
                                         pairAP(F, pi, r0, nr, x0),
                                         start=(j == 0), stop=False,
                                         perf_mode=DRm)

                # DVE: Dx first so Pool's M2 can start early
                Dx = tpool.tile([128, 20, 79], bf16, tag="dx", name=f"dx_{i}")
                nc.vector.tensor_tensor(Dx[:], B[:, m, 1:21, x0 + 1:x0 + 80],
                                        B[:, m, 1:21, x0 - 1:x0 + 78],
                                        op=AL.subtract)
                M2 = tpool.tile([128, 20, 79], bf16, tag="m2", name=f"m2_{i}")
                nc.gpsimd.tensor_tensor(M2[:], Dx[:],
                                        B[:, 2, 1:21, x0:x0 + 79],
                                        op=AL.mult)
                u["M2"] = M2
                Dy = tpool.tile([128, 20, 79], bf16, tag="dy", name=f"dy_{i}")
                nc.vector.tensor_tensor(Dy[:], B[:, m, 2:22, x0:x0 + 79],
                                        B[:, m, 0:20, x0:x0 + 79],
                                        op=AL.subtract)
                M1 = tpool.tile([128, 20, 79], bf16, tag="m1", name=f"m1_{i}")
                nc.vector.tensor_tensor(M1[:], Dy[:],
                                        B[:, 1, 1:21, x0:x0 + 79],
                                        op=AL.mult)
                u["M1"] = M1

                # prefetch next supertile mid-way through this one
                if m == 0 and xh == 1:
                    load_k(k + 1)

            # drain the pipeline
            last = len(units) - 1
            stage_T1(last)
            stage_ids(last)
            stage_sq(last)

            nc.sync.dma_start(out[:], acc[:])
    nc.compile()
    return nc


def build_program_v9a():
    """v8 + split A-drain (bank pairs) so V-fill chunks 0-1 start early,
    ACT queue [drain-a, drain-b, SQ1, SQ2], Dx emitted first so Pool's M2
    starts sooner."""
    from concourse.ap import AP as _AP
    f32 = mybir.dt.float32
    bf16 = mybir.dt.bfloat16
    f8 = mybir.dt.float8e4
    DRm = mybir.MatmulPerfMode.DoubleRow
    AL = mybir.AluOpType
    SQf = mybir.ActivationFunctionType.Square

    nc = bacc.Bacc("TRN2", target_bir_lowering=False, debug=False,
                   num_devices=8)
    slabb = nc.declare_dram_parameter("slabb", [NSUP, 128, 3, YROWS, NX],
                                      bf16, isOutput=False)
    slab8 = nc.declare_dram_parameter("slab8", [NSUP, 128, 4, YROWS, NX],
                                      f8, isOutput=False)
    wp = nc.declare_dram_parameter("wp", [128, 14, 2, 128], f8,
                                   isOutput=False)
    wi = nc.declare_dram_parameter("wi", [128, 128], bf16, isOutput=False)
    out = nc.declare_dram_parameter("out", [128, NSLOT7], f32, isOutput=True)

    units = [(k, m, xh) for k in range(3) for m in range(3)
             for xh in range(2)]

    with tile.TileContext(nc) as tc:
        with (
            tc.tile_pool(name="const", bufs=1) as cpool,
            tc.tile_pool(name="inp", bufs=2) as inpool,
            tc.tile_pool(name="drn", bufs=3) as dpool,
            tc.tile_pool(name="tmp", bufs=3) as tpool,
            tc.tile_pool(name="pv", bufs=2, space=bass.MemorySpace.PSUM) as pv,
        ):
            W = cpool.tile([128, 14, 2, 128], f8, tag="W")
            nc.sync.dma_start(W[:], wp[:])
            WI = cpool.tile([128, 128], bf16, tag="WI")
            nc.sync.dma_start(WI[:], wi[:])
            acc = cpool.tile([128, NSLOT7], f32, tag="acc")
            nc.vector.memset(acc[:], 0.0)

            BF = {}  # k -> (B tile, F tile)

            def load_k(k):
                if k in BF or k >= 3:
                    return
                B = inpool.tile([128, 3, YROWS, NX], bf16, tag="B",
                                name=f"B_{k}")
                nc.sync.dma_start(B[:], slabb[k])
                F = inpool.tile([128, 4, YROWS, NX], f8, tag="F",
                                name=f"F_{k}")
                nc.sync.dma_start(F[:], slab8[k])
                BF[k] = (B, F)

            load_k(0)

            def pairAP(F, pi, r0, nr, x0):
                fp_stride = F[:].ap[0][0]
                (c0, dy0, dx0, _), (c1, dy1, dx1, _) = _PAIRS7[pi]
                s0 = F[:, c0, 1 + r0 + dy0 : 1 + r0 + dy0 + nr,
                       x0 + dx0 : x0 + dx0 + 79]
                s1 = F[:, c1, 1 + r0 + dy1 : 1 + r0 + dy1 + nr,
                       x0 + dx1 : x0 + dx1 + 79]
                return _AP(s0.tensor, s0.offset,
                           [[fp_stride, 128], [s1.offset - s0.offset, 2],
                            [NX, nr], [1, 79]])

            st = {}  # unit index -> state dict

            def stage_T1(j):
                u = st[j]
                T1 = tpool.tile([128, 20, 79], bf16, tag="t1",
                                name=f"t1_{j}")
                nc.vector.tensor_tensor(
                    T1[:], u["A2b"][:, 0:20, :],
                    u["B"][:, 0, 1:21, u["x0"]:u["x0"] + 79], op=AL.mult)
                u["T1"] = T1

            def stage_ids(j):
                u = st[j]
                for fi, Ft in enumerate((u["M1"], u["M2"], u["T1"])):
                    for c, (r0, nr) in enumerate(_CH7):
                        nc.tensor.matmul(u["P"][:, c, : nr * 79], WI[:],
                                         Ft[:, r0 : r0 + nr, :],
                                         start=False, stop=(fi == 2))

            def stage_sq(j):
                u = st[j]
                P, s = u["P"], u["s"]
                Pq = _AP(P[:].tensor, P[:].offset,
                         [[P[:].ap[0][0], 128], [512, 3], [1, 474]])
                sq1 = tpool.tile([128, 3, 474], bf16, tag="sq1",
                                 name=f"sq1_{j}")
                nc.scalar.activation(sq1[:], Pq, SQf,
                                     accum_out=acc[:, s : s + 1])
                sq2 = tpool.tile([128, 158], bf16, tag="sq2",
                                 name=f"sq2_{j}")
                nc.scalar.activation(sq2[0:112], P[0:112, 3, 0:158], SQf,
                                     accum_out=acc[0:112, s + 1 : s + 2])
                del st[j]

            for i, (k, m, xh) in enumerate(units):
                B, F = BF[k]
                x0 = 1 + 79 * xh
                u = {"B": B, "x0": x0,
                     "s": ((k * 3 + m) * 2 + xh) * 2}
                st[i] = u

                # DVE: finish previous unit's T1 first (A2b ready long ago)
                if i - 1 in st:
                    stage_T1(i - 1)

                # PE: A fill
                P = pv.tile([128, 4, 512], f32, tag="pv", name=f"P_{i}")
                u["P"] = P
                for c, (r0, nr) in enumerate(_CH7):
                    nc.tensor.matmul(P[:, c, : nr * 79], W[:, m],
                                     pairAP(F, m, r0, nr, x0),
                                     start=True, stop=True, perf_mode=DRm)
                # ACT: A drain, split into bank pairs so V chunks 0-1 can
                # start as soon as the first half lands
                A2b = dpool.tile([128, 24, 79], bf16, tag="a2b",
                                 name=f"a2b_{i}")
                nc.scalar.copy(A2b[:, 0:12, :], P[:, 0:2, 0:474])
                nc.scalar.copy(A2b[:, 12:24, :], P[:, 2:4, 0:474])
                u["A2b"] = A2b

                # PE: previous unit's ids; ACT: previous unit's SQ
                if i - 1 in st:
                    stage_ids(i - 1)
                    stage_sq(i - 1)

                # PE: V fill
                for c, (r0, nr) in enumerate(_CH7):
                    for j, pi in enumerate(_VPAIRS7[m]):
                        nc.tensor.matmul(P[:, c, : nr * 79], W[:, pi],
                                         pairAP(F, pi, r0, nr, x0),
                                         start=(j == 0), stop=False,
                                         perf_mode=DRm)

                Dy = tpool.tile([128, 20, 79], bf16, tag="dy", name=f"dy_{i}")
                nc.vector.tensor_tensor(Dy[:], B[:, m, 2:22, x0:x0 + 79],
                                        B[:, m, 0:20, x0:x0 + 79],
                                        op=AL.subtract)
                Dx = tpool.tile([128, 20, 79], bf16, tag="dx", name=f"dx_{i}")
                nc.vector.tensor_tensor(Dx[:], B[:, m, 1:21, x0 + 1:x0 + 80],
                                        B[:, m, 1:21, x0 - 1:x0 + 78],
                                        op=AL.subtract)
                M1 = tpool.tile([128, 20, 79], bf16, tag="m1", name=f"m1_{i}")
                nc.vector.tensor_tensor(M1[:], Dy[:],
                                        B[:, 1, 1:21, x0:x0 + 79],
                                        op=AL.mult)
                u["M1"] = M1
                M2 = tpool.tile([128, 20, 79], bf16, tag="m2", name=f"m2_{i}")
                nc.gpsimd.tensor_tensor(M2[:], Dx[:],
                                        B[:, 2, 1:21, x0:x0 + 79],
                                        op=AL.mult)
                u["M2"] = M2

                # prefetch next supertile mid-way through this one
                if m == 0 and xh == 1:
                    load_k(k + 1)

            # drain the pipeline
            last = len(units) - 1
            stage_T1(last)
            stage_ids(last)
            stage_sq(last)

            nc.sync.dma_start(out[:], acc[:])
    nc.compile()
    return nc




def build_program_v9b():
    """v8 + split A-drain (bank pairs) so V-fill chunks 0-1 start early,
    ACT queue [drain-a, drain-b, SQ1, SQ2], Dx emitted first so Pool's M2
    starts sooner."""
    from concourse.ap import AP as _AP
    f32 = mybir.dt.float32
    bf16 = mybir.dt.bfloat16
    f8 = mybir.dt.float8e4
    DRm = mybir.MatmulPerfMode.DoubleRow
    AL = mybir.AluOpType
    SQf = mybir.ActivationFunctionType.Square

    nc = bacc.Bacc("TRN2", target_bir_lowering=False, debug=False,
                   num_devices=8)
    slabb = nc.declare_dram_parameter("slabb", [NSUP, 128, 3, YROWS, NX],
                                      bf16, isOutput=False)
    slab8 = nc.declare_dram_parameter("slab8", [NSUP, 128, 4, YROWS, NX],
                                      f8, isOutput=False)
    wp = nc.declare_dram_parameter("wp", [128, 14, 2, 128], f8,
                                   isOutput=False)
    wi = nc.declare_dram_parameter("wi", [128, 128], bf16, isOutput=False)
    out = nc.declare_dram_parameter("out", [128, NSLOT7], f32, isOutput=True)

    units = [(k, m, xh) for k in range(3) for m in range(3)
             for xh in range(2)]

    with tile.TileContext(nc) as tc:
        with (
            tc.tile_pool(name="const", bufs=1) as cpool,
            tc.tile_pool(name="inp", bufs=2) as inpool,
            tc.tile_pool(name="drn", bufs=3) as dpool,
            tc.tile_pool(name="tmp", bufs=3) as tpool,
            tc.tile_pool(name="pv", bufs=2, space=bass.MemorySpace.PSUM) as pv,
        ):
            W = cpool.tile([128, 14, 2, 128], f8, tag="W")
            nc.sync.dma_start(W[:], wp[:])
            WI = cpool.tile([128, 128], bf16, tag="WI")
            nc.sync.dma_start(WI[:], wi[:])
            acc = cpool.tile([128, NSLOT7], f32, tag="acc")
            nc.vector.memset(acc[:], 0.0)

            BF = {}  # k -> (B tile, F tile)

            def load_k(k):
                if k in BF or k >= 3:
                    return
                B = inpool.tile([128, 3, YROWS, NX], bf16, tag="B",
                                name=f"B_{k}")
                nc.sync.dma_start(B[:], slabb[k])
                F = inpool.tile([128, 4, YROWS, NX], f8, tag="F",
                                name=f"F_{k}")
                nc.sync.dma_start(F[:], slab8[k])
                BF[k] = (B, F)

            load_k(0)

            def pairAP(F, pi, r0, nr, x0):
                fp_stride = F[:].ap[0][0]
                (c0, dy0, dx0, _), (c1, dy1, dx1, _) = _PAIRS7[pi]
                s0 = F[:, c0, 1 + r0 + dy0 : 1 + r0 + dy0 + nr,
                       x0 + dx0 : x0 + dx0 + 79]
                s1 = F[:, c1, 1 + r0 + dy1 : 1 + r0 + dy1 + nr,
                       x0 + dx1 : x0 + dx1 + 79]
                return _AP(s0.tensor, s0.offset,
                           [[fp_stride, 128], [s1.offset - s0.offset, 2],
                            [NX, nr], [1, 79]])

            st = {}  # unit index -> state dict

            def stage_T1(j):
                u = st[j]
                T1 = tpool.tile([128, 20, 79], bf16, tag="t1",
                                name=f"t1_{j}")
                nc.vector.tensor_tensor(
                    T1[:], u["A2b"][:, 0:20, :],
                    u["B"][:, 0, 1:21, u["x0"]:u["x0"] + 79], op=AL.mult)
                u["T1"] = T1

            def stage_ids(j):
                u = st[j]
                for fi, Ft in enumerate((u["M1"], u["M2"], u["T1"])):
                    for c, (r0, nr) in enumerate(_CH7):
                        nc.tensor.matmul(u["P"][:, c, : nr * 79], WI[:],
                                         Ft[:, r0 : r0 + nr, :],
                                         start=False, stop=(fi == 2))

            def stage_sq(j):
                u = st[j]
                P, s = u["P"], u["s"]
                Pq = _AP(P[:].tensor, P[:].offset,
                         [[P[:].ap[0][0], 128], [512, 3], [1, 474]])
                sq1 = tpool.tile([128, 3, 474], bf16, tag="sq1",
                                 name=f"sq1_{j}")
                nc.scalar.activation(sq1[:], Pq, SQf,
                                     accum_out=acc[:, s : s + 1])
                sq2 = tpool.tile([128, 158], bf16, tag="sq2",
                                 name=f"sq2_{j}")
                nc.scalar.activation(sq2[0:112], P[0:112, 3, 0:158], SQf,
                                     accum_out=acc[0:112, s + 1 : s + 2])
                del st[j]

            for i, (k, m, xh) in enumerate(units):
                B, F = BF[k]
                x0 = 1 + 79 * xh
                u = {"B": B, "x0": x0,
                     "s": ((k * 3 + m) * 2 + xh) * 2}
                st[i] = u

                # DVE: finish previous unit's T1 first (A2b ready long ago)
                if i - 1 in st:
                    stage_T1(i - 1)

                # PE: A fill
                P = pv.tile([128, 4, 512], f32, tag="pv", name=f"P_{i}")
                u["P"] = P
                for c, (r0, nr) in enumerate(_CH7):
                    nc.tensor.matmul(P[:, c, : nr * 79], W[:, m],
                                     pairAP(F, m, r0, nr, x0),
                                     start=True, stop=True, perf_mode=DRm)
                A2b = dpool.tile([128, 24, 79], bf16, tag="a2b",
                                 name=f"a2b_{i}")
                nc.scalar.copy(A2b[:], P[:, :, 0:474])
                u["A2b"] = A2b

                # PE: previous unit's ids; ACT: previous unit's SQ
                if i - 1 in st:
                    stage_ids(i - 1)
                    stage_sq(i - 1)

                # PE: V fill
                for c, (r0, nr) in enumerate(_CH7):
                    for j, pi in enumerate(_VPAIRS7[m]):
                        nc.tensor.matmul(P[:, c, : nr * 79], W[:, pi],
                                         pairAP(F, pi, r0, nr, x0),
                                         start=(j == 0), stop=False,
                                         perf_mode=DRm)

                # DVE: Dx first so Pool's M2 can start early
                Dx = tpool.tile([128, 20, 79], bf16, tag="dx", name=f"dx_{i}")
                nc.vector.tensor_tensor(Dx[:], B[:, m, 1:21, x0 + 1:x0 + 80],
                                        B[:, m, 1:21, x0 - 1:x0 + 78],
                                        op=AL.subtract)
                M2 = tpool.tile([128, 20, 79], bf16, tag="m2", name=f"m2_{i}")
                nc.gpsimd.tensor_tensor(M2[:], Dx[:],
                                        B[:, 2, 1:21, x0:x0 + 79],
                                        op=AL.mult)
                u["M2"] = M2
                Dy = tpool.tile([128, 20, 79], bf16, tag="dy", name=f"dy_{i}")
                nc.vector.tensor_tensor(Dy[:], B[:, m, 2:22, x0:x0 + 79],
                                        B[:, m, 0:20, x0:x0 + 79],
                                        op=AL.subtract)
                M1 = tpool.tile([128, 20, 79], bf16, tag="m1", name=f"m1_{i}")
                nc.vector.tensor_tensor(M1[:], Dy[:],
                                        B[:, 1, 1:21, x0:x0 + 79],
                                        op=AL.mult)
                u["M1"] = M1

                # prefetch next supertile mid-way through this one
                if m == 0 and xh == 1:
                    load_k(k + 1)

            # drain the pipeline
            last = len(units) - 1
            stage_T1(last)
            stage_ids(last)
            stage_sq(last)

            nc.sync.dma_start(out[:], acc[:])
    nc.compile()
    return nc





# ---------------------------------------------------------------------------
# v11: v8 with single-channel DoubleRow pairs so every DMA is a whole-tile
# transfer (per-channel DRAM params); channel-priority DMA order shrinks the
# startup head without the partial-slice NEFF crash.

_PAIRS11 = [
    # A pairs (m = 0,1,2)
    (0, (0, 0, "D2"), (0, 1, "Z")),
    (1, (0, 0, "D2"), (0, 1, "Z")),
    (2, (0, 0, "D2"), (0, 1, "Z")),
    # V m=0
    (0, (0, 0, "VU2"), (1, 0, "IMU2")),
    (0, (-1, 0, "IMU2"), (0, 1, "IMU2")),
    (0, (0, -1, "IMU2"), (0, 0, "Z")),
    (3, (0, 0, "D2"), (0, 1, "Z")),
    # V m=1
    (1, (0, 0, "VU2"), (1, 0, "IMU2")),
    (1, (-1, 0, "IMU2"), (0, 1, "IMU2")),
    (1, (0, -1, "IMU2"), (0, 0, "Z")),
    (3, (1, 0, "IP2"), (-1, 0, "IM2")),
    # V m=2
    (2, (0, 0, "VU2"), (1, 0, "IMU2")),
    (2, (-1, 0, "IMU2"), (0, 1, "IMU2")),
    (2, (0, -1, "IMU2"), (0, 0, "Z")),
    (3, (0, 1, "IP2"), (0, -1, "IM2")),
]

_VPAIRS11 = {0: [3, 4, 5, 6], 1: [7, 8, 9, 10], 2: [11, 12, 13, 14]}


def _w_bands_v11():
    import ml_dtypes
    D2 = np.zeros((128, 128), dtype=np.float32)
    VU2 = np.zeros((128, 128), dtype=np.float32)
    for p in range(128):
        z = p % ZSUP
        if 1 <= z <= ZINT:
            D2[p + 1, p] = 1.0
            D2[p - 1, p] = -1.0
            VU2[p, p] = 12.0 * MU
            VU2[p + 1, p] = -2.0 * MU
            VU2[p - 1, p] = -2.0 * MU
    eye = np.eye(128, dtype=np.float32)
    mats = {"D2": D2, "VU2": VU2, "IP2": eye, "IM2": -eye,
            "IMU2": -2.0 * MU * eye, "Z": np.zeros((128, 128), np.float32)}
    W = np.zeros((15, 128, 2, 128), dtype=np.float32)
    for pi, (ch, s0, s1) in enumerate(_PAIRS11):
        W[pi, :, 0, :] = mats[s0[2]]
        W[pi, :, 1, :] = mats[s1[2]]
    W = np.transpose(W, (1, 0, 2, 3)).copy()
    return (W.astype(ml_dtypes.float8_e4m3), eye.astype(ml_dtypes.bfloat16))


def build_program_v11():
    from concourse.ap import AP as _AP
    f32 = mybir.dt.float32
    bf16 = mybir.dt.bfloat16
    f8 = mybir.dt.float8e4
    DRm = mybir.MatmulPerfMode.DoubleRow
    AL = mybir.AluOpType
    SQf = mybir.ActivationFunctionType.Square

    nc = bacc.Bacc("TRN2", target_bir_lowering=False, debug=False,
                   num_devices=8)
    dbs = [nc.declare_dram_parameter(f"b{c}", [NSUP, 128, YROWS, NX], bf16,
                                     isOutput=False) for c in range(3)]
    dfs = [nc.declare_dram_parameter(f"f{c}", [NSUP, 128, YROWS, NX], f8,
                                     isOutput=False) for c in range(4)]
    wp = nc.declare_dram_parameter("wp", [128, 15, 2, 128], f8,
                                   isOutput=False)
    wi = nc.declare_dram_parameter("wi", [128, 128], bf16, isOutput=False)
    out = nc.declare_dram_parameter("out", [128, NSLOT7], f32, isOutput=True)

    units = [(k, m, xh) for k in range(3) for m in range(3)
             for xh in range(2)]

    with tile.TileContext(nc) as tc:
        with (
            tc.tile_pool(name="const", bufs=1) as cpool,
            tc.tile_pool(name="inp", bufs=2) as inpool,
            tc.tile_pool(name="drn", bufs=3) as dpool,
            tc.tile_pool(name="tmp", bufs=3) as tpool,
            tc.tile_pool(name="pv", bufs=2, space=bass.MemorySpace.PSUM) as pv,
        ):
            W = cpool.tile([128, 15, 2, 128], f8, tag="W")
            nc.sync.dma_start(W[:], wp[:])
            WI = cpool.tile([128, 128], bf16, tag="WI")
            nc.sync.dma_start(WI[:], wi[:])
            acc = cpool.tile([128, NSLOT7], f32, tag="acc")
            nc.vector.memset(acc[:], 0.0)

            BF = {}  # k -> (list of 3 B tiles, list of 4 F tiles)

            def load_k(k):
                if k in BF or k >= 3:
                    return
                Bs = [inpool.tile([128, YROWS, NX], bf16, tag=f"B{c}",
                                  name=f"B{c}_{k}") for c in range(3)]
                Fs = [inpool.tile([128, YROWS, NX], f8, tag=f"F{c}",
                                  name=f"F{c}_{k}") for c in range(4)]
                # dependency-priority order: whole-tile transfers only
                nc.sync.dma_start(Bs[0][:], dbs[0][k])
                nc.sync.dma_start(Fs[0][:], dfs[0][k])
                nc.sync.dma_start(Bs[2][:], dbs[2][k])
                nc.sync.dma_start(Fs[3][:], dfs[3][k])
                nc.sync.dma_start(Bs[1][:], dbs[1][k])
                nc.sync.dma_start(Fs[1][:], dfs[1][k])
                nc.sync.dma_start(Fs[2][:], dfs[2][k])
                BF[k] = (Bs, Fs)

            load_k(0)

            def pairAP(Fs, pi, r0, nr, x0):
                ch, (dy0, dx0, _), (dy1, dx1, _) = _PAIRS11[pi]
                Ft = Fs[ch]
                fp_stride = Ft[:].ap[0][0]
                s0 = Ft[:, 1 + r0 + dy0 : 1 + r0 + dy0 + nr,
                        x0 + dx0 : x0 + dx0 + 79]
                s1 = Ft[:, 1 + r0 + dy1 : 1 + r0 + dy1 + nr,
                        x0 + dx1 : x0 + dx1 + 79]
                return _AP(s0.tensor, s0.offset,
                           [[fp_stride, 128], [s1.offset - s0.offset, 2],
                            [NX, nr], [1, 79]])

            st = {}

            def stage_T1(j):
                u = st[j]
                T1 = tpool.tile([128, 20, 79], bf16, tag="t1",
                                name=f"t1_{j}")
                nc.vector.tensor_tensor(
                    T1[:], u["A2b"][:, 0:20, :],
                    u["Bs"][0][:, 1:21, u["x0"]:u["x0"] + 79], op=AL.mult)
                u["T1"] = T1

            def stage_ids(j):
                u = st[j]
                for fi, Ft in enumerate((u["M1"], u["M2"], u["T1"])):
                    order = (3, 0, 1, 2) if fi == 2 else (0, 1, 2, 3)
                    for c in order:
                        r0, nr = _CH7[c]
                        nc.tensor.matmul(u["P"][:, c, : nr * 79], WI[:],
                                         Ft[:, r0 : r0 + nr, :],
                                         start=False, stop=(fi == 2))

            def stage_sq(j):
                u = st[j]
                P, s = u["P"], u["s"]
                Pq = _AP(P[:].tensor, P[:].offset,
                         [[P[:].ap[0][0], 128], [512, 3], [1, 474]])
                sq2 = tpool.tile([128, 158], bf16, tag="sq2",
                                 name=f"sq2_{j}")
                nc.scalar.activation(sq2[0:112], P[0:112, 3, 0:158], SQf,
                                     accum_out=acc[0:112, s + 1 : s + 2])
                sq1 = tpool.tile([128, 3, 474], bf16, tag="sq1",
                                 name=f"sq1_{j}")
                nc.scalar.activation(sq1[:], Pq, SQf,
                                     accum_out=acc[:, s : s + 1])
                del st[j]

            for i, (k, m, xh) in enumerate(units):
                Bs, Fs = BF[k]
                x0 = 1 + 79 * xh
                u = {"Bs": Bs, "x0": x0,
                     "s": ((k * 3 + m) * 2 + xh) * 2}
                st[i] = u

                if i - 1 in st:
                    stage_T1(i - 1)

                P = pv.tile([128, 4, 512], f32, tag="pv", name=f"P_{i}")
                u["P"] = P
                for c in (3, 0, 1, 2):
                    r0, nr = _CH7[c]
                    nc.tensor.matmul(P[:, c, : nr * 79], W[:, m],
                                     pairAP(Fs, m, r0, nr, x0),
                                     start=True, stop=True, perf_mode=DRm)
                A2b = dpool.tile([128, 24, 79], bf16, tag="a2b",
                                 name=f"a2b_{i}")
                nc.scalar.copy(A2b[:], P[:, :, 0:474])
                u["A2b"] = A2b

                if i - 1 in st:
                    stage_ids(i - 1)
                    stage_sq(i - 1)

                for c, (r0, nr) in enumerate(_CH7):
                    for j, pi in enumerate(_VPAIRS11[m]):
                        nc.tensor.matmul(P[:, c, : nr * 79], W[:, pi],
                                         pairAP(Fs, pi, r0, nr, x0),
                                         start=(j == 0), stop=False,
                                         perf_mode=DRm)

                Dy = tpool.tile([128, 20, 79], bf16, tag="dy", name=f"dy_{i}")
                nc.vector.tensor_tensor(Dy[:], Bs[m][:, 2:22, x0:x0 + 79],
                                        Bs[m][:, 0:20, x0:x0 + 79],
                                        op=AL.subtract)
                Dx = tpool.tile([128, 20, 79], bf16, tag="dx", name=f"dx_{i}")
                nc.vector.tensor_tensor(Dx[:],
                                        Bs[m][:, 1:21, x0 + 1:x0 + 80],
                                        Bs[m][:, 1:21, x0 - 1:x0 + 78],
                                        op=AL.subtract)
                M1 = tpool.tile([128, 20, 79], bf16, tag="m1", name=f"m1_{i}")
                nc.vector.tensor_tensor(M1[:], Dy[:],
                                        Bs[1][:, 1:21, x0:x0 + 79],
                                        op=AL.mult)
                u["M1"] = M1
                M2 = tpool.tile([128, 20, 79], bf16, tag="m2", name=f"m2_{i}")
                nc.gpsimd.tensor_tensor(M2[:], Dx[:],
                                        Bs[2][:, 1:21, x0:x0 + 79],
                                        op=AL.mult)
                u["M2"] = M2

                if m == 1 and xh == 0:
                    load_k(k + 1)

            last = len(units) - 1
            stage_T1(last)
            stage_ids(last)
            stage_sq(last)

            nc.sync.dma_start(out[:], acc[:])
    nc.compile()
    return nc


def build_program_v12():
    from concourse.ap import AP as _AP
    f32 = mybir.dt.float32
    bf16 = mybir.dt.bfloat16
    f8 = mybir.dt.float8e4
    DRm = mybir.MatmulPerfMode.DoubleRow
    AL = mybir.AluOpType
    SQf = mybir.ActivationFunctionType.Square

    nc = bacc.Bacc("TRN2", target_bir_lowering=False, debug=False,
                   num_devices=8)
    dbs = [nc.declare_dram_parameter(f"b{c}", [NSUP, 128, YROWS, NX], bf16,
                                     isOutput=False) for c in range(3)]
    dfs = [nc.declare_dram_parameter(f"f{c}", [NSUP, 128, YROWS, NX], f8,
                                     isOutput=False) for c in range(4)]
    wp = nc.declare_dram_parameter("wp", [128, 15, 2, 128], f8,
                                   isOutput=False)
    wi = nc.declare_dram_parameter("wi", [128, 128], bf16, isOutput=False)
    out = nc.declare_dram_parameter("out", [128, NSLOT7], f32, isOutput=True)

    units = [(k, m, xh) for k in range(3) for m in range(3)
             for xh in range(2)]

    with tile.TileContext(nc) as tc:
        with (
            tc.tile_pool(name="const", bufs=1) as cpool,
            tc.tile_pool(name="inp", bufs=2) as inpool,
            tc.tile_pool(name="drn", bufs=3) as dpool,
            tc.tile_pool(name="tmp", bufs=3) as tpool,
            tc.tile_pool(name="pv", bufs=2, space=bass.MemorySpace.PSUM) as pv,
        ):
            W = cpool.tile([128, 15, 2, 128], f8, tag="W")
            nc.sync.dma_start(W[:], wp[:])
            WI = cpool.tile([128, 128], bf16, tag="WI")
            nc.sync.dma_start(WI[:], wi[:])
            acc = cpool.tile([128, NSLOT7], f32, tag="acc")
            nc.vector.memset(acc[:], 0.0)

            BF = {}  # k -> (list of 3 B tiles, list of 4 F tiles)

            def load_k(k):
                if k in BF or k >= 3:
                    return
                Bs = [inpool.tile([128, YROWS, NX], bf16, tag=f"B{c}",
                                  name=f"B{c}_{k}") for c in range(3)]
                Fs = [inpool.tile([128, YROWS, NX], f8, tag=f"F{c}",
                                  name=f"F{c}_{k}") for c in range(4)]
                # dependency-priority order: whole-tile transfers only
                nc.sync.dma_start(Bs[0][:], dbs[0][k])
                nc.sync.dma_start(Fs[0][:], dfs[0][k])
                nc.sync.dma_start(Fs[3][:], dfs[3][k])
                nc.sync.dma_start(Bs[1][:], dbs[1][k])
                nc.sync.dma_start(Bs[2][:], dbs[2][k])
                nc.sync.dma_start(Fs[1][:], dfs[1][k])
                nc.sync.dma_start(Fs[2][:], dfs[2][k])
                BF[k] = (Bs, Fs)

            load_k(0)

            def pairAP(Fs, pi, r0, nr, x0):
                ch, (dy0, dx0, _), (dy1, dx1, _) = _PAIRS11[pi]
                Ft = Fs[ch]
                fp_stride = Ft[:].ap[0][0]
                s0 = Ft[:, 1 + r0 + dy0 : 1 + r0 + dy0 + nr,
                        x0 + dx0 : x0 + dx0 + 79]
                s1 = Ft[:, 1 + r0 + dy1 : 1 + r0 + dy1 + nr,
                        x0 + dx1 : x0 + dx1 + 79]
                return _AP(s0.tensor, s0.offset,
                           [[fp_stride, 128], [s1.offset - s0.offset, 2],
                            [NX, nr], [1, 79]])

            st = {}

            def stage_T1(j):
                u = st[j]
                T1 = tpool.tile([128, 20, 79], bf16, tag="t1",
                                name=f"t1_{j}")
                nc.vector.tensor_tensor(
                    T1[:], u["A2b"][:, 0:20, :],
                    u["Bs"][0][:, 1:21, u["x0"]:u["x0"] + 79], op=AL.mult)
                u["T1"] = T1

            def stage_ids(j):
                u = st[j]
                for fi, Ft in enumerate((u["M1"], u["M2"], u["T1"])):
                    for c, (r0, nr) in enumerate(_CH7):
                        nc.tensor.matmul(u["P"][:, c, : nr * 79], WI[:],
                                         Ft[:, r0 : r0 + nr, :],
                                         start=False, stop=(fi == 2))

            def stage_sq(j):
                u = st[j]
                P, s = u["P"], u["s"]
                Pq = _AP(P[:].tensor, P[:].offset,
                         [[P[:].ap[0][0], 128], [512, 3], [1, 474]])
                sq2 = tpool.tile([128, 158], bf16, tag="sq2",
                                 name=f"sq2_{j}")
                nc.scalar.activation(sq2[0:112], P[0:112, 3, 0:158], SQf,
                                     accum_out=acc[0:112, s + 1 : s + 2])
                sq1 = tpool.tile([128, 3, 474], bf16, tag="sq1",
                                 name=f"sq1_{j}")
                nc.scalar.activation(sq1[:], Pq, SQf,
                                     accum_out=acc[:, s : s + 1])
                del st[j]

            for i, (k, m, xh) in enumerate(units):
                Bs, Fs = BF[k]
                x0 = 1 + 79 * xh
                u = {"Bs": Bs, "x0": x0,
                     "s": ((k * 3 + m) * 2 + xh) * 2}
                st[i] = u

                # previous unit's ids + SQ lead this iteration: all their
                # inputs (T1 emitted last iter after the drain) are ready, so
                # the region frees as early as possible
                if i - 1 in st:
                    stage_ids(i - 1)
                    stage_sq(i - 1)

                P = pv.tile([128, 4, 512], f32, tag="pv", name=f"P_{i}")
                u["P"] = P
                for c in (3, 0, 1, 2):
                    r0, nr = _CH7[c]
                    nc.tensor.matmul(P[:, c, : nr * 79], W[:, m],
                                     pairAP(Fs, m, r0, nr, x0),
                                     start=True, stop=True, perf_mode=DRm)
                A2b = dpool.tile([128, 24, 79], bf16, tag="a2b",
                                 name=f"a2b_{i}")
                nc.scalar.copy(A2b[:], P[:, :, 0:474])
                u["A2b"] = A2b

                for c, (r0, nr) in enumerate(_CH7):
                    for j, pi in enumerate(_VPAIRS11[m]):
                        nc.tensor.matmul(P[:, c, : nr * 79], W[:, pi],
                                         pairAP(Fs, pi, r0, nr, x0),
                                         start=(j == 0), stop=False,
                                         perf_mode=DRm)

                Dy = tpool.tile([128, 20, 79], bf16, tag="dy", name=f"dy_{i}")
                nc.vector.tensor_tensor(Dy[:], Bs[m][:, 2:22, x0:x0 + 79],
                                        Bs[m][:, 0:20, x0:x0 + 79],
                                        op=AL.subtract)
                Dx = tpool.tile([128, 20, 79], bf16, tag="dx", name=f"dx_{i}")
                nc.vector.tensor_tensor(Dx[:],
                                        Bs[m][:, 1:21, x0 + 1:x0 + 80],
                                        Bs[m][:, 1:21, x0 - 1:x0 + 78],
                                        op=AL.subtract)
                M1 = tpool.tile([128, 20, 79], bf16, tag="m1", name=f"m1_{i}")
                nc.vector.tensor_tensor(M1[:], Dy[:],
                                        Bs[1][:, 1:21, x0:x0 + 79],
                                        op=AL.mult)
                u["M1"] = M1
                M2 = tpool.tile([128, 20, 79], bf16, tag="m2", name=f"m2_{i}")
                nc.gpsimd.tensor_tensor(M2[:], Dx[:],
                                        Bs[2][:, 1:21, x0:x0 + 79],
                                        op=AL.mult)
                u["M2"] = M2
                stage_T1(i)

                if m == 1 and xh == 0:
                    load_k(k + 1)

            last = len(units) - 1
            stage_ids(last)
            stage_sq(last)

            nc.sync.dma_start(out[:], acc[:])
    nc.compile()
    return nc




def make_zslab(output, b, zc):
    """[4, 44, 162, 160] f32 slab for core (b, zc) from output [2,4,160,...]."""
    slab = np.zeros((4, NZ_SLAB, NY_PAD, NX), dtype=np.float32)
    z0 = 40 * zc
    zn = min(NZ_SLAB, 160 - z0)
    slab[:, :zn, :160, :] = output[b, :, z0 : z0 + zn, :, :]
    return slab


def pack_slab(zslab):
    """Repack [4,44,162,160] -> device layout [4, 3, 128, 22, 160]."""
    out = np.empty((4, NSUP, 128, YROWS, NX), dtype=np.float32)
    for k in range(NSUP):
        zk = zslab[:, 14 * k : 14 * k + 16]          # [4,16,162,160]
        for q in range(NYB):
            out[:, k, 16 * q : 16 * q + 16] = zk[:, :, 20 * q : 20 * q + 22, :]
    return out


def pack_slab_chan(zslab):
    """Repack [4,44,162,160] -> [NSUP, 128, 4, YROWS, NX] (channel inside
    the partition's free dim, one big DMA per supertile)."""
    out = np.empty((NSUP, 128, 4, YROWS, NX), dtype=np.float32)
    for k in range(NSUP):
        zk = zslab[:, 14 * k : 14 * k + 16]          # [4,16,162,160]
        for q in range(NYB):
            # partition p = q*16 + z ; channel axis after partition
            out[k, 16 * q : 16 * q + 16] = np.transpose(
                zk[:, :, 20 * q : 20 * q + 22, :], (1, 0, 2, 3))
    return out


VARIANT = "v11"
_NC_CACHE = {}


_BUILDERS = {"v1": build_program, "v2": build_program_v2,
             "v3": build_program_v3, "v4": build_program_v4,
             "v5": build_program_v5, "v6": build_program_v6,
             "v7": build_program_v7, "v8": build_program_v8,
             "v9": build_program_v9, "v9a": build_program_v9a,
             "v9b": build_program_v9b,
             "v10sq": (lambda: build_program_v10("sqfirst")),
             "v10id": (lambda: build_program_v10("idsfirst")),
             "v11": build_program_v11, "v12": build_program_v12}


def _get_nc():
    if VARIANT not in _NC_CACHE:
        _NC_CACHE[VARIANT] = _BUILDERS[VARIANT]()
    return _NC_CACHE[VARIANT]


def make_in_maps(output):
    import ml_dtypes
    if VARIANT in ("v11", "v12"):
        w8, wi = _w_bands_v11()
        in_maps = []
        for core in range(8):
            b, zc = core // 4, core % 4
            s = pack_slab_chan(make_zslab(output, b, zc))
            im = {"wp": w8, "wi": wi}
            for c in range(3):
                im[f"b{c}"] = s[:, :, c].astype(ml_dtypes.bfloat16).copy()
            for c in range(4):
                im[f"f{c}"] = s[:, :, c].astype(ml_dtypes.float8_e4m3).copy()
            in_maps.append(im)
        return in_maps
    if VARIANT in ("v7", "v8", "v9", "v9a", "v9b", "v10sq", "v10id"):
        w8, wi = _w_bands_v7()
        in_maps = []
        for core in range(8):
            b, zc = core // 4, core % 4
            s = pack_slab_chan(make_zslab(output, b, zc))
            in_maps.append({
                "slabb": s[:, :, 0:3].astype(ml_dtypes.bfloat16).copy(),
                "slab8": s.astype(ml_dtypes.float8_e4m3),
                "wp": w8, "wi": wi})
        return in_maps
    if VARIANT == "v6":
        dmats = _band_matrices_x2()
        in_maps = []
        for core in range(8):
            b, zc = core // 4, core % 4
            s = pack_slab_chan(make_zslab(output, b, zc))
            in_maps.append({"slab": s.astype(ml_dtypes.bfloat16),
                            "dmats": dmats, "zmask": _zmask(zc)})
        return in_maps
    dmats = _band_matrices() if VARIANT == "v1" else _band_matrices_v2()
    in_maps = []
    for core in range(8):
        b, zc = core // 4, core % 4
        s = pack_slab(make_zslab(output, b, zc))
        if VARIANT != "v1":
            s = s.astype(ml_dtypes.bfloat16)
        im = {"slab": s, "dmats": dmats}
        if VARIANT in ("v3", "v4", "v5"):
            im["amask"] = _amask(zc)
        else:
            im["zmask"] = _zmask(zc)
        in_maps.append(im)
    return in_maps


def kernel(output, inp):
    output = np.asarray(output, dtype=np.float32)
    nc = _get_nc()
    res = run_bass_kernel_spmd(nc, make_in_maps(output),
                               core_ids=list(range(8)))
    total = np.float64(0.0)
    if VARIANT in ("v7", "v8", "v9", "v9a", "v9b", "v10sq", "v10id", "v11", "v12"):
        for core, r in enumerate(res.results):
            zc = core % 4
            zm3 = _zmask(zc).astype(np.float64)  # [3, 128]
            o = r["out"].astype(np.float64)      # [128, 36]
            for slot in range(NSLOT7):
                total += (o[:, slot] * zm3[slot // 12]).sum()
        total /= 4.0
    else:
        for r in res.results:
            total += np.float64(r["out"].astype(np.float64).sum())
        if VARIANT == "v6":
            total /= 4.0
    n = 2 * 158 * 158 * 158
    return np.float32(total / n)

